# revision 1
# baseline (speedup 1.0000x reference)
"""Trainium2 Bass kernel for Ernie4.5-VL vision attention (ragged segments).

Contract: kernel(**inputs) takes the FULL unsharded inputs (keyed as in
setup_inputs()) and returns the FULL [S, D] float32 output.

Strategy
--------
All matmuls run on the PE array in float32r (full-rate fp32, ~1.5e-4 rel
err); everything else is fp32. Attention is computed per segment
(block-diagonal, no masks) in a flash-like streaming form that only ever
materializes transposed score tiles:

  qkvT = Wpack @ hidden.T          (dims on partitions, tokens on free)
  RoPE on qT/kT slices (DVE/GPSIMD elementwise)
  v_aug = transpose(vT) with a ones column appended   (PE transposes)
  per (head, segment, 1024-query chunk):
     for each 128-key tile: ST = kT-tile.T @ qT-chunk  (scores^T in PSUM)
                            PT = exp(ST)               (ACT, no max-sub)
                            outT_aug += v_aug.T @ PT   (PSUM accumulate)
     attn_outT = outT_aug[:80] * (1 / outT_aug[80])    (denominator row)
  projT_partial = WprojT_shard.T @ attn_outT           (per-core partial)

Sharding (8 cores, SPMD - one program, per-core data):
  - uniform 4x1024 segments: 2 head-groups x 4 segments (24 MB/core DMA)
  - any other cu_seqlens: 8-way head parallel, every core sees all
    segments (identical program regardless of segment raggedness)
Host does only O(S*D) glue: input transposes/packing, summing the 2 (or
8) per-token partial projections, and the bias adds.
"""

import os
import sys

import numpy as np

H = 16
HD = 80
BLK = 40  # rotate_half half-width
SCALE = HD ** -0.5
N_CORES = 8
D = 1280
NK = D // 128  # contraction tiles for the qkv matmul
ATTN_STRIDE = 96  # head row pitch in the packed attention output
MM_DT_NAME = os.environ.get("KERNEL_MM_DT", "float32r")  # or "float32"
KERNEL_DEBUG = bool(int(os.environ.get("KERNEL_DEBUG", "0")))


def _segments(cu_seqlens, S):
    """Intervals matching reference's searchsorted(cu[1:], i, 'right')."""
    b = np.clip(np.sort(np.asarray(cu_seqlens, dtype=np.int64)[1:5]), 0, S)
    bounds = [0] + list(b) + [S]
    segs = []
    for a, e in zip(bounds[:-1], bounds[1:]):
        if e > a:
            segs.append((int(a), int(e)))
    return segs


def _pack_layout(n_h):
    """Pack per-core qkv dims as 40-row blocks, 3 per 128-row tile (8 pad).

    Each tile holds one v-block at row 0 (PE transpose operands must start
    at a 32-aligned partition) and two q/k blocks at rows 40 and 80.
    Returns pos[(sec, h, half)] = (tile, row) and the number of tiles.
    """
    ntiles = 2 * n_h
    pos = {}
    for h in range(n_h):
        for half in (0, 1):
            pos[("v", h, half)] = (2 * h + half, 0)
    qk = [("q", h, half) for h in range(n_h) for half in (0, 1)]
    qk += [("k", h, half) for h in range(n_h) for half in (0, 1)]
    for j, blk in enumerate(qk):
        pos[blk] = (j // 2, BLK + BLK * (j % 2))
    return pos, ntiles


def _pieces(start, length, tile_rows=128):
    """Split global row range [start, start+length) into per-tile pieces."""
    out = []
    off = 0
    while off < length:
        g = start + off
        t, r = g // tile_rows, g % tile_rows
        n = min(tile_rows - r, length - off)
        out.append((t, r, n, off))
        off += n
    return out


def _proj_k_tiles(n_h):
    rows = ATTN_STRIDE * n_h
    kt = [128] * (rows // 128)
    if rows % 128:
        kt.append(rows % 128)
    return kt


def _build_program(n_h, S_core, segs_local, resident_hidden):
    """Emit the SPMD program. Same structure for every core.

    Engine-AP partition rules on TRN2 (walrus birverifier): compute-engine
    accesses must start at a 32-aligned partition and must not cross a
    64-boundary unless they start on one; cross-partition data movement
    must go through DMA. The layout choices below all follow from this.
    """
    import concourse.mybir as mybir
    import concourse.tile as tile
    from concourse import bacc
    from concourse.masks import make_identity
    from contextlib import ExitStack

    f32 = mybir.dt.float32
    mm_dt = getattr(mybir.dt, MM_DT_NAME)
    AF = mybir.ActivationFunctionType

    k_proj = n_h
    pos, n_mtiles = _pack_layout(n_h)
    dims_pad = n_mtiles * 128
    VW = 97  # v_aug slot width: 80 v dims + 16 zero pad + ones col at 96

    # global key-tile list: (seg_idx, t0, t1)
    t_tiles = []
    for si, (a, e) in enumerate(segs_local):
        t = a
        while t < e:
            t_tiles.append((si, t, min(t + 128, e)))
            t += 128
    n_tt = len(t_tiles)

    nc = bacc.Bacc("TRN2", target_bir_lowering=False, debug=False,
                   enable_asserts=False, num_devices=N_CORES)

    # host supplies hiddenT/wqkvT pre-tiled into 128-partition-major layout
    hiddenT = nc.dram_tensor("hiddenT", [128, NK * S_core], mm_dt,
                             kind="ExternalInput").ap()
    wqkvT = nc.dram_tensor("wqkvT", [128, NK * dims_pad], mm_dt,
                           kind="ExternalInput").ap()
    bias2d = nc.dram_tensor("bias2d", [128, n_mtiles], f32,
                            kind="ExternalInput").ap()
    # cosP/sin2P are host-packed [128, S]: rows 0:40 and 64:104 hold the
    # lo/hi rope coefficients, all other rows zero (zeroes the junk rows
    # of the rotated q/k so the K=104 score matmuls see exact zeros).
    cosP = nc.dram_tensor("cosP", [128, S_core], mm_dt,
                          kind="ExternalInput").ap()
    sin2P = nc.dram_tensor("sin2P", [128, S_core], mm_dt,
                           kind="ExternalInput").ap()
    wprojT = nc.dram_tensor("wprojT", [n_h * HD, D], mm_dt,
                            kind="ExternalInput").ap()
    # per-key-tile v_aug tail init: 16 zero pad cols + ones col (f32r memset
    # fails walrus codegen, so this comes in via DMA)
    vinit = nc.dram_tensor("vinit", [128, n_tt * (VW - HD)], mm_dt,
                           kind="ExternalInput").ap()
    outT = nc.dram_tensor("outT", [D, S_core], f32, kind="ExternalOutput").ap()
    if KERNEL_DEBUG:
        dbg_qkv = nc.dram_tensor("dbg_qkv", [128, n_mtiles * S_core], f32,
                                 kind="ExternalOutput").ap()
        dbg_rot = nc.dram_tensor("dbg_rot", [128, 2 * n_h * S_core], f32,
                                 kind="ExternalOutput").ap()
        dbg_vaug = nc.dram_tensor("dbg_vaug", [128, n_h * n_tt * VW], f32,
                                  kind="ExternalOutput").ap()
        dbg_attn = nc.dram_tensor("dbg_attn", [128, n_h * S_core], f32,
                                  kind="ExternalOutput").ap()

    def r_(ap):
        return ap.bitcast(mm_dt)

    BC = 1024  # psum tile width (2 banks); matmuls stream <=512
    big_chunks = [(c, min(c + BC, S_core)) for c in range(0, S_core, BC)]

    def halves(c0, c1):
        out = []
        q = c0
        while q < c1:
            out.append((q, min(q + 512, c1)))
            q = q + 512
        return out

    with tile.TileContext(nc) as tc, ExitStack() as ctx:
        persist = ctx.enter_context(tc.tile_pool(name="persist", bufs=1))
        ident = persist.tile([128, 128], f32, tag="ident", name="ident")
        make_identity(nc, ident[:])
        bias_sb = persist.tile([128, n_mtiles], f32, tag="bias", name="bias")
        nc.sync.dma_start(bias_sb[:], bias2d[:])

        # PSUM: two 2-bank slots (t0/t1) shared by qkv/scores/proj, two
        # 1-bank slots for v-transposes, one 2-bank slot for PV accumulate
        psum_all_cm = tc.tile_pool(name="psum_all", bufs=1, space="PSUM")
        psum_all = psum_all_cm.__enter__()
        # big pool: qkvT tiles (phases 1-3), slots reused by attn (phases 4-5)
        qkv_pool = ctx.enter_context(tc.tile_pool(name="big", bufs=1))
        qkv_sb = [qkv_pool.tile([128, S_core], mm_dt, tag=f"qkvT{j}",
                                name=f"qkvT{j}") for j in range(n_mtiles)]
        # rope output (rows 0:104 live, 40:64 zeroed via cosP/sin2P pads)
        rot_cm = tc.tile_pool(name="rot", bufs=1)
        rv = rot_cm.__enter__()
        rot_sb = {}
        for h in range(n_h):
            for sec in ("q", "k"):
                rot_sb[(sec, h)] = rv.tile([128, S_core], mm_dt,
                                           tag=f"rot_{sec}{h}",
                                           name=f"rot_{sec}{h}")
        RC = 1024
        rope_cm = tc.tile_pool(name="rope_scr", bufs=2)
        rope_scr = rope_cm.__enter__()

        # ------------ phase 1: qkvT = Wpack @ hidden.T --------------
        with ExitStack() as p1:
            hidden3 = hiddenT.rearrange("p (k s) -> p k s", k=NK)
            w3 = wqkvT.rearrange("p (k m) -> p k m", k=NK)
            if resident_hidden:
                hid_pool = p1.enter_context(tc.tile_pool(name="hid", bufs=1))
                w_pool = p1.enter_context(tc.tile_pool(name="wstream", bufs=3))
                hid_sb = [hid_pool.tile([128, S_core], mm_dt, tag=f"hid{k}",
                                        name=f"hid{k}") for k in range(NK)]
                wj0 = w_pool.tile([128, NK * 128], mm_dt, tag="wj", name="wj")
                nc.sync.dma_start(hid_sb[0][:], hidden3[:, 0, :])
                nc.sync.dma_start(
                    wj0.rearrange("p (k m) -> p k m", k=NK)[:, :, :],
                    w3[:, :, 0:128])
                for k in range(1, NK):
                    nc.sync.dma_start(hid_sb[k][:], hidden3[:, k, :])
                for j in range(n_mtiles):
                    if j == 0:
                        wj = wj0
                    else:
                        wj = w_pool.tile([128, NK * 128], mm_dt, tag="wj",
                                         name="wj")
                        nc.sync.dma_start(
                            wj.rearrange("p (k m) -> p k m", k=NK)[:, :, :],
                            w3[:, :, j * 128:(j + 1) * 128])
                    for (h0, h1) in halves(0, S_core):
                        hw = h1 - h0
                        ps = psum_all.tile([128, 512], f32,
                                           tag=f"t{(h0 // 512) % 2}",
                                           name="qkvp")
                        for k in range(NK):
                            nc.tensor.matmul(
                                ps[:, :hw],
                                r_(wj[:, k * 128:(k + 1) * 128]),
                                r_(hid_sb[k][:, h0:h1]),
                                start=(k == 0), stop=(k == NK - 1))
                        nc.scalar.activation(qkv_sb[j][:, h0:h1], ps[:, :hw],
                                             AF.Identity,
                                             bias=bias_sb[:, j:j + 1])
            else:
                # k-outer streaming: two psum slots hold four j-streams
                # (columns 0:512 and 512:1024), hidden tiles are tiny
                w_pool = p1.enter_context(tc.tile_pool(name="wres", bufs=1))
                w_sb = [w_pool.tile([128, dims_pad], mm_dt, tag=f"w{k}",
                                    name=f"w{k}") for k in range(NK)]
                for k in range(NK):
                    nc.sync.dma_start(w_sb[k][:], w3[:, k, :])
                assert n_mtiles == 4
                hid_pool = p1.enter_context(tc.tile_pool(name="hidstream",
                                                         bufs=3))
                for (h0, h1) in halves(0, S_core):
                    hw = h1 - h0
                    ps01 = psum_all.tile([128, BC], f32, tag="t0", name="ps01")
                    ps23 = psum_all.tile([128, BC], f32, tag="t1", name="ps23")
                    pj_of = lambda j: (ps01 if j < 2 else ps23,
                                       (j % 2) * 512)
                    for k in range(NK):
                        ht = hid_pool.tile([128, 512], mm_dt, tag="hidc",
                                           name="hidc")
                        nc.sync.dma_start(ht[:, :hw], hidden3[:, k, h0:h1])
                        for j in range(n_mtiles):
                            psj, co = pj_of(j)
                            nc.tensor.matmul(
                                psj[:, co:co + hw],
                                r_(w_sb[k][:, j * 128:(j + 1) * 128]),
                                r_(ht[:, :hw]),
                                start=(k == 0), stop=(k == NK - 1))
                    for j in range(n_mtiles):
                        psj, co = pj_of(j)
                        nc.scalar.activation(qkv_sb[j][:, h0:h1],
                                             psj[:, co:co + hw], AF.Identity,
                                             bias=bias_sb[:, j:j + 1])

        psum_all_cm.__exit__(None, None, None)
        ps_att = ctx.enter_context(tc.tile_pool(name="ps_att", bufs=1,
                                                space="PSUM"))

        # ------------ phase 2: RoPE --------------------------------
        # DMA-stage lo/hi into 0:40 / 64:104 (stgA) and swapped (stgB),
        # then rot = stgA*cosP + stgB*sin2P as three same-base wide ops.
        # double-buffered persistent staging tensors; rows 40:64 zeroed once
        # from cosP's zero rows so the [0:104) products read defined zeros
        stg = {}
        for nm in ("sa0", "sa1", "sb0", "sb1"):
            stg[nm] = rope_scr.tile([128, RC], mm_dt, tag=nm, name=nm, bufs=1)
        pair_i = 0
        for ci, f0 in enumerate(range(0, S_core, RC)):
            f1 = min(f0 + RC, S_core)
            fs = f1 - f0
            cos_sb = rope_scr.tile([128, RC], mm_dt, tag="cos", name="cos",
                                   bufs=1)
            sin_sb = rope_scr.tile([128, RC], mm_dt, tag="sin", name="sin",
                                   bufs=1)
            nc.scalar.dma_start(cos_sb[:, :fs], cosP[:, f0:f1])
            nc.scalar.dma_start(sin_sb[:, :fs], sin2P[:, f0:f1])
            if ci == 0:
                for nm in stg:
                    nc.scalar.dma_start(stg[nm][BLK:64, :], cos_sb[BLK:64, :])
            for h in range(n_h):
                for sec in ("q", "k"):
                    lo_t, lo_r = pos[(sec, h, 0)]
                    hi_t, hi_r = pos[(sec, h, 1)]
                    assert hi_t == lo_t and hi_r == lo_r + BLK
                    x = qkv_sb[lo_t]
                    dst = rot_sb[(sec, h)]
                    stga = stg[f"sa{pair_i % 2}"]
                    stgb = stg[f"sb{pair_i % 2}"]
                    nc.scalar.dma_start(stga[0:BLK, :fs],
                                        x[lo_r:lo_r + BLK, f0:f1])
                    nc.scalar.dma_start(stga[64:64 + BLK, :fs],
                                        x[hi_r:hi_r + BLK, f0:f1])
                    nc.scalar.dma_start(stgb[0:BLK, :fs],
                                        x[hi_r:hi_r + BLK, f0:f1])
                    nc.scalar.dma_start(stgb[64:64 + BLK, :fs],
                                        x[lo_r:lo_r + BLK, f0:f1])
                    nc.vector.tensor_mul(dst[0:104, f0:f1], stga[0:104, :fs],
                                         cos_sb[0:104, :fs])
                    eng = nc.gpsimd if pair_i % 2 == 0 else nc.vector
                    eng.tensor_mul(stgb[0:104, :fs], stgb[0:104, :fs],
                                   sin_sb[0:104, :fs])
                    nc.vector.tensor_add(dst[0:104, f0:f1], dst[0:104, f0:f1],
                                         stgb[0:104, :fs])
                    pair_i += 1
        rope_cm.__exit__(None, None, None)

        # v_aug tiles + per-head emitter (invoked right after each head's
        # rope so attention unblocks head by head)
        vaug_cm = tc.tile_pool(name="vaug", bufs=1)
        vaug_pool = vaug_cm.__enter__()
        vaug_sb = [vaug_pool.tile([128, n_tt * VW], mm_dt, tag=f"vaug{h}",
                                  name=f"vaug{h}") for h in range(n_h)]
        vinit3 = vinit.rearrange("p (t c) -> p t c", c=VW - HD)
        for h in range(n_h):
            nc.sync.dma_start(
                vaug_sb[h].rearrange("p (t c) -> p t c", c=VW)[:, :, HD:VW],
                vinit3[:, :, :])
        GRP = 4  # key tiles transposed per psum tile / copy (1 psum bank)

        def emit_vaug(h):
            gi = 0
            while gi < n_tt:
                hi_g = min(gi + GRP, n_tt)
                if all(t_tiles[g][2] - t_tiles[g][1] == 128
                       for g in range(gi, hi_g)):
                    grp = list(range(gi, hi_g))
                else:
                    grp = [gi]
                ng = len(grp)
                tp = ps_att.tile([128, GRP * HD], f32, tag="tp", name="tp")
                for x, g in enumerate(grp):
                    si, t0, t1 = t_tiles[g]
                    sz = t1 - t0
                    for half in (0, 1):
                        vt, vr = pos[("v", h, half)]
                        nc.tensor.transpose(
                            tp[:sz, x * HD + half * BLK:
                               x * HD + (half + 1) * BLK],
                            qkv_sb[vt][0:BLK, t0:t1].bitcast(f32),
                            ident[:BLK, :BLK])
                sz0 = t_tiles[grp[0]][2] - t_tiles[grp[0]][1]
                dst = vaug_sb[h].rearrange("p (t c) -> p t c", c=VW)
                src_ap = tp.rearrange("p (t c) -> p t c", c=HD)
                if h % 2 == 0:
                    nc.vector.tensor_copy(dst[:sz0, grp[0]:grp[0] + ng, 0:HD],
                                          src_ap[:sz0, 0:ng, :])
                else:
                    nc.scalar.activation(dst[:sz0, grp[0]:grp[0] + ng, 0:HD],
                                         src_ap[:sz0, 0:ng, :], AF.Identity)
                gi += ng




        if KERNEL_DEBUG:
            for j in range(n_mtiles):
                nc.sync.dma_start(
                    dbg_qkv[:, j * S_core:(j + 1) * S_core],
                    qkv_sb[j][:].bitcast(f32))
            i_ = 0
            for h in range(n_h):
                for sec in ("q", "k"):
                    nc.sync.dma_start(
                        dbg_rot[:, i_ * S_core:(i_ + 1) * S_core],
                        rot_sb[(sec, h)][:].bitcast(f32))
                    i_ += 1

        # ------------ phase 4: attention ----------------------------
        # one attn tile per head (rows 0:80) so every compute access is
        # partition-0 based; tiles reuse the dead qkvT slots
        attn_sb = [qkv_pool.tile([128, S_core], mm_dt, tag=f"qkvT{h}",
                                 name=f"attnT{h}") for h in range(n_h)]

        seg_ttiles = {}
        for ti, (si, t0, t1) in enumerate(t_tiles):
            seg_ttiles.setdefault(si, []).append((ti, t0, t1))

        BA = 512  # attention query-chunk width (1-bank psum slots)
        with ExitStack() as p4:
            pt_pool = p4.enter_context(tc.tile_pool(name="pt", bufs=3))
            nrm_pool = p4.enter_context(tc.tile_pool(name="nrm", bufs=2))
            unit_box = [0]

            def emit_attention(h, si, a, e):
                qT = rot_sb[("q", h)]
                kT = rot_sb[("k", h)]
                q = a
                while q < e:
                    q0, q1 = q, min(q + BA, e)
                    qs = q1 - q0
                    po = ps_att.tile([128, BA], f32,
                                     tag=f"po{unit_box[0] % 2}", name="pv")
                    tts = seg_ttiles[si]
                    for idx, (ti, t0, t1) in enumerate(tts):
                        sz = t1 - t0
                        ps = ps_att.tile([128, BA], f32, tag=f"st{idx % 2}",
                                         name="st")
                        nc.tensor.matmul(ps[:sz, :qs], r_(kT[0:104, t0:t1]),
                                         r_(qT[0:104, q0:q1]),
                                         start=True, stop=True)
                        pt = pt_pool.tile([128, BA], mm_dt, tag="pt", name="pt")
                        nc.scalar.activation(pt[:sz, :qs], ps[:sz, :qs], AF.Exp)
                        nc.tensor.matmul(
                            po[:VW, :qs],
                            r_(vaug_sb[h][:sz, ti * VW:(ti + 1) * VW]),
                            r_(pt[:sz, :qs]),
                            start=(idx == 0), stop=(idx == len(tts) - 1))
                    # partition_broadcast ucode reads physical partition 0,
                    # so shift the denominator row 96 -> 0 via DMA
                    rc = nrm_pool.tile([128, BA], f32, tag="rc", name="rc")
                    nc.vector.tensor_copy(rc[96:97, :qs], po[96:97, :qs])
                    nc.sync.dma_start(rc[0:1, :qs], rc[96:97, :qs])
                    nc.vector.reciprocal(rc[0:1, :qs], rc[0:1, :qs])
                    bc = nrm_pool.tile([128, BA], mm_dt, tag="bc", name="bc")
                    nc.gpsimd.partition_broadcast(
                        bc[0:HD, :qs], rc[0:1, :qs].bitcast(mm_dt))
                    nc.vector.tensor_mul(attn_sb[h][0:HD, q0:q1],
                                         po[0:HD, :qs], bc[0:HD, :qs])
                    unit_box[0] += 1
                    q = q1

            if len(segs_local) == 1:
                a, e = segs_local[0]
                for h in range(n_h):
                    emit_vaug(h)
                    emit_attention(h, 0, a, e)
            else:
                for h in range(n_h):
                    emit_vaug(h)
                for si, (a, e) in enumerate(segs_local):
                    for h in range(n_h):
                        emit_attention(h, si, a, e)

        vaug_cm.__exit__(None, None, None)
        rot_cm.__exit__(None, None, None)

        # ------------ phase 5: projection partial -------------------
        with ExitStack() as p5:
            wp_pool = p5.enter_context(tc.tile_pool(name="wp", bufs=1))
            wp_sb = []
            for kt in range(k_proj):
                t = wp_pool.tile([HD, D], mm_dt, tag=f"wp{kt}", name=f"wp{kt}")
                nc.sync.dma_start(t[:], wprojT[kt * HD:(kt + 1) * HD, :])
                wp_sb.append(t)
            out_pool = p5.enter_context(tc.tile_pool(name="outsb", bufs=3))
            for (c0, c1) in big_chunks:
                cs = c1 - c0
                for j in range(D // 128):
                    ob = out_pool.tile([128, BC], f32, tag="ob", name="ob")
                    for (h0, h1) in halves(c0, c1):
                        ps = ps_att.tile([128, 512], f32, tag=f"st{j % 2}",
                                         name="pj")
                        for kt in range(k_proj):
                            nc.tensor.matmul(
                                ps[:, :h1 - h0],
                                r_(wp_sb[kt][:, j * 128:(j + 1) * 128]),
                                r_(attn_sb[kt][0:HD, h0:h1]),
                                start=(kt == 0), stop=(kt == k_proj - 1))
                        if j % 2 == 0:
                            nc.vector.tensor_copy(ob[:, h0 - c0:h1 - c0],
                                                  ps[:, :h1 - h0])
                        else:
                            nc.scalar.activation(ob[:, h0 - c0:h1 - c0],
                                                 ps[:, :h1 - h0], AF.Identity)
                    nc.sync.dma_start(outT[j * 128:(j + 1) * 128, c0:c1],
                                      ob[:, :cs])

    nc.compile()
    return nc


def _pack_w(Wqkv, bqkv, heads, n_h):
    """Per-core packed qkv weights (q rows pre-scaled).

    Returns wqkvT_tiled [128, NK*dims_pad] (k-major blocks of [128, dims_pad])
    and bias2d [128, n_mtiles]."""
    pos, n_mtiles = _pack_layout(n_h)
    dims_pad = n_mtiles * 128
    W = np.zeros((dims_pad, D), np.float32)
    b = np.zeros((dims_pad,), np.float32)
    sec_off = {"q": 0, "k": D, "v": 2 * D}
    for i, h in enumerate(heads):
        for sec in ("q", "k", "v"):
            for half in (0, 1):
                t, r = pos[(sec, i, half)]
                src = sec_off[sec] + h * HD + half * BLK
                w = Wqkv[src:src + BLK, :]
                bb = bqkv[src:src + BLK]
                if sec == "q":
                    w = w * SCALE
                    bb = bb * SCALE
                W[t * 128 + r:t * 128 + r + BLK] = w
                b[t * 128 + r:t * 128 + r + BLK] = bb
    w_tiled = _tile_rows(np.ascontiguousarray(W.T))
    bias2d = np.ascontiguousarray(b.reshape(n_mtiles, 128).T)
    return w_tiled, bias2d


def _tile_rows(x):
    """[R, C] with R = nk*128 -> [128, nk*C] k-major tiling."""
    R, C = x.shape
    nk = R // 128
    return np.ascontiguousarray(
        x.reshape(nk, 128, C).transpose(1, 0, 2).reshape(128, nk * C))


def _pack_wproj(Wproj, heads):
    """Rows of Wproj.T for this core's head dims, stacked per head."""
    W = np.zeros((len(heads) * HD, Wproj.shape[0]), np.float32)
    for i, h in enumerate(heads):
        W[i * HD:(i + 1) * HD] = Wproj[:, h * HD:(h + 1) * HD].T
    return W


def _pack_cos_sin(cos, sin):
    """cosP/sin2P [128, S]: lo coeffs at rows 0:40, hi at 64:104, rest 0.

    sin2P row signs match rot = x*cosP + swap(x)*sin2P: lo rows hold
    -sin_lo (they multiply x_hi), hi rows hold +sin_hi (they multiply x_lo).
    """
    S = cos.shape[0]
    cosP = np.zeros((128, S), np.float32)
    sinP = np.zeros((128, S), np.float32)
    cosP[0:BLK] = cos.T[0:BLK]
    cosP[64:64 + BLK] = cos.T[BLK:HD]
    sinP[0:BLK] = -sin.T[0:BLK]
    sinP[64:64 + BLK] = sin.T[BLK:HD]
    return cosP, sinP


_CACHE = {}


def kernel(hidden_states, cos, sin, Wqkv, bqkv, Wproj, bproj, cu_seqlens):
    sys.path.insert(0, "/opt/trn_rl_repo")
    from concourse import bass_utils

    hidden_states = np.asarray(hidden_states, np.float32)
    cos = np.asarray(cos, np.float32)
    sin = np.asarray(sin, np.float32)
    Wqkv = np.asarray(Wqkv, np.float32)
    bqkv = np.asarray(bqkv, np.float32)
    Wproj = np.asarray(Wproj, np.float32)
    bproj = np.asarray(bproj, np.float32)

    S, D_ = hidden_states.shape
    assert D_ == D
    segs = _segments(cu_seqlens, S)
    uniform = (S % 4 == 0) and segs == [(i * S // 4, (i + 1) * S // 4)
                                        for i in range(4)]

    hiddenT = np.ascontiguousarray(hidden_states.T)
    cosP, sin2P = _pack_cos_sin(cos, sin)

    def _vinit(segs_local):
        n_tt = sum(-(-(e - a) // 128) for a, e in segs_local)
        v = np.zeros((128, n_tt, 17), np.float32)
        v[:, :, 16] = 1.0
        return np.ascontiguousarray(v.reshape(128, n_tt * 17))

    if uniform:
        # mode A: 2 head-groups x 4 segments
        n_h, S_core = H // 2, S // 4
        key = ("A", S)
        if key not in _CACHE:
            _CACHE[key] = _build_program(n_h, S_core, [(0, S_core)],
                                         resident_hidden=True)
        nc = _CACHE[key]
        vinit = _vinit([(0, S_core)])
        in_maps = []
        meta = []
        for g in range(2):
            heads = list(range(g * n_h, (g + 1) * n_h))
            wt, b2 = _pack_w(Wqkv, bqkv, heads, n_h)
            wprojT = _pack_wproj(Wproj, heads)
            for s in range(4):
                sl = slice(s * S_core, (s + 1) * S_core)
                in_maps.append({
                    "hiddenT": _tile_rows(hiddenT[:, sl]),
                    "wqkvT": wt,
                    "bias2d": b2,
                    "cosP": np.ascontiguousarray(cosP[:, sl]),
                    "sin2P": np.ascontiguousarray(sin2P[:, sl]),
                    "wprojT": wprojT,
                    "vinit": vinit,
                })
                meta.append((g, s))
        res = bass_utils.run_bass_kernel_spmd(nc, in_maps,
                                              core_ids=list(range(N_CORES)))
        out = np.zeros((D, S), np.float32)
        for c, (g, s) in enumerate(meta):
            out[:, s * S_core:(s + 1) * S_core] += res.results[c]["outT"]
    else:
        # mode C: 8-way head parallel, full sequence per core
        n_h, S_core = H // N_CORES, S
        key = ("C", S, tuple(np.asarray(cu_seqlens).tolist()))
        if key not in _CACHE:
            _CACHE[key] = _build_program(n_h, S_core, segs,
                                         resident_hidden=False)
        nc = _CACHE[key]
        vinit = _vinit(segs)
        hid_tiled = _tile_rows(hiddenT)
        in_maps = []
        for c in range(N_CORES):
            heads = list(range(c * n_h, (c + 1) * n_h))
            wt, b2 = _pack_w(Wqkv, bqkv, heads, n_h)
            in_maps.append({
                "hiddenT": hid_tiled,
                "wqkvT": wt,
                "bias2d": b2,
                "cosP": cosP,
                "sin2P": sin2P,
                "wprojT": _pack_wproj(Wproj, heads),
                "vinit": vinit,
            })
        res = bass_utils.run_bass_kernel_spmd(nc, in_maps,
                                              core_ids=list(range(N_CORES)))
        out = np.zeros((D, S), np.float32)
        for c in range(N_CORES):
            out += res.results[c]["outT"]

    return np.ascontiguousarray(out.T) + bproj[None, :]



# revision 46
# speedup vs baseline: 1.2399x; 1.2399x over previous
"""Trainium2 Bass kernel for Ernie4.5-VL vision attention (ragged segments).

Contract: kernel(**inputs) takes the FULL unsharded inputs (keyed as in
setup_inputs()) and returns the FULL [S, D] float32 output.

Strategy
--------
All matmuls run on the PE array in float32r (full-rate fp32, ~1.5e-4 rel
err); everything else is fp32. Attention is computed per segment
(block-diagonal, no masks) in a flash-like streaming form that only ever
materializes transposed score tiles:

  qkvT = Wpack @ hidden.T          (dims on partitions, tokens on free)
  RoPE on qT/kT slices (DVE/GPSIMD elementwise)
  v_aug = transpose(vT) with a ones column appended   (PE transposes)
  per (head, segment, 1024-query chunk):
     for each 128-key tile: ST = kT-tile.T @ qT-chunk  (scores^T in PSUM)
                            PT = exp(ST)               (ACT, no max-sub)
                            outT_aug += v_aug.T @ PT   (PSUM accumulate)
     attn_outT = outT_aug[:80] * (1 / outT_aug[80])    (denominator row)
  projT_partial = WprojT_shard.T @ attn_outT           (per-core partial)

Sharding (8 cores, SPMD - one program, per-core data):
  - uniform 4x1024 segments: 2 head-groups x 4 segments (24 MB/core DMA)
  - any other cu_seqlens: 8-way head parallel, every core sees all
    segments (identical program regardless of segment raggedness)
Host does only O(S*D) glue: input transposes/packing, summing the 2 (or
8) per-token partial projections, and the bias adds.
"""

import os
import sys

import numpy as np

H = 16
HD = 80
BLK = 40  # rotate_half half-width
SCALE = HD ** -0.5
N_CORES = 8
D = 1280
NK = D // 128  # contraction tiles for the qkv matmul
ATTN_STRIDE = 96  # head row pitch in the packed attention output
MM_DT_NAME = os.environ.get("KERNEL_MM_DT", "float32r")  # or "float32"
KERNEL_DEBUG = bool(int(os.environ.get("KERNEL_DEBUG", "0")))


def _segments(cu_seqlens, S):
    """Intervals matching reference's searchsorted(cu[1:], i, 'right')."""
    b = np.clip(np.sort(np.asarray(cu_seqlens, dtype=np.int64)[1:5]), 0, S)
    bounds = [0] + list(b) + [S]
    segs = []
    for a, e in zip(bounds[:-1], bounds[1:]):
        if e > a:
            segs.append((int(a), int(e)))
    return segs


def _pack_layout(n_h):
    """Pack per-core qkv dims as 40-row blocks, 3 per 128-row tile (8 pad).

    Each tile holds one v-block at row 0 (PE transpose operands must start
    at a 32-aligned partition) and two q/k blocks at rows 40 and 80.
    Returns pos[(sec, h, half)] = (tile, row) and the number of tiles.
    """
    ntiles = 2 * n_h
    pos = {}
    for h in range(n_h):
        for half in (0, 1):
            pos[("v", h, half)] = (2 * h + half, 0)
    qk = [("q", h, half) for h in range(n_h) for half in (0, 1)]
    qk += [("k", h, half) for h in range(n_h) for half in (0, 1)]
    for j, blk in enumerate(qk):
        pos[blk] = (j // 2, BLK + BLK * (j % 2))
    return pos, ntiles


def _pieces(start, length, tile_rows=128):
    """Split global row range [start, start+length) into per-tile pieces."""
    out = []
    off = 0
    while off < length:
        g = start + off
        t, r = g // tile_rows, g % tile_rows
        n = min(tile_rows - r, length - off)
        out.append((t, r, n, off))
        off += n
    return out


def _proj_k_tiles(n_h):
    rows = ATTN_STRIDE * n_h
    kt = [128] * (rows // 128)
    if rows % 128:
        kt.append(rows % 128)
    return kt


def _build_program(n_h, S_core, segs_local, resident_hidden):
    """Emit the SPMD program. Same structure for every core.

    Engine-AP partition rules on TRN2 (walrus birverifier): compute-engine
    accesses must start at a 32-aligned partition and must not cross a
    64-boundary unless they start on one; cross-partition data movement
    must go through DMA. The layout choices below all follow from this.
    """
    import concourse.mybir as mybir
    import concourse.tile as tile
    from concourse import bacc
    from concourse.masks import make_identity
    from contextlib import ExitStack

    f32 = mybir.dt.float32
    mm_dt = getattr(mybir.dt, MM_DT_NAME)
    AF = mybir.ActivationFunctionType

    k_proj = n_h
    pos, n_mtiles = _pack_layout(n_h)
    dims_pad = n_mtiles * 128
    VW = 97  # v_aug slot width: 80 v dims + 16 zero pad + ones col at 96

    # global key-tile list: (seg_idx, t0, t1)
    t_tiles = []
    for si, (a, e) in enumerate(segs_local):
        t = a
        while t < e:
            t_tiles.append((si, t, min(t + 128, e)))
            t += 128
    n_tt = len(t_tiles)

    nc = bacc.Bacc("TRN2", target_bir_lowering=False, debug=False,
                   enable_asserts=False, num_devices=N_CORES)

    # host supplies hiddenT/wqkvT pre-tiled into 128-partition-major layout
    hiddenT = nc.dram_tensor("hiddenT", [128, NK * S_core], mm_dt,
                             kind="ExternalInput").ap()
    wqkvT = nc.dram_tensor("wqkvT", [128, NK * dims_pad], mm_dt,
                           kind="ExternalInput").ap()
    bias2d = nc.dram_tensor("bias2d", [128, n_mtiles], f32,
                            kind="ExternalInput").ap()
    # cosP/sin2P are host-packed [128, S]: rows 0:40 and 64:104 hold the
    # lo/hi rope coefficients, all other rows zero (zeroes the junk rows
    # of the rotated q/k so the K=104 score matmuls see exact zeros).
    cosP = nc.dram_tensor("cosP", [128, S_core], mm_dt,
                          kind="ExternalInput").ap()
    sin2P = nc.dram_tensor("sin2P", [128, S_core], mm_dt,
                           kind="ExternalInput").ap()
    wprojT = nc.dram_tensor("wprojT", [n_h * HD, D], mm_dt,
                            kind="ExternalInput").ap()
    # per-key-tile v_aug tail init: 16 zero pad cols + ones col (f32r memset
    # fails walrus codegen, so this comes in via DMA)
    vinit = nc.dram_tensor("vinit", [128, n_tt * (VW - HD)], mm_dt,
                           kind="ExternalInput").ap()
    outT = nc.dram_tensor("outT", [D, S_core], f32, kind="ExternalOutput").ap()
    if KERNEL_DEBUG:
        dbg_qkv = nc.dram_tensor("dbg_qkv", [128, n_mtiles * S_core], f32,
                                 kind="ExternalOutput").ap()
        dbg_rot = nc.dram_tensor("dbg_rot", [128, 2 * n_h * S_core], f32,
                                 kind="ExternalOutput").ap()
        dbg_vaug = nc.dram_tensor("dbg_vaug", [128, n_h * n_tt * VW], f32,
                                  kind="ExternalOutput").ap()
        dbg_attn = nc.dram_tensor("dbg_attn", [128, n_h * S_core], f32,
                                  kind="ExternalOutput").ap()

    def r_(ap):
        return ap.bitcast(mm_dt)

    BC = 1024  # psum tile width (2 banks); matmuls stream <=512
    big_chunks = [(c, min(c + BC, S_core)) for c in range(0, S_core, BC)]

    def halves(c0, c1):
        out = []
        q = c0
        while q < c1:
            out.append((q, min(q + 512, c1)))
            q = q + 512
        return out

    with tile.TileContext(nc) as tc, ExitStack() as ctx:
        persist = ctx.enter_context(tc.tile_pool(name="persist", bufs=1))
        ident = persist.tile([128, 128], f32, tag="ident", name="ident")
        make_identity(nc, ident[:])
        bias_sb = persist.tile([128, n_mtiles], f32, tag="bias", name="bias")
        nc.sync.dma_start(bias_sb[:], bias2d[:])

        # PSUM: two 2-bank slots (t0/t1) shared by qkv/scores/proj, two
        # 1-bank slots for v-transposes, one 2-bank slot for PV accumulate
        psum_all_cm = tc.tile_pool(name="psum_all", bufs=1, space="PSUM")
        psum_all = psum_all_cm.__enter__()
        # big pool: qkvT tiles (phases 1-3), slots reused by attn (phases 4-5)
        qkv_pool = ctx.enter_context(tc.tile_pool(name="big", bufs=1))
        qkv_sb = [qkv_pool.tile([128, S_core], mm_dt, tag=f"qkvT{j}",
                                name=f"qkvT{j}") for j in range(n_mtiles)]
        # rope output (rows 0:104 live, 40:64 zeroed via cosP/sin2P pads)
        rot_cm = tc.tile_pool(name="rot", bufs=1)
        rv = rot_cm.__enter__()
        rot_sb = {}
        for h in range(n_h):
            for sec in ("q", "k"):
                rot_sb[(sec, h)] = rv.tile([128, S_core], mm_dt,
                                           tag=f"rot_{sec}{h}",
                                           name=f"rot_{sec}{h}")
        RC = 1024
        rope_cm = tc.tile_pool(name="rope_scr", bufs=2)
        rope_scr = rope_cm.__enter__()

        # ------------ phase 1: qkvT = Wpack @ hidden.T --------------
        with ExitStack() as p1:
            hidden3 = hiddenT.rearrange("p (k s) -> p k s", k=NK)
            w3 = wqkvT.rearrange("p (k m) -> p k m", k=NK)
            if resident_hidden:
                hid_pool = p1.enter_context(tc.tile_pool(name="hid", bufs=1))
                w_pool = p1.enter_context(tc.tile_pool(name="wstream", bufs=3))
                hid_sb = [hid_pool.tile([128, S_core], mm_dt, tag=f"hid{k}",
                                        name=f"hid{k}") for k in range(NK)]
                wj0 = w_pool.tile([128, NK * 128], mm_dt, tag="wj", name="wj")
                nc.sync.dma_start(hid_sb[0][:], hidden3[:, 0, :])
                nc.sync.dma_start(
                    wj0.rearrange("p (k m) -> p k m", k=NK)[:, :, :],
                    w3[:, :, 0:128])
                for k in range(1, NK):
                    nc.sync.dma_start(hid_sb[k][:], hidden3[:, k, :])
                for j in range(n_mtiles):
                    if j == 0:
                        wj = wj0
                    else:
                        wj = w_pool.tile([128, NK * 128], mm_dt, tag="wj",
                                         name="wj")
                        nc.sync.dma_start(
                            wj.rearrange("p (k m) -> p k m", k=NK)[:, :, :],
                            w3[:, :, j * 128:(j + 1) * 128])
                    for (h0, h1) in halves(0, S_core):
                        hw = h1 - h0
                        ps = psum_all.tile([128, 512], f32,
                                           tag=f"t{(h0 // 512) % 2}",
                                           name="qkvp")
                        for k in range(NK):
                            nc.tensor.matmul(
                                ps[:, :hw],
                                r_(wj[:, k * 128:(k + 1) * 128]),
                                r_(hid_sb[k][:, h0:h1]),
                                start=(k == 0), stop=(k == NK - 1))
                        nc.scalar.activation(qkv_sb[j][:, h0:h1], ps[:, :hw],
                                             AF.Identity,
                                             bias=bias_sb[:, j:j + 1])
            else:
                # k-outer streaming: two psum slots hold four j-streams
                # (columns 0:512 and 512:1024), hidden tiles are tiny
                w_pool = p1.enter_context(tc.tile_pool(name="wres", bufs=1))
                w_sb = [w_pool.tile([128, dims_pad], mm_dt, tag=f"w{k}",
                                    name=f"w{k}") for k in range(NK)]
                for k in range(NK):
                    nc.sync.dma_start(w_sb[k][:], w3[:, k, :])
                assert n_mtiles == 4
                hid_pool = p1.enter_context(tc.tile_pool(name="hidstream",
                                                         bufs=3))
                for (h0, h1) in halves(0, S_core):
                    hw = h1 - h0
                    ps01 = psum_all.tile([128, BC], f32, tag="t0", name="ps01")
                    ps23 = psum_all.tile([128, BC], f32, tag="t1", name="ps23")
                    pj_of = lambda j: (ps01 if j < 2 else ps23,
                                       (j % 2) * 512)
                    for k in range(NK):
                        ht = hid_pool.tile([128, 512], mm_dt, tag="hidc",
                                           name="hidc")
                        nc.sync.dma_start(ht[:, :hw], hidden3[:, k, h0:h1])
                        for j in range(n_mtiles):
                            psj, co = pj_of(j)
                            nc.tensor.matmul(
                                psj[:, co:co + hw],
                                r_(w_sb[k][:, j * 128:(j + 1) * 128]),
                                r_(ht[:, :hw]),
                                start=(k == 0), stop=(k == NK - 1))
                    for j in range(n_mtiles):
                        psj, co = pj_of(j)
                        nc.scalar.activation(qkv_sb[j][:, h0:h1],
                                             psj[:, co:co + hw], AF.Identity,
                                             bias=bias_sb[:, j:j + 1])

        psum_all_cm.__exit__(None, None, None)
        ps_att = ctx.enter_context(tc.tile_pool(name="ps_att", bufs=1,
                                                space="PSUM"))

        # ------------ phase 2: RoPE --------------------------------
        # DMA-stage lo/hi into 0:40 / 64:104 (stgA) and swapped (stgB),
        # then rot = stgA*cosP + stgB*sin2P as three same-base wide ops.
        # double-buffered persistent staging tensors; rows 40:64 zeroed once
        # from cosP's zero rows so the [0:104) products read defined zeros
        stg = {}
        for nm in ("sa0", "sa1", "sb0", "sb1"):
            stg[nm] = rope_scr.tile([128, RC], mm_dt, tag=nm, name=nm, bufs=1)
        pair_i = 0
        for ci, f0 in enumerate(range(0, S_core, RC)):
            f1 = min(f0 + RC, S_core)
            fs = f1 - f0
            cos_sb = rope_scr.tile([128, RC], mm_dt, tag="cos", name="cos",
                                   bufs=1)
            sin_sb = rope_scr.tile([128, RC], mm_dt, tag="sin", name="sin",
                                   bufs=1)
            nc.scalar.dma_start(cos_sb[:, :fs], cosP[:, f0:f1])
            nc.scalar.dma_start(sin_sb[:, :fs], sin2P[:, f0:f1])
            if ci == 0:
                for nm in stg:
                    nc.scalar.dma_start(stg[nm][BLK:64, :], cos_sb[BLK:64, :])
            for h in range(n_h):
                for sec in ("q", "k"):
                    lo_t, lo_r = pos[(sec, h, 0)]
                    hi_t, hi_r = pos[(sec, h, 1)]
                    assert hi_t == lo_t and hi_r == lo_r + BLK
                    x = qkv_sb[lo_t]
                    dst = rot_sb[(sec, h)]
                    stga = stg[f"sa{pair_i % 2}"]
                    stgb = stg[f"sb{pair_i % 2}"]
                    nc.scalar.dma_start(stga[0:BLK, :fs],
                                        x[lo_r:lo_r + BLK, f0:f1])
                    nc.scalar.dma_start(stga[64:64 + BLK, :fs],
                                        x[hi_r:hi_r + BLK, f0:f1])
                    nc.scalar.dma_start(stgb[0:BLK, :fs],
                                        x[hi_r:hi_r + BLK, f0:f1])
                    nc.scalar.dma_start(stgb[64:64 + BLK, :fs],
                                        x[lo_r:lo_r + BLK, f0:f1])
                    nc.vector.tensor_mul(dst[0:104, f0:f1], stga[0:104, :fs],
                                         cos_sb[0:104, :fs])
                    eng = nc.gpsimd if pair_i % 2 == 0 else nc.vector
                    eng.tensor_mul(stgb[0:104, :fs], stgb[0:104, :fs],
                                   sin_sb[0:104, :fs])
                    nc.vector.tensor_add(dst[0:104, f0:f1], dst[0:104, f0:f1],
                                         stgb[0:104, :fs])
                    pair_i += 1
        rope_cm.__exit__(None, None, None)

        # v_aug tiles + per-head emitter (invoked right after each head's
        # rope so attention unblocks head by head)
        vaug_cm = tc.tile_pool(name="vaug", bufs=1)
        vaug_pool = vaug_cm.__enter__()
        vaug_sb = [vaug_pool.tile([128, n_tt * VW], mm_dt, tag=f"vaug{h}",
                                  name=f"vaug{h}") for h in range(n_h)]
        vinit3 = vinit.rearrange("p (t c) -> p t c", c=VW - HD)
        for h in range(n_h):
            nc.sync.dma_start(
                vaug_sb[h].rearrange("p (t c) -> p t c", c=VW)[:, :, HD:VW],
                vinit3[:, :, :])
        GRP = 4  # key tiles transposed per psum tile / copy (1 psum bank)

        def emit_vaug(h):
            gi = 0
            while gi < n_tt:
                hi_g = min(gi + GRP, n_tt)
                if all(t_tiles[g][2] - t_tiles[g][1] == 128
                       for g in range(gi, hi_g)):
                    grp = list(range(gi, hi_g))
                else:
                    grp = [gi]
                ng = len(grp)
                tp = ps_att.tile([128, GRP * HD], f32, tag="tp", name="tp")
                for x, g in enumerate(grp):
                    si, t0, t1 = t_tiles[g]
                    sz = t1 - t0
                    for half in (0, 1):
                        vt, vr = pos[("v", h, half)]
                        nc.tensor.transpose(
                            tp[:sz, x * HD + half * BLK:
                               x * HD + (half + 1) * BLK],
                            qkv_sb[vt][0:BLK, t0:t1].bitcast(f32),
                            ident[:BLK, :BLK])
                sz0 = t_tiles[grp[0]][2] - t_tiles[grp[0]][1]
                dst = vaug_sb[h].rearrange("p (t c) -> p t c", c=VW)
                src_ap = tp.rearrange("p (t c) -> p t c", c=HD)
                if h % 2 == 0:
                    nc.vector.tensor_copy(dst[:sz0, grp[0]:grp[0] + ng, 0:HD],
                                          src_ap[:sz0, 0:ng, :])
                else:
                    nc.scalar.activation(dst[:sz0, grp[0]:grp[0] + ng, 0:HD],
                                         src_ap[:sz0, 0:ng, :], AF.Identity)
                gi += ng




        if KERNEL_DEBUG:
            for j in range(n_mtiles):
                nc.sync.dma_start(
                    dbg_qkv[:, j * S_core:(j + 1) * S_core],
                    qkv_sb[j][:].bitcast(f32))
            i_ = 0
            for h in range(n_h):
                for sec in ("q", "k"):
                    nc.sync.dma_start(
                        dbg_rot[:, i_ * S_core:(i_ + 1) * S_core],
                        rot_sb[(sec, h)][:].bitcast(f32))
                    i_ += 1

        # ------------ phase 4: attention ----------------------------
        # one attn tile per head (rows 0:80) so every compute access is
        # partition-0 based; tiles reuse the dead qkvT slots
        attn_sb = [qkv_pool.tile([128, S_core], mm_dt, tag=f"qkvT{h}",
                                 name=f"attnT{h}") for h in range(n_h)]

        seg_ttiles = {}
        for ti, (si, t0, t1) in enumerate(t_tiles):
            seg_ttiles.setdefault(si, []).append((ti, t0, t1))

        BA = 512  # attention query-chunk width (1-bank psum slots)
        with ExitStack() as p4:
            pt_pool = p4.enter_context(tc.tile_pool(name="pt", bufs=3))
            nrm_pool = p4.enter_context(tc.tile_pool(name="nrm", bufs=2))
            unit_box = [0]

            def emit_attention(h, si, a, e):
                qT = rot_sb[("q", h)]
                kT = rot_sb[("k", h)]
                q = a
                while q < e:
                    q0, q1 = q, min(q + BA, e)
                    qs = q1 - q0
                    po = ps_att.tile([128, BA], f32,
                                     tag=f"po{unit_box[0] % 2}", name="pv")
                    tts = seg_ttiles[si]
                    for idx, (ti, t0, t1) in enumerate(tts):
                        sz = t1 - t0
                        ps = ps_att.tile([128, BA], f32, tag=f"st{idx % 2}",
                                         name="st")
                        nc.tensor.matmul(ps[:sz, :qs], r_(kT[0:104, t0:t1]),
                                         r_(qT[0:104, q0:q1]),
                                         start=True, stop=True)
                        pt = pt_pool.tile([128, BA], mm_dt, tag="pt", name="pt")
                        nc.scalar.activation(pt[:sz, :qs], ps[:sz, :qs], AF.Exp)
                        nc.tensor.matmul(
                            po[:VW, :qs],
                            r_(vaug_sb[h][:sz, ti * VW:(ti + 1) * VW]),
                            r_(pt[:sz, :qs]),
                            start=(idx == 0), stop=(idx == len(tts) - 1))
                    # partition_broadcast ucode reads physical partition 0,
                    # so shift the denominator row 96 -> 0 via DMA
                    rc = nrm_pool.tile([128, BA], f32, tag="rc", name="rc")
                    nc.vector.tensor_copy(rc[96:97, :qs], po[96:97, :qs])
                    nc.sync.dma_start(rc[0:1, :qs], rc[96:97, :qs])
                    nc.vector.reciprocal(rc[0:1, :qs], rc[0:1, :qs])
                    bc = nrm_pool.tile([128, BA], mm_dt, tag="bc", name="bc")
                    nc.gpsimd.partition_broadcast(
                        bc[0:HD, :qs], rc[0:1, :qs].bitcast(mm_dt))
                    nc.vector.tensor_mul(attn_sb[h][0:HD, q0:q1],
                                         po[0:HD, :qs], bc[0:HD, :qs])
                    unit_box[0] += 1
                    q = q1

            if len(segs_local) == 1:
                a, e = segs_local[0]
                for h in range(n_h):
                    emit_vaug(h)
                    emit_attention(h, 0, a, e)
            else:
                for h in range(n_h):
                    emit_vaug(h)
                for si, (a, e) in enumerate(segs_local):
                    for h in range(n_h):
                        emit_attention(h, si, a, e)

        vaug_cm.__exit__(None, None, None)
        rot_cm.__exit__(None, None, None)

        # ------------ phase 5: projection partial -------------------
        with ExitStack() as p5:
            wp_pool = p5.enter_context(tc.tile_pool(name="wp", bufs=1))
            wp_sb = []
            for kt in range(k_proj):
                t = wp_pool.tile([HD, D], mm_dt, tag=f"wp{kt}", name=f"wp{kt}")
                nc.sync.dma_start(t[:], wprojT[kt * HD:(kt + 1) * HD, :])
                wp_sb.append(t)
            out_pool = p5.enter_context(tc.tile_pool(name="outsb", bufs=3))
            for (c0, c1) in big_chunks:
                cs = c1 - c0
                for j in range(D // 128):
                    ob = out_pool.tile([128, BC], f32, tag="ob", name="ob")
                    for (h0, h1) in halves(c0, c1):
                        ps = ps_att.tile([128, 512], f32, tag=f"st{j % 2}",
                                         name="pj")
                        for kt in range(k_proj):
                            nc.tensor.matmul(
                                ps[:, :h1 - h0],
                                r_(wp_sb[kt][:, j * 128:(j + 1) * 128]),
                                r_(attn_sb[kt][0:HD, h0:h1]),
                                start=(kt == 0), stop=(kt == k_proj - 1))
                        if j % 2 == 0:
                            nc.vector.tensor_copy(ob[:, h0 - c0:h1 - c0],
                                                  ps[:, :h1 - h0])
                        else:
                            nc.scalar.activation(ob[:, h0 - c0:h1 - c0],
                                                 ps[:, :h1 - h0], AF.Identity)
                    nc.sync.dma_start(outT[j * 128:(j + 1) * 128, c0:c1],
                                      ob[:, :cs])

    nc.compile()
    return nc


def _pack_w(Wqkv, bqkv, heads, n_h):
    """Per-core packed qkv weights (q rows pre-scaled).

    Returns wqkvT_tiled [128, NK*dims_pad] (k-major blocks of [128, dims_pad])
    and bias2d [128, n_mtiles]."""
    pos, n_mtiles = _pack_layout(n_h)
    dims_pad = n_mtiles * 128
    W = np.zeros((dims_pad, D), np.float32)
    b = np.zeros((dims_pad,), np.float32)
    sec_off = {"q": 0, "k": D, "v": 2 * D}
    for i, h in enumerate(heads):
        for sec in ("q", "k", "v"):
            for half in (0, 1):
                t, r = pos[(sec, i, half)]
                src = sec_off[sec] + h * HD + half * BLK
                w = Wqkv[src:src + BLK, :]
                bb = bqkv[src:src + BLK]
                if sec == "q":
                    w = w * SCALE
                    bb = bb * SCALE
                W[t * 128 + r:t * 128 + r + BLK] = w
                b[t * 128 + r:t * 128 + r + BLK] = bb
    w_tiled = _tile_rows(np.ascontiguousarray(W.T))
    bias2d = np.ascontiguousarray(b.reshape(n_mtiles, 128).T)
    return w_tiled, bias2d


def _tile_rows(x):
    """[R, C] with R = nk*128 -> [128, nk*C] k-major tiling."""
    R, C = x.shape
    nk = R // 128
    return np.ascontiguousarray(
        x.reshape(nk, 128, C).transpose(1, 0, 2).reshape(128, nk * C))


def _pack_wproj(Wproj, heads):
    """Rows of Wproj.T for this core's head dims, stacked per head."""
    W = np.zeros((len(heads) * HD, Wproj.shape[0]), np.float32)
    for i, h in enumerate(heads):
        W[i * HD:(i + 1) * HD] = Wproj[:, h * HD:(h + 1) * HD].T
    return W


def _pack_cos_sin(cos, sin):
    """cosP/sin2P [128, S]: lo coeffs at rows 0:40, hi at 64:104, rest 0.

    sin2P row signs match rot = x*cosP + swap(x)*sin2P: lo rows hold
    -sin_lo (they multiply x_hi), hi rows hold +sin_hi (they multiply x_lo).
    """
    S = cos.shape[0]
    cosP = np.zeros((128, S), np.float32)
    sinP = np.zeros((128, S), np.float32)
    cosP[0:BLK] = cos.T[0:BLK]
    cosP[64:64 + BLK] = cos.T[BLK:HD]
    sinP[0:BLK] = -sin.T[0:BLK]
    sinP[64:64 + BLK] = sin.T[BLK:HD]
    return cosP, sinP


_CACHE = {}

# ---------------------------------------------------------------------------
# v3 fast path (uniform 4x1024 segments): 2 head-groups x 4 segments SPMD.
#
# Per core: 8 heads, 1024 tokens, one segment. All on-chip data bf16 except
# PSUM (f32) and the normalization scalars (f32).
#   - q/k packed dense: 32 40-row halves, 3 per 128-row tile (11 j-tiles)
#   - v computed untransposed ([tokens, vdim]) straight into PV operand slots
#   - RoPE: DMA-stage [lo;hi]/[hi;lo] into dense [0:80] layouts, 3 DVE ops
#     at bf16 2x rate; scores contract K=80 (no zero padding rows)
#   - scores^T per (head, key-tile) into [128,1024] PSUM, one wide exp
#   - PV with ones column at slot col 96 -> denominators at PSUM row 96
#   - projection over 5 dense 128-row K-tiles (attn heads re-packed via DMA)
# ---------------------------------------------------------------------------

V3_S = 1024     # tokens per core
V3_NH = 8       # heads per core
V3_NJ = 11      # dense qk j-tiles (3 x 40-row halves each, 8 junk rows)
V3_NK = 10      # contraction tiles (D / 128)
V3_NT = 8       # token/key tiles (S / 128)
V3_VW = 97      # v slot: 80 v dims + 16 zero pad + ones col at 96
V3_NPJ = 5      # dense proj k-tiles (8 heads * 80 / 128)


def _half_pos(m):
    """Packed position of 40-row half m: (j_tile, row in {0, 40, 80})."""
    return m // 3, 40 * (m % 3)


def _build_v3():
    import concourse.mybir as mybir
    import concourse.tile as tile
    from concourse import bacc
    from contextlib import ExitStack

    f32 = mybir.dt.float32
    bf16 = mybir.dt.bfloat16
    AF = mybir.ActivationFunctionType
    S, NH, NJ, NK, NT, VW = V3_S, V3_NH, V3_NJ, V3_NK, V3_NT, V3_VW

    nc = bacc.Bacc("TRN2", target_bir_lowering=False, debug=False,
                   enable_asserts=False, num_devices=N_CORES)

    hidT = nc.dram_tensor("hidT", [128, NK * S], bf16,
                          kind="ExternalInput").ap()
    wqkT = nc.dram_tensor("wqkT", [128, NJ * NK * 128], bf16,
                          kind="ExternalInput").ap()
    bias2d = nc.dram_tensor("bias2d", [128, NJ], f32,
                            kind="ExternalInput").ap()
    wvT = nc.dram_tensor("wvT", [128, NK * 640], bf16,
                         kind="ExternalInput").ap()
    vtmpl = nc.dram_tensor("vtmpl", [128, NH * VW], bf16,
                           kind="ExternalInput").ap()
    cosP = nc.dram_tensor("cosP", [128, S], bf16, kind="ExternalInput").ap()
    sinP = nc.dram_tensor("sinP", [128, S], bf16, kind="ExternalInput").ap()
    wprojT = nc.dram_tensor("wprojT", [128, V3_NPJ * D], bf16,
                            kind="ExternalInput").ap()
    outT = nc.dram_tensor("outT", [D, S], bf16, kind="ExternalOutput").ap()
    if KERNEL_DEBUG:
        dbg_qk = nc.dram_tensor("dbg_qk", [128, NJ * S], bf16,
                                kind="ExternalOutput").ap()
        dbg_rot = nc.dram_tensor("dbg_rot", [128, 16 * S], bf16,
                                 kind="ExternalOutput").ap()
        dbg_vaug = nc.dram_tensor("dbg_vaug", [128, NT * NH * VW], bf16,
                                  kind="ExternalOutput").ap()
        dbg_attn = nc.dram_tensor("dbg_attn", [128, NH * S], bf16,
                                  kind="ExternalOutput").ap()

    hid3 = hidT.rearrange("p (k s) -> p k s", k=NK)
    wqk4 = wqkT.rearrange("p (j k c) -> p j k c", j=NJ, k=NK)
    wv3 = wvT.rearrange("p (k c) -> p k c", k=NK)

    with tile.TileContext(nc) as tc, ExitStack() as ctx:
        persist = ctx.enter_context(tc.tile_pool(name="persist", bufs=1))
        bias_sb = persist.tile([128, NJ], f32, tag="bias", name="bias")
        cos_sb = persist.tile([128, S], bf16, tag="cos", name="cos")
        sin_sb = persist.tile([128, S], bf16, tag="sin", name="sin")
        vt_sb = persist.tile([128, NH * VW], bf16, tag="vt", name="vt")
        nc.scalar.dma_start(bias_sb[:], bias2d[:])

        psum = ctx.enter_context(tc.tile_pool(name="psum", bufs=1,
                                              space="PSUM"))
        unit = [0]

        def qs_tile():
            t = psum.tile([128, 1024], f32, tag=f"qs{unit[0] % 2}", name="qs")
            unit[0] += 1
            return t

        qk_pool = ctx.enter_context(tc.tile_pool(name="qk", bufs=1))
        qk_sb = qk_pool.tile([128, NJ * S], bf16, tag="qk", name="qk")
        rot_pool = ctx.enter_context(tc.tile_pool(name="rotp", bufs=1))
        rot = rot_pool.tile([128, 16 * S], bf16, tag="rot", name="rot")
        stgb_pool = ctx.enter_context(tc.tile_pool(name="stgb", bufs=6))
        vaug_pool = ctx.enter_context(tc.tile_pool(name="vaug", bufs=1))
        vaug = vaug_pool.tile([128, NT * NH * VW], bf16, tag="va", name="va")
        vaug3 = vaug.rearrange("p (t h c) -> p t h c", t=NT, h=NH)
        attn_pool = ctx.enter_context(tc.tile_pool(name="attn", bufs=1))
        # unnormalized PV output incl. bf16 denominator row at partition 96;
        # normalized in place at the end
        attn = attn_pool.tile([128, NH * S], bf16, tag="at", name="at")
        pt_pool = ctx.enter_context(tc.tile_pool(name="pt", bufs=4))
        rc_pool = ctx.enter_context(tc.tile_pool(name="rc", bufs=2))
        rcf_pool = ctx.enter_context(tc.tile_pool(name="rcf", bufs=2))
        bc_pool = ctx.enter_context(tc.tile_pool(name="bc", bufs=2))

        # weights/hidden (dead after phase 1; proj pools reuse the space) —
        # entered last among open pools so the mid-program release is LIFO
        ph1 = tc.tile_pool(name="ph1", bufs=1)
        p1 = ph1.__enter__()
        hid_sb = p1.tile([128, NK * S], bf16, tag="hid", name="hid")
        wqk_sb = p1.tile([128, NJ * NK * 128], bf16, tag="wqk", name="wqk")
        wv_sb = p1.tile([128, NK * 640], bf16, tag="wv", name="wv")
        hid3s = hid_sb.rearrange("p (k s) -> p k s", k=NK)
        wqk4s = wqk_sb.rearrange("p (j k c) -> p j k c", j=NJ, k=NK)
        wv3s = wv_sb.rearrange("p (k c) -> p k c", k=NK)
        # fine-grained loads spread over two queues so the first qk matmul
        # group (needs all hid k-tiles + wqk j=0) unblocks ASAP
        for c0 in (0, 512):
            for k in range(NK):
                nc.sync.dma_start(hid3s[:, k, c0:c0 + 512],
                                  hid3[:, k, c0:c0 + 512])
        nc.scalar.dma_start(wqk4s[:, 0:2, :, :], wqk4[:, 0:2, :, :])
        nc.scalar.dma_start(wv3s[:, :, :], wv3[:, :, :])
        nc.scalar.dma_start(wqk4s[:, 2:4, :, :], wqk4[:, 2:4, :, :])
        nc.scalar.dma_start(cos_sb[:], cosP[:])
        nc.scalar.dma_start(sin_sb[:], sinP[:])
        nc.scalar.dma_start(vt_sb[:], vtmpl[:])
        for j0 in range(4, NJ, 2):
            j1 = min(j0 + 2, NJ)
            nc.scalar.dma_start(wqk4s[:, j0:j1, :, :], wqk4[:, j0:j1, :, :])

        vt3 = vt_sb.rearrange("p (h c) -> p h c", h=NH)

        def emit_qk(j):
            ps = qs_tile()
            for half in range(2):
                c0 = half * 512
                for k in range(NK):
                    nc.tensor.matmul(ps[:, c0:c0 + 512],
                                     wqk4s[:, j, k, :],
                                     hid3s[:, k, c0:c0 + 512],
                                     start=(k == 0), stop=(k == NK - 1))
            nc.vector.tensor_scalar_add(qk_sb[:, j * S:(j + 1) * S], ps[:, :],
                                        bias_sb[:, j:j + 1])

        def emit_v(tt):
            ps = qs_tile()
            for (c0, w) in ((0, 512), (512, 128)):
                for k in range(NK):
                    nc.tensor.matmul(ps[:, c0:c0 + w],
                                     hid3s[:, k, tt * 128:(tt + 1) * 128],
                                     wv3s[:, k, c0:c0 + w],
                                     start=(k == 0), stop=(k == NK - 1))
            src = ps[:, 0:NH * HD].rearrange("p (h c) -> p h c", c=HD)
            nc.vector.tensor_add(vaug3[:, tt, :, 0:HD], src[:, :, :],
                                 vt3[:, :, 0:HD])
            # pad + ones columns (disjoint from the v region written above)
            nc.vector.tensor_copy(vaug3[:, tt, :, HD:VW], vt3[:, :, HD:VW])

        qk3 = qk_sb.rearrange("p (j s) -> p j s", j=NJ)
        stgb_tiles = {}

        def emit_rope_dma(p):
            lo_t, lo_r = _half_pos(2 * p)
            hi_t, hi_r = _half_pos(2 * p + 1)
            b0 = p * S
            eng = [nc.gpsimd, nc.sync][p % 2]
            if lo_t == hi_t and hi_r == lo_r + 40:
                eng.dma_start(rot[0:80, b0:b0 + S],
                              qk3[lo_r:lo_r + 80, lo_t, :])
            else:
                eng.dma_start(rot[0:40, b0:b0 + S],
                              qk3[lo_r:lo_r + 40, lo_t, :])
                eng.dma_start(rot[40:80, b0:b0 + S],
                              qk3[hi_r:hi_r + 40, hi_t, :])
            sb = stgb_pool.tile([128, S], bf16, tag="sb", name="sb")
            stgb_tiles[p] = sb
            eng.dma_start(sb[0:40, :], qk3[hi_r:hi_r + 40, hi_t, :])
            eng.dma_start(sb[40:80, :], qk3[lo_r:lo_r + 40, lo_t, :])

        def emit_rope_mul(p):
            b0 = p * S
            sb = stgb_tiles.pop(p)
            nc.vector.tensor_mul(rot[0:80, b0:b0 + S], rot[0:80, b0:b0 + S],
                                 cos_sb[0:80, :])
            meng = nc.gpsimd if p % 4 == 3 else nc.vector
            meng.tensor_mul(sb[0:80, :], sb[0:80, :], sin_sb[0:80, :])
            nc.vector.tensor_add(rot[0:80, b0:b0 + S], rot[0:80, b0:b0 + S],
                                 sb[0:80, :])

        def emit_attn(h):
            pq, pk = 2 * h, 2 * h + 1
            po = psum.tile([128, 1024], f32, tag=f"po{h % 2}", name="po")
            for kt in range(NT):
                st = qs_tile()
                for half in range(2):
                    c0 = half * 512
                    nc.tensor.matmul(
                        st[:, c0:c0 + 512],
                        rot[0:80, pk * S + kt * 128:pk * S + (kt + 1) * 128],
                        rot[0:80, pq * S + c0:pq * S + c0 + 512],
                        start=True, stop=True)
                pt = pt_pool.tile([128, S], bf16, tag="pt", name="pt")
                nc.scalar.activation(pt[:, :], st[:, :], AF.Exp)
                for half in range(2):
                    c0 = half * 512
                    nc.tensor.matmul(po[0:VW, c0:c0 + 512],
                                     vaug3[:, kt, h, :],
                                     pt[:, c0:c0 + 512],
                                     start=(kt == 0), stop=(kt == NT - 1))
            # evacuate PV to SBUF on ACT so the PSUM slot frees immediately
            # and the later normalize never touches the compute-critical FIFOs
            nc.scalar.activation(attn[0:VW, h * S:(h + 1) * S], po[0:VW, :],
                                 AF.Identity)

        def emit_norm(h):
            rc = rc_pool.tile([1, S], bf16, tag="rc", name="rc")
            nc.scalar.dma_start(rc[0:1, :], attn[96:97, h * S:(h + 1) * S])
            rcf = rcf_pool.tile([1, S], f32, tag="rcf", name="rcf")
            nc.vector.reciprocal(rcf[0:1, :], rc[0:1, :])
            bc = bc_pool.tile([80, S], f32, tag="bc", name="bc")
            nc.gpsimd.partition_broadcast(bc[0:80, :], rcf[0:1, :])
            nc.vector.tensor_mul(attn[0:80, h * S:(h + 1) * S],
                                 attn[0:80, h * S:(h + 1) * S], bc[0:80, :])

        # emission driver: qk j-tiles paced with v, rope, attention
        v_at = {1: (0, 1), 2: (2, 3), 3: (4, 5), 4: (6, 7)}
        rope_at = {}
        for p in range(16):
            jmax = max(_half_pos(2 * p)[0], _half_pos(2 * p + 1)[0])
            rope_at.setdefault(jmax, []).append(p)
        pending = []
        attn_q = []
        for j in range(NJ):
            emit_qk(j)
            # rope muls one round behind their staging DMAs so the DVE FIFO
            # never stalls on in-flight DMA latency
            for p in pending:
                emit_rope_mul(p)
                if p % 2 == 1:
                    attn_q.append(p // 2)
            pending = []
            for tt in v_at.get(j, ()):
                emit_v(tt)
            if j >= 4:
                # attention only after ALL v-groups are emitted: a PV that
                # precedes its vaug writer in program order would make Tile
                # order the write AFTER the read (stale data)
                for h in attn_q:
                    emit_attn(h)
                attn_q = []
            for p in rope_at.get(j, ()):
                emit_rope_dma(p)
                pending.append(p)
        for p in pending:
            emit_rope_mul(p)
            if p % 2 == 1:
                attn_q.append(p // 2)
        for h in attn_q:
            emit_attn(h)

        if KERNEL_DEBUG:
            nc.sync.dma_start(dbg_qk[:, :], qk_sb[:, :])
            nc.sync.dma_start(dbg_rot[0:80, :], rot[0:80, :])
            nc.sync.dma_start(dbg_vaug[:, :], vaug[:, :])
            nc.sync.dma_start(dbg_attn[0:97, :], attn[0:97, :])

        ph1.__exit__(None, None, None)

        # normalize + dense re-pack of attn heads + split projection:
        # kt0-3 prepass overlaps the last heads' attention, kt4 finishes
        with ExitStack() as p5:
            late = p5.enter_context(tc.tile_pool(name="late", bufs=1))
            # wp allocated first: lands on the hid/wv region (dead after the
            # last v matmul) so its load doesn't wait for the last qk matmul
            wp_sb = late.tile([128, V3_NPJ * D], bf16, tag="wp", name="wp")
            dense = late.tile([128, V3_NPJ * S], bf16, tag="dn", name="dn")
            ob_pool = p5.enter_context(tc.tile_pool(name="ob", bufs=3))
            oa_pool = p5.enter_context(tc.tile_pool(name="oa", bufs=1))
            nc.sync.dma_start(wp_sb[:], wprojT[:])
            dense3 = dense.rearrange("p (t s) -> p t s", t=V3_NPJ)
            wp3 = wp_sb.rearrange("p (t c) -> p t c", t=V3_NPJ)

            def emit_densify(h):
                for (dt, r, n, off) in _pieces(80 * h, 80):
                    nc.sync.dma_start(
                        dense3[r:r + n, dt, :],
                        attn[off:off + n, h * S:(h + 1) * S])

            for h in range(NH - 1):
                emit_norm(h)
                emit_densify(h)
            oa_tiles = []
            for j in range(NK):
                ps = qs_tile()
                for half in range(2):
                    c0 = half * 512
                    for kt in range(4):
                        nc.tensor.matmul(
                            ps[:, c0:c0 + 512],
                            wp3[:, kt, j * 128:(j + 1) * 128],
                            dense3[:, kt, c0:c0 + 512],
                            start=(kt == 0), stop=(kt == 3))
                oa = oa_pool.tile([128, S], f32, tag=f"oa{j}", name=f"oa{j}",
                                  bufs=1)
                if j % 2:
                    nc.scalar.activation(oa[:, :], ps[:, :], AF.Identity)
                else:
                    nc.vector.tensor_copy(oa[:, :], ps[:, :])
                oa_tiles.append(oa)
            emit_norm(NH - 1)
            emit_densify(NH - 1)
            for j in range(NK):
                ps = qs_tile()
                for half in range(2):
                    c0 = half * 512
                    nc.tensor.matmul(ps[:, c0:c0 + 512],
                                     wp3[:, 4, j * 128:(j + 1) * 128],
                                     dense3[:, 4, c0:c0 + 512],
                                     start=True, stop=True)
                ob = ob_pool.tile([128, S], bf16, tag="ob", name="ob")
                nc.vector.tensor_add(ob[:, :], ps[:, :], oa_tiles[j][:, :])
                nc.sync.dma_start(outT[j * 128:(j + 1) * 128, :], ob[:, :])

    nc.compile()
    return nc


def _pack_v3(Wqkv, bqkv, Wproj, bproj, g):
    """Host-side per-head-group weight packing for the v3 program."""
    import concourse.mybir as mybir
    bf16 = mybir.dt.np(mybir.dt.bfloat16)
    NH, NJ, NK, VW = V3_NH, V3_NJ, V3_NK, V3_VW

    Wp = np.zeros((NJ * 128, D), np.float32)
    bp = np.zeros((NJ * 128,), np.float32)
    for m in range(32):
        h = m // 4
        sec = (m % 4) // 2       # 0 = q, 1 = k
        half = m % 2
        src = sec * D + (g * NH + h) * HD + half * BLK
        w = Wqkv[src:src + BLK, :]
        b = bqkv[src:src + BLK]
        if sec == 0:
            w = w * SCALE
            b = b * SCALE
        t, r = _half_pos(m)
        Wp[t * 128 + r:t * 128 + r + BLK] = w
        bp[t * 128 + r:t * 128 + r + BLK] = b
    # lhsT layout [128, j, k, 128]: wqkT[p, j, k, c] = Wp[j*128+c, k*128+p]
    wqkT = np.ascontiguousarray(
        Wp.reshape(NJ, 128, NK, 128).transpose(3, 0, 2, 1)
        .reshape(128, NJ * NK * 128)).astype(bf16)
    bias2d = np.ascontiguousarray(bp.reshape(NJ, 128).T)

    Wv = Wqkv[2 * D + g * 640:2 * D + (g + 1) * 640, :]
    wvT = _tile_rows(np.ascontiguousarray(Wv.T)).astype(bf16)
    bv = bqkv[2 * D + g * 640:2 * D + (g + 1) * 640]

    vt = np.zeros((128, NH * VW), np.float32)
    for h in range(NH):
        vt[:, h * VW:h * VW + HD] = bv[h * HD:(h + 1) * HD][None, :]
        vt[:, h * VW + 96] = 1.0
    vtmpl = vt.astype(bf16)

    Wpd = Wproj[:, g * 640:(g + 1) * 640].T  # [640, 1280] dense attn rows
    wprojT = _tile_rows(np.ascontiguousarray(Wpd)).astype(bf16)
    return wqkT, bias2d, wvT, vtmpl, wprojT


def _cos_sin_v3(cos, sin):
    """Dense [0:80] rope coefficient layouts (bf16), full sequence."""
    import concourse.mybir as mybir
    bf16 = mybir.dt.np(mybir.dt.bfloat16)
    S = cos.shape[0]
    cp = np.zeros((128, S), np.float32)
    sp = np.zeros((128, S), np.float32)
    cp[0:BLK] = cos.T[0:BLK]
    cp[BLK:HD] = cos.T[BLK:HD]
    sp[0:BLK] = -sin.T[0:BLK]
    sp[BLK:HD] = sin.T[BLK:HD]
    return cp.astype(bf16), sp.astype(bf16)


def kernel(hidden_states, cos, sin, Wqkv, bqkv, Wproj, bproj, cu_seqlens):
    sys.path.insert(0, "/opt/trn_rl_repo")
    from concourse import bass_utils

    hidden_states = np.asarray(hidden_states, np.float32)
    cos = np.asarray(cos, np.float32)
    sin = np.asarray(sin, np.float32)
    Wqkv = np.asarray(Wqkv, np.float32)
    bqkv = np.asarray(bqkv, np.float32)
    Wproj = np.asarray(Wproj, np.float32)
    bproj = np.asarray(bproj, np.float32)

    S, D_ = hidden_states.shape
    assert D_ == D
    segs = _segments(cu_seqlens, S)
    uniform = (S % 4 == 0) and segs == [(i * S // 4, (i + 1) * S // 4)
                                        for i in range(4)]

    hiddenT = np.ascontiguousarray(hidden_states.T)
    cosP, sin2P = _pack_cos_sin(cos, sin)

    def _vinit(segs_local):
        n_tt = sum(-(-(e - a) // 128) for a, e in segs_local)
        v = np.zeros((128, n_tt, 17), np.float32)
        v[:, :, 16] = 1.0
        return np.ascontiguousarray(v.reshape(128, n_tt * 17))

    if uniform:
        # v3: 2 head-groups x 4 segments, bf16 on-chip
        import concourse.mybir as mybir
        bf16 = mybir.dt.np(mybir.dt.bfloat16)
        S_core = S // 4
        key = ("V3", S)
        if key not in _CACHE:
            _CACHE[key] = _build_v3()
        nc = _CACHE[key]
        cosPd, sinPd = _cos_sin_v3(cos, sin)
        hidT_b = hiddenT.astype(bf16)
        in_maps = []
        meta = []
        for g in range(2):
            wqkT, b2, wvT, vtmpl, wprojT = _pack_v3(Wqkv, bqkv, Wproj,
                                                    bproj, g)
            for s in range(4):
                sl = slice(s * S_core, (s + 1) * S_core)
                in_maps.append({
                    "hidT": _tile_rows(hidT_b[:, sl]),
                    "wqkT": wqkT,
                    "bias2d": b2,
                    "wvT": wvT,
                    "vtmpl": vtmpl,
                    "cosP": np.ascontiguousarray(cosPd[:, sl]),
                    "sinP": np.ascontiguousarray(sinPd[:, sl]),
                    "wprojT": wprojT,
                })
                meta.append((g, s))
        res = bass_utils.run_bass_kernel_spmd(nc, in_maps,
                                              core_ids=list(range(N_CORES)))
        out = np.zeros((D, S), np.float32)
        for c, (g, s) in enumerate(meta):
            out[:, s * S_core:(s + 1) * S_core] += \
                res.results[c]["outT"].astype(np.float32)
    else:
        # mode C: 8-way head parallel, full sequence per core
        n_h, S_core = H // N_CORES, S
        key = ("C", S, tuple(np.asarray(cu_seqlens).tolist()))
        if key not in _CACHE:
            _CACHE[key] = _build_program(n_h, S_core, segs,
                                         resident_hidden=False)
        nc = _CACHE[key]
        vinit = _vinit(segs)
        hid_tiled = _tile_rows(hiddenT)
        in_maps = []
        for c in range(N_CORES):
            heads = list(range(c * n_h, (c + 1) * n_h))
            wt, b2 = _pack_w(Wqkv, bqkv, heads, n_h)
            in_maps.append({
                "hiddenT": hid_tiled,
                "wqkvT": wt,
                "bias2d": b2,
                "cosP": cosP,
                "sin2P": sin2P,
                "wprojT": _pack_wproj(Wproj, heads),
                "vinit": vinit,
            })
        res = bass_utils.run_bass_kernel_spmd(nc, in_maps,
                                              core_ids=list(range(N_CORES)))
        out = np.zeros((D, S), np.float32)
        for c in range(N_CORES):
            out += res.results[c]["outT"]

    return np.ascontiguousarray(out.T) + bproj[None, :]



# revision 75
# speedup vs baseline: 1.3289x; 1.0718x over previous
"""Trainium2 Bass kernel for Ernie4.5-VL vision attention (ragged segments).

Contract: kernel(**inputs) takes the FULL unsharded inputs (keyed as in
setup_inputs()) and returns the FULL [S, D] float32 output.

Strategy
--------
All matmuls run on the PE array in float32r (full-rate fp32, ~1.5e-4 rel
err); everything else is fp32. Attention is computed per segment
(block-diagonal, no masks) in a flash-like streaming form that only ever
materializes transposed score tiles:

  qkvT = Wpack @ hidden.T          (dims on partitions, tokens on free)
  RoPE on qT/kT slices (DVE/GPSIMD elementwise)
  v_aug = transpose(vT) with a ones column appended   (PE transposes)
  per (head, segment, 1024-query chunk):
     for each 128-key tile: ST = kT-tile.T @ qT-chunk  (scores^T in PSUM)
                            PT = exp(ST)               (ACT, no max-sub)
                            outT_aug += v_aug.T @ PT   (PSUM accumulate)
     attn_outT = outT_aug[:80] * (1 / outT_aug[80])    (denominator row)
  projT_partial = WprojT_shard.T @ attn_outT           (per-core partial)

Sharding (8 cores, SPMD - one program, per-core data):
  - uniform 4x1024 segments: 2 head-groups x 4 segments (24 MB/core DMA)
  - any other cu_seqlens: 8-way head parallel, every core sees all
    segments (identical program regardless of segment raggedness)
Host does only O(S*D) glue: input transposes/packing, summing the 2 (or
8) per-token partial projections, and the bias adds.
"""

import os
import sys

import numpy as np

H = 16
HD = 80
BLK = 40  # rotate_half half-width
SCALE = HD ** -0.5
N_CORES = 8
D = 1280
NK = D // 128  # contraction tiles for the qkv matmul
ATTN_STRIDE = 96  # head row pitch in the packed attention output
MM_DT_NAME = os.environ.get("KERNEL_MM_DT", "float32r")  # or "float32"
KERNEL_DEBUG = bool(int(os.environ.get("KERNEL_DEBUG", "0")))


def _segments(cu_seqlens, S):
    """Intervals matching reference's searchsorted(cu[1:], i, 'right')."""
    b = np.clip(np.sort(np.asarray(cu_seqlens, dtype=np.int64)[1:5]), 0, S)
    bounds = [0] + list(b) + [S]
    segs = []
    for a, e in zip(bounds[:-1], bounds[1:]):
        if e > a:
            segs.append((int(a), int(e)))
    return segs


def _pack_layout(n_h):
    """Pack per-core qkv dims as 40-row blocks, 3 per 128-row tile (8 pad).

    Each tile holds one v-block at row 0 (PE transpose operands must start
    at a 32-aligned partition) and two q/k blocks at rows 40 and 80.
    Returns pos[(sec, h, half)] = (tile, row) and the number of tiles.
    """
    ntiles = 2 * n_h
    pos = {}
    for h in range(n_h):
        for half in (0, 1):
            pos[("v", h, half)] = (2 * h + half, 0)
    qk = [("q", h, half) for h in range(n_h) for half in (0, 1)]
    qk += [("k", h, half) for h in range(n_h) for half in (0, 1)]
    for j, blk in enumerate(qk):
        pos[blk] = (j // 2, BLK + BLK * (j % 2))
    return pos, ntiles


def _pieces(start, length, tile_rows=128):
    """Split global row range [start, start+length) into per-tile pieces."""
    out = []
    off = 0
    while off < length:
        g = start + off
        t, r = g // tile_rows, g % tile_rows
        n = min(tile_rows - r, length - off)
        out.append((t, r, n, off))
        off += n
    return out


def _proj_k_tiles(n_h):
    rows = ATTN_STRIDE * n_h
    kt = [128] * (rows // 128)
    if rows % 128:
        kt.append(rows % 128)
    return kt


def _build_program(n_h, S_core, segs_local, resident_hidden):
    """Emit the SPMD program. Same structure for every core.

    Engine-AP partition rules on TRN2 (walrus birverifier): compute-engine
    accesses must start at a 32-aligned partition and must not cross a
    64-boundary unless they start on one; cross-partition data movement
    must go through DMA. The layout choices below all follow from this.
    """
    import concourse.mybir as mybir
    import concourse.tile as tile
    from concourse import bacc
    from concourse.masks import make_identity
    from contextlib import ExitStack

    f32 = mybir.dt.float32
    mm_dt = getattr(mybir.dt, MM_DT_NAME)
    AF = mybir.ActivationFunctionType

    k_proj = n_h
    pos, n_mtiles = _pack_layout(n_h)
    dims_pad = n_mtiles * 128
    VW = 97  # v_aug slot width: 80 v dims + 16 zero pad + ones col at 96

    # global key-tile list: (seg_idx, t0, t1)
    t_tiles = []
    for si, (a, e) in enumerate(segs_local):
        t = a
        while t < e:
            t_tiles.append((si, t, min(t + 128, e)))
            t += 128
    n_tt = len(t_tiles)

    nc = bacc.Bacc("TRN2", target_bir_lowering=False, debug=False,
                   enable_asserts=False, num_devices=N_CORES)

    # host supplies hiddenT/wqkvT pre-tiled into 128-partition-major layout
    hiddenT = nc.dram_tensor("hiddenT", [128, NK * S_core], mm_dt,
                             kind="ExternalInput").ap()
    wqkvT = nc.dram_tensor("wqkvT", [128, NK * dims_pad], mm_dt,
                           kind="ExternalInput").ap()
    bias2d = nc.dram_tensor("bias2d", [128, n_mtiles], f32,
                            kind="ExternalInput").ap()
    # cosP/sin2P are host-packed [128, S]: rows 0:40 and 64:104 hold the
    # lo/hi rope coefficients, all other rows zero (zeroes the junk rows
    # of the rotated q/k so the K=104 score matmuls see exact zeros).
    cosP = nc.dram_tensor("cosP", [128, S_core], mm_dt,
                          kind="ExternalInput").ap()
    sin2P = nc.dram_tensor("sin2P", [128, S_core], mm_dt,
                           kind="ExternalInput").ap()
    wprojT = nc.dram_tensor("wprojT", [n_h * HD, D], mm_dt,
                            kind="ExternalInput").ap()
    # per-key-tile v_aug tail init: 16 zero pad cols + ones col (f32r memset
    # fails walrus codegen, so this comes in via DMA)
    vinit = nc.dram_tensor("vinit", [128, n_tt * (VW - HD)], mm_dt,
                           kind="ExternalInput").ap()
    outT = nc.dram_tensor("outT", [D, S_core], f32, kind="ExternalOutput").ap()
    if KERNEL_DEBUG:
        dbg_qkv = nc.dram_tensor("dbg_qkv", [128, n_mtiles * S_core], f32,
                                 kind="ExternalOutput").ap()
        dbg_rot = nc.dram_tensor("dbg_rot", [128, 2 * n_h * S_core], f32,
                                 kind="ExternalOutput").ap()
        dbg_vaug = nc.dram_tensor("dbg_vaug", [128, n_h * n_tt * VW], f32,
                                  kind="ExternalOutput").ap()
        dbg_attn = nc.dram_tensor("dbg_attn", [128, n_h * S_core], f32,
                                  kind="ExternalOutput").ap()

    def r_(ap):
        return ap.bitcast(mm_dt)

    BC = 1024  # psum tile width (2 banks); matmuls stream <=512
    big_chunks = [(c, min(c + BC, S_core)) for c in range(0, S_core, BC)]

    def halves(c0, c1):
        out = []
        q = c0
        while q < c1:
            out.append((q, min(q + 512, c1)))
            q = q + 512
        return out

    with tile.TileContext(nc) as tc, ExitStack() as ctx:
        persist = ctx.enter_context(tc.tile_pool(name="persist", bufs=1))
        ident = persist.tile([128, 128], f32, tag="ident", name="ident")
        make_identity(nc, ident[:])
        bias_sb = persist.tile([128, n_mtiles], f32, tag="bias", name="bias")
        nc.sync.dma_start(bias_sb[:], bias2d[:])

        # PSUM: two 2-bank slots (t0/t1) shared by qkv/scores/proj, two
        # 1-bank slots for v-transposes, one 2-bank slot for PV accumulate
        psum_all_cm = tc.tile_pool(name="psum_all", bufs=1, space="PSUM")
        psum_all = psum_all_cm.__enter__()
        # big pool: qkvT tiles (phases 1-3), slots reused by attn (phases 4-5)
        qkv_pool = ctx.enter_context(tc.tile_pool(name="big", bufs=1))
        qkv_sb = [qkv_pool.tile([128, S_core], mm_dt, tag=f"qkvT{j}",
                                name=f"qkvT{j}") for j in range(n_mtiles)]
        # rope output (rows 0:104 live, 40:64 zeroed via cosP/sin2P pads)
        rot_cm = tc.tile_pool(name="rot", bufs=1)
        rv = rot_cm.__enter__()
        rot_sb = {}
        for h in range(n_h):
            for sec in ("q", "k"):
                rot_sb[(sec, h)] = rv.tile([128, S_core], mm_dt,
                                           tag=f"rot_{sec}{h}",
                                           name=f"rot_{sec}{h}")
        RC = 1024
        rope_cm = tc.tile_pool(name="rope_scr", bufs=2)
        rope_scr = rope_cm.__enter__()

        # ------------ phase 1: qkvT = Wpack @ hidden.T --------------
        with ExitStack() as p1:
            hidden3 = hiddenT.rearrange("p (k s) -> p k s", k=NK)
            w3 = wqkvT.rearrange("p (k m) -> p k m", k=NK)
            if resident_hidden:
                hid_pool = p1.enter_context(tc.tile_pool(name="hid", bufs=1))
                w_pool = p1.enter_context(tc.tile_pool(name="wstream", bufs=3))
                hid_sb = [hid_pool.tile([128, S_core], mm_dt, tag=f"hid{k}",
                                        name=f"hid{k}") for k in range(NK)]
                wj0 = w_pool.tile([128, NK * 128], mm_dt, tag="wj", name="wj")
                nc.sync.dma_start(hid_sb[0][:], hidden3[:, 0, :])
                nc.sync.dma_start(
                    wj0.rearrange("p (k m) -> p k m", k=NK)[:, :, :],
                    w3[:, :, 0:128])
                for k in range(1, NK):
                    nc.sync.dma_start(hid_sb[k][:], hidden3[:, k, :])
                for j in range(n_mtiles):
                    if j == 0:
                        wj = wj0
                    else:
                        wj = w_pool.tile([128, NK * 128], mm_dt, tag="wj",
                                         name="wj")
                        nc.sync.dma_start(
                            wj.rearrange("p (k m) -> p k m", k=NK)[:, :, :],
                            w3[:, :, j * 128:(j + 1) * 128])
                    for (h0, h1) in halves(0, S_core):
                        hw = h1 - h0
                        ps = psum_all.tile([128, 512], f32,
                                           tag=f"t{(h0 // 512) % 2}",
                                           name="qkvp")
                        for k in range(NK):
                            nc.tensor.matmul(
                                ps[:, :hw],
                                r_(wj[:, k * 128:(k + 1) * 128]),
                                r_(hid_sb[k][:, h0:h1]),
                                start=(k == 0), stop=(k == NK - 1))
                        nc.scalar.activation(qkv_sb[j][:, h0:h1], ps[:, :hw],
                                             AF.Identity,
                                             bias=bias_sb[:, j:j + 1])
            else:
                # k-outer streaming: two psum slots hold four j-streams
                # (columns 0:512 and 512:1024), hidden tiles are tiny
                w_pool = p1.enter_context(tc.tile_pool(name="wres", bufs=1))
                w_sb = [w_pool.tile([128, dims_pad], mm_dt, tag=f"w{k}",
                                    name=f"w{k}") for k in range(NK)]
                for k in range(NK):
                    nc.sync.dma_start(w_sb[k][:], w3[:, k, :])
                assert n_mtiles == 4
                hid_pool = p1.enter_context(tc.tile_pool(name="hidstream",
                                                         bufs=3))
                for (h0, h1) in halves(0, S_core):
                    hw = h1 - h0
                    ps01 = psum_all.tile([128, BC], f32, tag="t0", name="ps01")
                    ps23 = psum_all.tile([128, BC], f32, tag="t1", name="ps23")
                    pj_of = lambda j: (ps01 if j < 2 else ps23,
                                       (j % 2) * 512)
                    for k in range(NK):
                        ht = hid_pool.tile([128, 512], mm_dt, tag="hidc",
                                           name="hidc")
                        nc.sync.dma_start(ht[:, :hw], hidden3[:, k, h0:h1])
                        for j in range(n_mtiles):
                            psj, co = pj_of(j)
                            nc.tensor.matmul(
                                psj[:, co:co + hw],
                                r_(w_sb[k][:, j * 128:(j + 1) * 128]),
                                r_(ht[:, :hw]),
                                start=(k == 0), stop=(k == NK - 1))
                    for j in range(n_mtiles):
                        psj, co = pj_of(j)
                        nc.scalar.activation(qkv_sb[j][:, h0:h1],
                                             psj[:, co:co + hw], AF.Identity,
                                             bias=bias_sb[:, j:j + 1])

        psum_all_cm.__exit__(None, None, None)
        ps_att = ctx.enter_context(tc.tile_pool(name="ps_att", bufs=1,
                                                space="PSUM"))

        # ------------ phase 2: RoPE --------------------------------
        # DMA-stage lo/hi into 0:40 / 64:104 (stgA) and swapped (stgB),
        # then rot = stgA*cosP + stgB*sin2P as three same-base wide ops.
        # double-buffered persistent staging tensors; rows 40:64 zeroed once
        # from cosP's zero rows so the [0:104) products read defined zeros
        stg = {}
        for nm in ("sa0", "sa1", "sb0", "sb1"):
            stg[nm] = rope_scr.tile([128, RC], mm_dt, tag=nm, name=nm, bufs=1)
        pair_i = 0
        for ci, f0 in enumerate(range(0, S_core, RC)):
            f1 = min(f0 + RC, S_core)
            fs = f1 - f0
            cos_sb = rope_scr.tile([128, RC], mm_dt, tag="cos", name="cos",
                                   bufs=1)
            sin_sb = rope_scr.tile([128, RC], mm_dt, tag="sin", name="sin",
                                   bufs=1)
            nc.scalar.dma_start(cos_sb[:, :fs], cosP[:, f0:f1])
            nc.scalar.dma_start(sin_sb[:, :fs], sin2P[:, f0:f1])
            if ci == 0:
                for nm in stg:
                    nc.scalar.dma_start(stg[nm][BLK:64, :], cos_sb[BLK:64, :])
            for h in range(n_h):
                for sec in ("q", "k"):
                    lo_t, lo_r = pos[(sec, h, 0)]
                    hi_t, hi_r = pos[(sec, h, 1)]
                    assert hi_t == lo_t and hi_r == lo_r + BLK
                    x = qkv_sb[lo_t]
                    dst = rot_sb[(sec, h)]
                    stga = stg[f"sa{pair_i % 2}"]
                    stgb = stg[f"sb{pair_i % 2}"]
                    nc.scalar.dma_start(stga[0:BLK, :fs],
                                        x[lo_r:lo_r + BLK, f0:f1])
                    nc.scalar.dma_start(stga[64:64 + BLK, :fs],
                                        x[hi_r:hi_r + BLK, f0:f1])
                    nc.scalar.dma_start(stgb[0:BLK, :fs],
                                        x[hi_r:hi_r + BLK, f0:f1])
                    nc.scalar.dma_start(stgb[64:64 + BLK, :fs],
                                        x[lo_r:lo_r + BLK, f0:f1])
                    nc.vector.tensor_mul(dst[0:104, f0:f1], stga[0:104, :fs],
                                         cos_sb[0:104, :fs])
                    eng = nc.gpsimd if pair_i % 2 == 0 else nc.vector
                    eng.tensor_mul(stgb[0:104, :fs], stgb[0:104, :fs],
                                   sin_sb[0:104, :fs])
                    nc.vector.tensor_add(dst[0:104, f0:f1], dst[0:104, f0:f1],
                                         stgb[0:104, :fs])
                    pair_i += 1
        rope_cm.__exit__(None, None, None)

        # v_aug tiles + per-head emitter (invoked right after each head's
        # rope so attention unblocks head by head)
        vaug_cm = tc.tile_pool(name="vaug", bufs=1)
        vaug_pool = vaug_cm.__enter__()
        vaug_sb = [vaug_pool.tile([128, n_tt * VW], mm_dt, tag=f"vaug{h}",
                                  name=f"vaug{h}") for h in range(n_h)]
        vinit3 = vinit.rearrange("p (t c) -> p t c", c=VW - HD)
        for h in range(n_h):
            nc.sync.dma_start(
                vaug_sb[h].rearrange("p (t c) -> p t c", c=VW)[:, :, HD:VW],
                vinit3[:, :, :])
        GRP = 4  # key tiles transposed per psum tile / copy (1 psum bank)

        def emit_vaug(h):
            gi = 0
            while gi < n_tt:
                hi_g = min(gi + GRP, n_tt)
                if all(t_tiles[g][2] - t_tiles[g][1] == 128
                       for g in range(gi, hi_g)):
                    grp = list(range(gi, hi_g))
                else:
                    grp = [gi]
                ng = len(grp)
                tp = ps_att.tile([128, GRP * HD], f32, tag="tp", name="tp")
                for x, g in enumerate(grp):
                    si, t0, t1 = t_tiles[g]
                    sz = t1 - t0
                    for half in (0, 1):
                        vt, vr = pos[("v", h, half)]
                        nc.tensor.transpose(
                            tp[:sz, x * HD + half * BLK:
                               x * HD + (half + 1) * BLK],
                            qkv_sb[vt][0:BLK, t0:t1].bitcast(f32),
                            ident[:BLK, :BLK])
                sz0 = t_tiles[grp[0]][2] - t_tiles[grp[0]][1]
                dst = vaug_sb[h].rearrange("p (t c) -> p t c", c=VW)
                src_ap = tp.rearrange("p (t c) -> p t c", c=HD)
                if h % 2 == 0:
                    nc.vector.tensor_copy(dst[:sz0, grp[0]:grp[0] + ng, 0:HD],
                                          src_ap[:sz0, 0:ng, :])
                else:
                    nc.scalar.activation(dst[:sz0, grp[0]:grp[0] + ng, 0:HD],
                                         src_ap[:sz0, 0:ng, :], AF.Identity)
                gi += ng




        if KERNEL_DEBUG:
            for j in range(n_mtiles):
                nc.sync.dma_start(
                    dbg_qkv[:, j * S_core:(j + 1) * S_core],
                    qkv_sb[j][:].bitcast(f32))
            i_ = 0
            for h in range(n_h):
                for sec in ("q", "k"):
                    nc.sync.dma_start(
                        dbg_rot[:, i_ * S_core:(i_ + 1) * S_core],
                        rot_sb[(sec, h)][:].bitcast(f32))
                    i_ += 1

        # ------------ phase 4: attention ----------------------------
        # one attn tile per head (rows 0:80) so every compute access is
        # partition-0 based; tiles reuse the dead qkvT slots
        attn_sb = [qkv_pool.tile([128, S_core], mm_dt, tag=f"qkvT{h}",
                                 name=f"attnT{h}") for h in range(n_h)]

        seg_ttiles = {}
        for ti, (si, t0, t1) in enumerate(t_tiles):
            seg_ttiles.setdefault(si, []).append((ti, t0, t1))

        BA = 512  # attention query-chunk width (1-bank psum slots)
        with ExitStack() as p4:
            pt_pool = p4.enter_context(tc.tile_pool(name="pt", bufs=3))
            nrm_pool = p4.enter_context(tc.tile_pool(name="nrm", bufs=2))
            unit_box = [0]

            def emit_attention(h, si, a, e):
                qT = rot_sb[("q", h)]
                kT = rot_sb[("k", h)]
                q = a
                while q < e:
                    q0, q1 = q, min(q + BA, e)
                    qs = q1 - q0
                    po = ps_att.tile([128, BA], f32,
                                     tag=f"po{unit_box[0] % 2}", name="pv")
                    tts = seg_ttiles[si]
                    for idx, (ti, t0, t1) in enumerate(tts):
                        sz = t1 - t0
                        ps = ps_att.tile([128, BA], f32, tag=f"st{idx % 2}",
                                         name="st")
                        nc.tensor.matmul(ps[:sz, :qs], r_(kT[0:104, t0:t1]),
                                         r_(qT[0:104, q0:q1]),
                                         start=True, stop=True)
                        pt = pt_pool.tile([128, BA], mm_dt, tag="pt", name="pt")
                        nc.scalar.activation(pt[:sz, :qs], ps[:sz, :qs], AF.Exp)
                        nc.tensor.matmul(
                            po[:VW, :qs],
                            r_(vaug_sb[h][:sz, ti * VW:(ti + 1) * VW]),
                            r_(pt[:sz, :qs]),
                            start=(idx == 0), stop=(idx == len(tts) - 1))
                    # partition_broadcast ucode reads physical partition 0,
                    # so shift the denominator row 96 -> 0 via DMA
                    rc = nrm_pool.tile([128, BA], f32, tag="rc", name="rc")
                    nc.vector.tensor_copy(rc[96:97, :qs], po[96:97, :qs])
                    nc.sync.dma_start(rc[0:1, :qs], rc[96:97, :qs])
                    nc.vector.reciprocal(rc[0:1, :qs], rc[0:1, :qs])
                    bc = nrm_pool.tile([128, BA], mm_dt, tag="bc", name="bc")
                    nc.gpsimd.partition_broadcast(
                        bc[0:HD, :qs], rc[0:1, :qs].bitcast(mm_dt))
                    nc.vector.tensor_mul(attn_sb[h][0:HD, q0:q1],
                                         po[0:HD, :qs], bc[0:HD, :qs])
                    unit_box[0] += 1
                    q = q1

            if len(segs_local) == 1:
                a, e = segs_local[0]
                for h in range(n_h):
                    emit_vaug(h)
                    emit_attention(h, 0, a, e)
            else:
                for h in range(n_h):
                    emit_vaug(h)
                for si, (a, e) in enumerate(segs_local):
                    for h in range(n_h):
                        emit_attention(h, si, a, e)

        vaug_cm.__exit__(None, None, None)
        rot_cm.__exit__(None, None, None)

        # ------------ phase 5: projection partial -------------------
        with ExitStack() as p5:
            wp_pool = p5.enter_context(tc.tile_pool(name="wp", bufs=1))
            wp_sb = []
            for kt in range(k_proj):
                t = wp_pool.tile([HD, D], mm_dt, tag=f"wp{kt}", name=f"wp{kt}")
                nc.sync.dma_start(t[:], wprojT[kt * HD:(kt + 1) * HD, :])
                wp_sb.append(t)
            out_pool = p5.enter_context(tc.tile_pool(name="outsb", bufs=3))
            for (c0, c1) in big_chunks:
                cs = c1 - c0
                for j in range(D // 128):
                    ob = out_pool.tile([128, BC], f32, tag="ob", name="ob")
                    for (h0, h1) in halves(c0, c1):
                        ps = ps_att.tile([128, 512], f32, tag=f"st{j % 2}",
                                         name="pj")
                        for kt in range(k_proj):
                            nc.tensor.matmul(
                                ps[:, :h1 - h0],
                                r_(wp_sb[kt][:, j * 128:(j + 1) * 128]),
                                r_(attn_sb[kt][0:HD, h0:h1]),
                                start=(kt == 0), stop=(kt == k_proj - 1))
                        if j % 2 == 0:
                            nc.vector.tensor_copy(ob[:, h0 - c0:h1 - c0],
                                                  ps[:, :h1 - h0])
                        else:
                            nc.scalar.activation(ob[:, h0 - c0:h1 - c0],
                                                 ps[:, :h1 - h0], AF.Identity)
                    nc.sync.dma_start(outT[j * 128:(j + 1) * 128, c0:c1],
                                      ob[:, :cs])

    nc.compile()
    return nc


def _pack_w(Wqkv, bqkv, heads, n_h):
    """Per-core packed qkv weights (q rows pre-scaled).

    Returns wqkvT_tiled [128, NK*dims_pad] (k-major blocks of [128, dims_pad])
    and bias2d [128, n_mtiles]."""
    pos, n_mtiles = _pack_layout(n_h)
    dims_pad = n_mtiles * 128
    W = np.zeros((dims_pad, D), np.float32)
    b = np.zeros((dims_pad,), np.float32)
    sec_off = {"q": 0, "k": D, "v": 2 * D}
    for i, h in enumerate(heads):
        for sec in ("q", "k", "v"):
            for half in (0, 1):
                t, r = pos[(sec, i, half)]
                src = sec_off[sec] + h * HD + half * BLK
                w = Wqkv[src:src + BLK, :]
                bb = bqkv[src:src + BLK]
                if sec == "q":
                    w = w * SCALE
                    bb = bb * SCALE
                W[t * 128 + r:t * 128 + r + BLK] = w
                b[t * 128 + r:t * 128 + r + BLK] = bb
    w_tiled = _tile_rows(np.ascontiguousarray(W.T))
    bias2d = np.ascontiguousarray(b.reshape(n_mtiles, 128).T)
    return w_tiled, bias2d


def _tile_rows(x):
    """[R, C] with R = nk*128 -> [128, nk*C] k-major tiling."""
    R, C = x.shape
    nk = R // 128
    return np.ascontiguousarray(
        x.reshape(nk, 128, C).transpose(1, 0, 2).reshape(128, nk * C))


def _pack_wproj(Wproj, heads):
    """Rows of Wproj.T for this core's head dims, stacked per head."""
    W = np.zeros((len(heads) * HD, Wproj.shape[0]), np.float32)
    for i, h in enumerate(heads):
        W[i * HD:(i + 1) * HD] = Wproj[:, h * HD:(h + 1) * HD].T
    return W


def _pack_cos_sin(cos, sin):
    """cosP/sin2P [128, S]: lo coeffs at rows 0:40, hi at 64:104, rest 0.

    sin2P row signs match rot = x*cosP + swap(x)*sin2P: lo rows hold
    -sin_lo (they multiply x_hi), hi rows hold +sin_hi (they multiply x_lo).
    """
    S = cos.shape[0]
    cosP = np.zeros((128, S), np.float32)
    sinP = np.zeros((128, S), np.float32)
    cosP[0:BLK] = cos.T[0:BLK]
    cosP[64:64 + BLK] = cos.T[BLK:HD]
    sinP[0:BLK] = -sin.T[0:BLK]
    sinP[64:64 + BLK] = sin.T[BLK:HD]
    return cosP, sinP


_CACHE = {}

# ---------------------------------------------------------------------------
# v3 fast path (uniform 4x1024 segments): 2 head-groups x 4 segments SPMD.
#
# Per core: 8 heads, 1024 tokens, one segment. All on-chip data bf16 except
# PSUM (f32) and the normalization scalars (f32).
#   - q/k packed dense: 32 40-row halves, 3 per 128-row tile (11 j-tiles)
#   - v computed untransposed ([tokens, vdim]) straight into PV operand slots
#   - RoPE: DMA-stage [lo;hi]/[hi;lo] into dense [0:80] layouts, 3 DVE ops
#     at bf16 2x rate; scores contract K=80 (no zero padding rows)
#   - scores^T per (head, key-tile) into [128,1024] PSUM, one wide exp
#   - PV with ones column at slot col 96 -> denominators at PSUM row 96
#   - projection over 5 dense 128-row K-tiles (attn heads re-packed via DMA)
# ---------------------------------------------------------------------------

V3_S = 1024     # tokens per core
V3_NH = 8       # heads per core
V3_NJ = 11      # dense qk j-tiles (3 x 40-row halves each, 8 junk rows)
V3_NK = 10      # contraction tiles (D / 128)
V3_NT = 8       # token/key tiles (S / 128)
V3_VW = 97      # v slot: 80 v dims + 16 zero pad + ones col at 96
V3_NPJ = 5      # dense proj k-tiles (8 heads * 80 / 128)


def _half_pos(m):
    """Packed position of 40-row half m: (j_tile, row in {0, 40, 80})."""
    return m // 3, 40 * (m % 3)


def _build_v3():
    import concourse.mybir as mybir
    import concourse.tile as tile
    from concourse import bacc
    from contextlib import ExitStack

    f32 = mybir.dt.float32
    bf16 = mybir.dt.bfloat16
    AF = mybir.ActivationFunctionType
    S, NH, NJ, NK, NT, VW = V3_S, V3_NH, V3_NJ, V3_NK, V3_NT, V3_VW

    nc = bacc.Bacc("TRN2", target_bir_lowering=False, debug=False,
                   enable_asserts=False, num_devices=N_CORES)

    hidT = nc.dram_tensor("hidT", [128, NK * S], bf16,
                          kind="ExternalInput").ap()
    wqkT = nc.dram_tensor("wqkT", [128, NJ * NK * 128], bf16,
                          kind="ExternalInput").ap()
    bias2d = nc.dram_tensor("bias2d", [128, NJ], f32,
                            kind="ExternalInput").ap()
    wvT = nc.dram_tensor("wvT", [128, NK * 640], bf16,
                         kind="ExternalInput").ap()
    vtmpl = nc.dram_tensor("vtmpl", [128, NH * VW], bf16,
                           kind="ExternalInput").ap()
    cosP = nc.dram_tensor("cosP", [128, S], bf16, kind="ExternalInput").ap()
    sinP = nc.dram_tensor("sinP", [128, S], bf16, kind="ExternalInput").ap()
    wprojT = nc.dram_tensor("wprojT", [128, V3_NPJ * D], bf16,
                            kind="ExternalInput").ap()
    outT = nc.dram_tensor("outT", [D, S], bf16, kind="ExternalOutput").ap()
    if KERNEL_DEBUG:
        dbg_qk = nc.dram_tensor("dbg_qk", [128, NJ * S], bf16,
                                kind="ExternalOutput").ap()
        dbg_rot = nc.dram_tensor("dbg_rot", [128, 16 * S], bf16,
                                 kind="ExternalOutput").ap()
        dbg_vaug = nc.dram_tensor("dbg_vaug", [128, NT * NH * VW], bf16,
                                  kind="ExternalOutput").ap()
        dbg_attn = nc.dram_tensor("dbg_attn", [128, NH * S], bf16,
                                  kind="ExternalOutput").ap()

    hid3 = hidT.rearrange("p (k s) -> p k s", k=NK)
    wqk4 = wqkT.rearrange("p (j k c) -> p j k c", j=NJ, k=NK)
    wv3 = wvT.rearrange("p (k c) -> p k c", k=NK)

    with tile.TileContext(nc) as tc, ExitStack() as ctx:
        persist = ctx.enter_context(tc.tile_pool(name="persist", bufs=1))
        bias_sb = persist.tile([128, NJ], f32, tag="bias", name="bias")
        cos_sb = persist.tile([128, S], bf16, tag="cos", name="cos")
        sin_sb = persist.tile([128, S], bf16, tag="sin", name="sin")
        vt_sb = persist.tile([128, NH * VW], bf16, tag="vt", name="vt")
        pass

        psum = ctx.enter_context(tc.tile_pool(name="psum", bufs=1,
                                              space="PSUM"))
        unit = [0]

        def qs_tile():
            t = psum.tile([128, 1024], f32, tag=f"qs{unit[0] % 2}", name="qs")
            unit[0] += 1
            return t

        qk_pool = ctx.enter_context(tc.tile_pool(name="qk", bufs=1))
        qk_sb = qk_pool.tile([128, NJ * S], bf16, tag="qk", name="qk")
        rot_pool = ctx.enter_context(tc.tile_pool(name="rotp", bufs=1))
        rot = rot_pool.tile([128, 16 * S], bf16, tag="rot", name="rot")
        stgb_pool = ctx.enter_context(tc.tile_pool(name="stgb", bufs=6))
        vaug_pool = ctx.enter_context(tc.tile_pool(name="vaug", bufs=1))
        vaug = vaug_pool.tile([128, NT * NH * VW], bf16, tag="va", name="va")
        vaug3 = vaug.rearrange("p (t h c) -> p t h c", t=NT, h=NH)
        attn_pool = ctx.enter_context(tc.tile_pool(name="attn", bufs=1))
        # unnormalized PV output incl. bf16 denominator row at partition 96;
        # normalized in place at the end
        attn = attn_pool.tile([128, NH * S], bf16, tag="at", name="at")
        pt_pool = ctx.enter_context(tc.tile_pool(name="pt", bufs=12))
        rc_pool = ctx.enter_context(tc.tile_pool(name="rc", bufs=2))
        rcf_pool = ctx.enter_context(tc.tile_pool(name="rcf", bufs=2))
        bc_pool = ctx.enter_context(tc.tile_pool(name="bc", bufs=2))
        wp_pool = ctx.enter_context(tc.tile_pool(name="wp", bufs=1))
        wp_sb = wp_pool.tile([128, V3_NPJ * D], bf16, tag="wp", name="wp")

        # weights/hidden (dead after phase 1; proj pools reuse the space) —
        # entered last among open pools so the mid-program release is LIFO
        ph1 = tc.tile_pool(name="ph1", bufs=1)
        p1 = ph1.__enter__()
        hid_sb = p1.tile([128, NK * S], bf16, tag="hid", name="hid")
        wqk_sb = p1.tile([128, NJ * NK * 128], bf16, tag="wqk", name="wqk")
        wv_sb = p1.tile([128, NK * 640], bf16, tag="wv", name="wv")
        hid3s = hid_sb.rearrange("p (k s) -> p k s", k=NK)
        wqk4s = wqk_sb.rearrange("p (j k c) -> p j k c", j=NJ, k=NK)
        wv3s = wv_sb.rearrange("p (k c) -> p k c", k=NK)
        # PE warm-up: the cost model prices p-state at dispatch; keep the
        # PE busy on junk matmuls while the input loads stream so the real
        # matmuls dispatch against a warm (2.4 GHz) clock
        wa = p1.tile([128, 16], bf16, tag="wa", name="wa")
        wb = p1.tile([128, 256], bf16, tag="wb", name="wb")
        nc.vector.memset(wa[:, :], 0.0)
        nc.vector.memset(wb[:, :], 0.0)
        wps = psum.tile([128, 1024], f32, tag="po", name="warm")
        for _ in range(26):
            nc.tensor.matmul(wps[0:16, 0:256], wa[:, :], wb[:, :],
                             start=True, stop=True)
        nc.vector.tensor_copy(wa[0:16, 0:4], wps[0:16, 0:4])

        # loads on one queue in priority order; the tail weight loads are
        # deferred into the round loop (just-in-time) so their transfers
        # never sit ahead of the rope staging DMAs on the serial DMA device
        nc.sync.dma_start(vt_sb[:], vtmpl[:])
        nc.sync.dma_start(bias_sb[:], bias2d[:])
        nc.sync.dma_start(wqk4s[:, 0:3, :, :], wqk4[:, 0:3, :, :])
        nc.sync.dma_start(hid3s[:, 0:5, :], hid3[:, 0:5, :])
        nc.sync.dma_start(hid3s[:, 5:NK, :], hid3[:, 5:NK, :])
        nc.sync.dma_start(cos_sb[:], cosP[:])
        nc.sync.dma_start(sin_sb[:], sinP[:])
        # JIT loads must be EMITTED before their first consumer (emission
        # order defines dependency direction), while issuing late enough
        # that their transfers don't delay the rope staging DMAs
        load_at = {
            1: lambda: nc.sync.dma_start(wv3s[:, :, :], wv3[:, :, :]),
            2: lambda: nc.sync.dma_start(wqk4s[:, 3:6, :, :],
                                         wqk4[:, 3:6, :, :]),
            3: lambda: nc.sync.dma_start(wqk4s[:, 6:9, :, :],
                                         wqk4[:, 6:9, :, :]),
            4: lambda: nc.sync.dma_start(wqk4s[:, 9:NJ, :, :],
                                         wqk4[:, 9:NJ, :, :]),
            5: lambda: nc.sync.dma_start(wp_sb[:], wprojT[:]),
        }

        vt3 = vt_sb.rearrange("p (h c) -> p h c", h=NH)

        def emit_qk(j):
            ps = qs_tile()
            for half in range(2):
                c0 = half * 512
                for k in range(NK):
                    nc.tensor.matmul(ps[:, c0:c0 + 512],
                                     wqk4s[:, j, k, :],
                                     hid3s[:, k, c0:c0 + 512],
                                     start=(k == 0), stop=(k == NK - 1))
            if j < 5:
                # early copies on ACT (idle pre-exp): their rope staging
                # DMAs directly follow on the same queue, so the critical
                # startup rope chain never waits in a clogged FIFO
                nc.scalar.activation(qk_sb[:, j * S:(j + 1) * S], ps[:, :],
                                     AF.Identity, bias=bias_sb[:, j:j + 1])
            else:
                nc.vector.tensor_scalar_add(qk_sb[:, j * S:(j + 1) * S],
                                            ps[:, :], bias_sb[:, j:j + 1])

        def emit_v(tt):
            ps = qs_tile()
            for (c0, w) in ((0, 512), (512, 128)):
                for k in range(NK):
                    nc.tensor.matmul(ps[:, c0:c0 + w],
                                     hid3s[:, k, tt * 128:(tt + 1) * 128],
                                     wv3s[:, k, c0:c0 + w],
                                     start=(k == 0), stop=(k == NK - 1))
            src = ps[:, 0:NH * HD].rearrange("p (h c) -> p h c", c=HD)
            nc.vector.tensor_add(vaug3[:, tt, :, 0:HD], src[:, :, :],
                                 vt3[:, :, 0:HD])
            # pad + ones columns (disjoint from the v region written above)
            nc.vector.tensor_copy(vaug3[:, tt, :, HD:VW], vt3[:, :, HD:VW])

        qk3 = qk_sb.rearrange("p (j s) -> p j s", j=NJ)
        stgb_tiles = {}

        def emit_rope_dma(p):
            lo_t, lo_r = _half_pos(2 * p)
            hi_t, hi_r = _half_pos(2 * p + 1)
            b0 = p * S
            if p <= 6:
                eng = nc.scalar
            elif p >= 13:
                eng = nc.gpsimd
            else:
                eng = [nc.gpsimd, nc.sync][p % 2]
            if lo_t == hi_t and hi_r == lo_r + 40:
                eng.dma_start(rot[0:80, b0:b0 + S],
                              qk3[lo_r:lo_r + 80, lo_t, :])
            else:
                eng.dma_start(rot[0:40, b0:b0 + S],
                              qk3[lo_r:lo_r + 40, lo_t, :])
                eng.dma_start(rot[40:80, b0:b0 + S],
                              qk3[hi_r:hi_r + 40, hi_t, :])
            sb = stgb_pool.tile([128, S], bf16, tag="sb", name="sb")
            stgb_tiles[p] = sb
            eng.dma_start(sb[0:40, :], qk3[hi_r:hi_r + 40, hi_t, :])
            eng.dma_start(sb[40:80, :], qk3[lo_r:lo_r + 40, lo_t, :])

        def emit_rope_mul(p):
            b0 = p * S
            sb = stgb_tiles.pop(p)
            nc.vector.tensor_mul(rot[0:80, b0:b0 + S], rot[0:80, b0:b0 + S],
                                 cos_sb[0:80, :])
            meng = nc.gpsimd if p % 4 == 3 else nc.vector
            meng.tensor_mul(sb[0:80, :], sb[0:80, :], sin_sb[0:80, :])
            nc.vector.tensor_add(rot[0:80, b0:b0 + S], rot[0:80, b0:b0 + S],
                                 sb[0:80, :])

        # Attention pump. 512-wide score half-units (unit = (kt, half),
        # 16 per head) on dedicated 1-bank PSUM slots decouple the exp
        # stream from the qk/v slot rotation. Invariant: every emitted
        # instruction's dependencies (incl. slot predecessors) are emitted
        # before it — PVs of head h follow head h-1's evacuation (single po
        # slot), exps run ahead of PVs by at most PT_AHEAD pt tiles.
        PT_AHEAD = 10
        heads_q = []      # started heads, in order
        v_done = [0]
        st_unit = [0]
        outstanding = [0]

        def start_attn(h):
            heads_q.append({"h": h, "se": 0, "pv": 0, "po": None})

        def emit_st_exp(hs):
            h, u = hs["h"], hs["se"]
            kt, half = u // 2, u % 2
            pq, pk = 2 * h, 2 * h + 1
            c0 = half * 512
            st = psum.tile([128, 512], f32, tag=f"st{st_unit[0] % 2}",
                           name="st")
            st_unit[0] += 1
            nc.tensor.matmul(
                st[:, :],
                rot[0:80, pk * S + kt * 128:pk * S + (kt + 1) * 128],
                rot[0:80, pq * S + c0:pq * S + c0 + 512],
                start=True, stop=True)
            pt = pt_pool.tile([128, 512], bf16, tag="pt", name="pt")
            nc.scalar.activation(pt[:, :], st[:, :], AF.Exp)
            hs.setdefault("pts", []).append(pt)
            hs["se"] += 1
            outstanding[0] += 1

        def emit_pv(hs):
            h, u = hs["h"], hs["pv"]
            kt, half = u // 2, u % 2
            if hs["po"] is None:
                hs["po"] = psum.tile([128, 1024], f32, tag="po", name="po")
            c0 = half * 512
            nc.tensor.matmul(hs["po"][0:VW, c0:c0 + 512],
                             vaug3[:, kt, h, :],
                             hs["pts"][u][:, :],
                             start=(kt == 0), stop=(kt == NT - 1))
            hs["pv"] += 1
            outstanding[0] -= 1

        def pump_attn():
            progress = True
            while progress:
                progress = False
                if heads_q:
                    hs = heads_q[0]
                    while (hs["pv"] < hs["se"]
                           and hs["pv"] // 2 < v_done[0]):
                        emit_pv(hs)
                        progress = True
                    if hs["pv"] == 2 * NT:
                        h = hs["h"]
                        # evacuate on ACT so the po slot frees immediately
                        # and normalize never touches compute FIFOs
                        nc.scalar.activation(attn[0:VW, h * S:(h + 1) * S],
                                             hs["po"][0:VW, :], AF.Identity)
                        heads_q.pop(0)
                        progress = True
                        continue
                for hs in heads_q:
                    while (hs["se"] < 2 * NT
                           and outstanding[0] < PT_AHEAD):
                        emit_st_exp(hs)
                        progress = True

        def emit_norm(h):
            rc = rc_pool.tile([1, S], bf16, tag="rc", name="rc")
            nc.scalar.dma_start(rc[0:1, :], attn[96:97, h * S:(h + 1) * S])
            rcf = rcf_pool.tile([1, S], f32, tag="rcf", name="rcf")
            nc.vector.reciprocal(rcf[0:1, :], rc[0:1, :])
            bc = bc_pool.tile([80, S], f32, tag="bc", name="bc")
            nc.gpsimd.partition_broadcast(bc[0:80, :], rcf[0:1, :])
            nc.vector.tensor_mul(attn[0:80, h * S:(h + 1) * S],
                                 attn[0:80, h * S:(h + 1) * S], bc[0:80, :])

        # emission driver: qk j-tiles paced with v, rope, attention
        v_at = {1: (0, 1), 2: (2, 3), 3: (4, 5), 4: (6, 7)}
        rope_at = {}
        for p in range(16):
            jmax = max(_half_pos(2 * p)[0], _half_pos(2 * p + 1)[0])
            rope_at.setdefault(jmax, []).append(p)
        pending = []
        for j in range(NJ):
            if j in load_at:
                load_at[j]()
            emit_qk(j)
            # rope muls one round behind their staging DMAs so the DVE FIFO
            # never stalls on in-flight DMA latency
            for p in pending:
                emit_rope_mul(p)
                if p % 2 == 1:
                    start_attn(p // 2)
            pending = []
            for tt in v_at.get(j, ()):
                emit_v(tt)
                v_done[0] += 1
            pump_attn()
            for p in rope_at.get(j, ()):
                emit_rope_dma(p)
                pending.append(p)
        for p in pending:
            emit_rope_mul(p)
            if p % 2 == 1:
                start_attn(p // 2)
        pump_attn()

        if KERNEL_DEBUG:
            nc.sync.dma_start(dbg_qk[:, :], qk_sb[:, :])
            nc.sync.dma_start(dbg_rot[0:80, :], rot[0:80, :])
            nc.sync.dma_start(dbg_vaug[:, :], vaug[:, :])
            nc.sync.dma_start(dbg_attn[0:97, :], attn[0:97, :])

        ph1.__exit__(None, None, None)

        # normalize + dense re-pack of attn heads + split projection:
        # kt0-3 prepass overlaps the last heads' attention, kt4 finishes
        with ExitStack() as p5:
            late = p5.enter_context(tc.tile_pool(name="late", bufs=1))
            dense = late.tile([128, V3_NPJ * S], bf16, tag="dn", name="dn")
            ob_pool = p5.enter_context(tc.tile_pool(name="ob", bufs=3))
            oa_pool = p5.enter_context(tc.tile_pool(name="oa", bufs=1))
            dense3 = dense.rearrange("p (t s) -> p t s", t=V3_NPJ)
            wp3 = wp_sb.rearrange("p (t c) -> p t c", t=V3_NPJ)

            def emit_densify(h):
                for (dt, r, n, off) in _pieces(80 * h, 80):
                    nc.sync.dma_start(
                        dense3[r:r + n, dt, :],
                        attn[off:off + n, h * S:(h + 1) * S])

            for h in range(NH - 1):
                emit_norm(h)
                emit_densify(h)
            oa_tiles = []
            for j in range(NK):
                ps = qs_tile()
                for half in range(2):
                    c0 = half * 512
                    for kt in range(4):
                        nc.tensor.matmul(
                            ps[:, c0:c0 + 512],
                            wp3[:, kt, j * 128:(j + 1) * 128],
                            dense3[:, kt, c0:c0 + 512],
                            start=(kt == 0), stop=(kt == 3))
                oa = oa_pool.tile([128, S], f32, tag=f"oa{j}", name=f"oa{j}",
                                  bufs=1)
                if j % 2:
                    nc.scalar.activation(oa[:, :], ps[:, :], AF.Identity)
                else:
                    nc.vector.tensor_copy(oa[:, :], ps[:, :])
                oa_tiles.append(oa)
            emit_norm(NH - 1)
            emit_densify(NH - 1)
            for j in range(NK):
                ps = qs_tile()
                for half in range(2):
                    c0 = half * 512
                    nc.tensor.matmul(ps[:, c0:c0 + 512],
                                     wp3[:, 4, j * 128:(j + 1) * 128],
                                     dense3[:, 4, c0:c0 + 512],
                                     start=True, stop=True)
                ob = ob_pool.tile([128, S], bf16, tag="ob", name="ob")
                nc.vector.tensor_add(ob[:, :], ps[:, :], oa_tiles[j][:, :])
                nc.sync.dma_start(outT[j * 128:(j + 1) * 128, :], ob[:, :])

    nc.compile()
    return nc


def _pack_v3(Wqkv, bqkv, Wproj, bproj, g):
    """Host-side per-head-group weight packing for the v3 program."""
    import concourse.mybir as mybir
    bf16 = mybir.dt.np(mybir.dt.bfloat16)
    NH, NJ, NK, VW = V3_NH, V3_NJ, V3_NK, V3_VW

    Wp = np.zeros((NJ * 128, D), np.float32)
    bp = np.zeros((NJ * 128,), np.float32)
    for m in range(32):
        h = m // 4
        sec = (m % 4) // 2       # 0 = q, 1 = k
        half = m % 2
        src = sec * D + (g * NH + h) * HD + half * BLK
        w = Wqkv[src:src + BLK, :]
        b = bqkv[src:src + BLK]
        if sec == 0:
            w = w * SCALE
            b = b * SCALE
        t, r = _half_pos(m)
        Wp[t * 128 + r:t * 128 + r + BLK] = w
        bp[t * 128 + r:t * 128 + r + BLK] = b
    # lhsT layout [128, j, k, 128]: wqkT[p, j, k, c] = Wp[j*128+c, k*128+p]
    wqkT = np.ascontiguousarray(
        Wp.reshape(NJ, 128, NK, 128).transpose(3, 0, 2, 1)
        .reshape(128, NJ * NK * 128)).astype(bf16)
    bias2d = np.ascontiguousarray(bp.reshape(NJ, 128).T)

    Wv = Wqkv[2 * D + g * 640:2 * D + (g + 1) * 640, :]
    wvT = _tile_rows(np.ascontiguousarray(Wv.T)).astype(bf16)
    bv = bqkv[2 * D + g * 640:2 * D + (g + 1) * 640]

    vt = np.zeros((128, NH * VW), np.float32)
    for h in range(NH):
        vt[:, h * VW:h * VW + HD] = bv[h * HD:(h + 1) * HD][None, :]
        vt[:, h * VW + 96] = 1.0
    vtmpl = vt.astype(bf16)

    Wpd = Wproj[:, g * 640:(g + 1) * 640].T  # [640, 1280] dense attn rows
    wprojT = _tile_rows(np.ascontiguousarray(Wpd)).astype(bf16)
    return wqkT, bias2d, wvT, vtmpl, wprojT


def _cos_sin_v3(cos, sin):
    """Dense [0:80] rope coefficient layouts (bf16), full sequence."""
    import concourse.mybir as mybir
    bf16 = mybir.dt.np(mybir.dt.bfloat16)
    S = cos.shape[0]
    cp = np.zeros((128, S), np.float32)
    sp = np.zeros((128, S), np.float32)
    cp[0:BLK] = cos.T[0:BLK]
    cp[BLK:HD] = cos.T[BLK:HD]
    sp[0:BLK] = -sin.T[0:BLK]
    sp[BLK:HD] = sin.T[BLK:HD]
    return cp.astype(bf16), sp.astype(bf16)


def kernel(hidden_states, cos, sin, Wqkv, bqkv, Wproj, bproj, cu_seqlens):
    sys.path.insert(0, "/opt/trn_rl_repo")
    from concourse import bass_utils

    hidden_states = np.asarray(hidden_states, np.float32)
    cos = np.asarray(cos, np.float32)
    sin = np.asarray(sin, np.float32)
    Wqkv = np.asarray(Wqkv, np.float32)
    bqkv = np.asarray(bqkv, np.float32)
    Wproj = np.asarray(Wproj, np.float32)
    bproj = np.asarray(bproj, np.float32)

    S, D_ = hidden_states.shape
    assert D_ == D
    segs = _segments(cu_seqlens, S)
    uniform = (S % 4 == 0) and segs == [(i * S // 4, (i + 1) * S // 4)
                                        for i in range(4)]

    hiddenT = np.ascontiguousarray(hidden_states.T)
    cosP, sin2P = _pack_cos_sin(cos, sin)

    def _vinit(segs_local):
        n_tt = sum(-(-(e - a) // 128) for a, e in segs_local)
        v = np.zeros((128, n_tt, 17), np.float32)
        v[:, :, 16] = 1.0
        return np.ascontiguousarray(v.reshape(128, n_tt * 17))

    if uniform:
        # v3: 2 head-groups x 4 segments, bf16 on-chip
        import concourse.mybir as mybir
        bf16 = mybir.dt.np(mybir.dt.bfloat16)
        S_core = S // 4
        key = ("V3", S)
        if key not in _CACHE:
            _CACHE[key] = _build_v3()
        nc = _CACHE[key]
        cosPd, sinPd = _cos_sin_v3(cos, sin)
        hidT_b = hiddenT.astype(bf16)
        in_maps = []
        meta = []
        for g in range(2):
            wqkT, b2, wvT, vtmpl, wprojT = _pack_v3(Wqkv, bqkv, Wproj,
                                                    bproj, g)
            for s in range(4):
                sl = slice(s * S_core, (s + 1) * S_core)
                in_maps.append({
                    "hidT": _tile_rows(hidT_b[:, sl]),
                    "wqkT": wqkT,
                    "bias2d": b2,
                    "wvT": wvT,
                    "vtmpl": vtmpl,
                    "cosP": np.ascontiguousarray(cosPd[:, sl]),
                    "sinP": np.ascontiguousarray(sinPd[:, sl]),
                    "wprojT": wprojT,
                })
                meta.append((g, s))
        res = bass_utils.run_bass_kernel_spmd(nc, in_maps,
                                              core_ids=list(range(N_CORES)))
        out = np.zeros((D, S), np.float32)
        for c, (g, s) in enumerate(meta):
            out[:, s * S_core:(s + 1) * S_core] += \
                res.results[c]["outT"].astype(np.float32)
    else:
        # mode C: 8-way head parallel, full sequence per core
        n_h, S_core = H // N_CORES, S
        key = ("C", S, tuple(np.asarray(cu_seqlens).tolist()))
        if key not in _CACHE:
            _CACHE[key] = _build_program(n_h, S_core, segs,
                                         resident_hidden=False)
        nc = _CACHE[key]
        vinit = _vinit(segs)
        hid_tiled = _tile_rows(hiddenT)
        in_maps = []
        for c in range(N_CORES):
            heads = list(range(c * n_h, (c + 1) * n_h))
            wt, b2 = _pack_w(Wqkv, bqkv, heads, n_h)
            in_maps.append({
                "hiddenT": hid_tiled,
                "wqkvT": wt,
                "bias2d": b2,
                "cosP": cosP,
                "sin2P": sin2P,
                "wprojT": _pack_wproj(Wproj, heads),
                "vinit": vinit,
            })
        res = bass_utils.run_bass_kernel_spmd(nc, in_maps,
                                              core_ids=list(range(N_CORES)))
        out = np.zeros((D, S), np.float32)
        for c in range(N_CORES):
            out += res.results[c]["outT"]

    return np.ascontiguousarray(out.T) + bproj[None, :]



# revision 91
# speedup vs baseline: 1.3481x; 1.0144x over previous
"""Trainium2 Bass kernel for Ernie4.5-VL vision attention (ragged segments).

Contract: kernel(**inputs) takes the FULL unsharded inputs (keyed as in
setup_inputs()) and returns the FULL [S, D] float32 output.

Fast path (uniform 4x1024 segments) — the v3 program, see _build_v3:
2 head-groups x 4 segments across 8 cores, bf16 on-chip, ~196us. Key
ideas: dense q/k weight packing (11 j-tiles), v computed untransposed
straight into the PV operand slots, RoPE staged via DMA into dense
[0:80] layouts (K=80 score contraction), 512-wide exp units on
dedicated PSUM slots, ones-column denominator trick, and a projection
over 5 dense re-packed K-tiles split into a kt0-3 prepass + kt4 finish.
Scheduling: emission order defines both Tile dependency direction and
scheduler priority, so readers are never emitted before their writers,
latency-bound chains (normalize) are kept off the compute-critical
FIFOs, and a PE warm-up bridges the load phase (the cost model prices
the PE p-state at dispatch time).

Fallback (any other cu_seqlens): the original fp32r program below —
8-way head parallel, every core sees all segments.

Host does only O(S*D) glue: input transposes/packing bf16 conversion,
summing the per-token partial projections, and the bias adds.
"""

import os
import sys

import numpy as np

H = 16
HD = 80
BLK = 40  # rotate_half half-width
SCALE = HD ** -0.5
N_CORES = 8
D = 1280
NK = D // 128  # contraction tiles for the qkv matmul
ATTN_STRIDE = 96  # head row pitch in the packed attention output
MM_DT_NAME = os.environ.get("KERNEL_MM_DT", "float32r")  # or "float32"
KERNEL_DEBUG = bool(int(os.environ.get("KERNEL_DEBUG", "0")))


def _segments(cu_seqlens, S):
    """Intervals matching reference's searchsorted(cu[1:], i, 'right')."""
    b = np.clip(np.sort(np.asarray(cu_seqlens, dtype=np.int64)[1:5]), 0, S)
    bounds = [0] + list(b) + [S]
    segs = []
    for a, e in zip(bounds[:-1], bounds[1:]):
        if e > a:
            segs.append((int(a), int(e)))
    return segs


def _pack_layout(n_h):
    """Pack per-core qkv dims as 40-row blocks, 3 per 128-row tile (8 pad).

    Each tile holds one v-block at row 0 (PE transpose operands must start
    at a 32-aligned partition) and two q/k blocks at rows 40 and 80.
    Returns pos[(sec, h, half)] = (tile, row) and the number of tiles.
    """
    ntiles = 2 * n_h
    pos = {}
    for h in range(n_h):
        for half in (0, 1):
            pos[("v", h, half)] = (2 * h + half, 0)
    qk = [("q", h, half) for h in range(n_h) for half in (0, 1)]
    qk += [("k", h, half) for h in range(n_h) for half in (0, 1)]
    for j, blk in enumerate(qk):
        pos[blk] = (j // 2, BLK + BLK * (j % 2))
    return pos, ntiles


def _pieces(start, length, tile_rows=128):
    """Split global row range [start, start+length) into per-tile pieces."""
    out = []
    off = 0
    while off < length:
        g = start + off
        t, r = g // tile_rows, g % tile_rows
        n = min(tile_rows - r, length - off)
        out.append((t, r, n, off))
        off += n
    return out


def _proj_k_tiles(n_h):
    rows = ATTN_STRIDE * n_h
    kt = [128] * (rows // 128)
    if rows % 128:
        kt.append(rows % 128)
    return kt


def _build_program(n_h, S_core, segs_local, resident_hidden):
    """Emit the SPMD program. Same structure for every core.

    Engine-AP partition rules on TRN2 (walrus birverifier): compute-engine
    accesses must start at a 32-aligned partition and must not cross a
    64-boundary unless they start on one; cross-partition data movement
    must go through DMA. The layout choices below all follow from this.
    """
    import concourse.mybir as mybir
    import concourse.tile as tile
    from concourse import bacc
    from concourse.masks import make_identity
    from contextlib import ExitStack

    f32 = mybir.dt.float32
    mm_dt = getattr(mybir.dt, MM_DT_NAME)
    AF = mybir.ActivationFunctionType

    k_proj = n_h
    pos, n_mtiles = _pack_layout(n_h)
    dims_pad = n_mtiles * 128
    VW = 97  # v_aug slot width: 80 v dims + 16 zero pad + ones col at 96

    # global key-tile list: (seg_idx, t0, t1)
    t_tiles = []
    for si, (a, e) in enumerate(segs_local):
        t = a
        while t < e:
            t_tiles.append((si, t, min(t + 128, e)))
            t += 128
    n_tt = len(t_tiles)

    nc = bacc.Bacc("TRN2", target_bir_lowering=False, debug=False,
                   enable_asserts=False, num_devices=N_CORES)

    # host supplies hiddenT/wqkvT pre-tiled into 128-partition-major layout
    hiddenT = nc.dram_tensor("hiddenT", [128, NK * S_core], mm_dt,
                             kind="ExternalInput").ap()
    wqkvT = nc.dram_tensor("wqkvT", [128, NK * dims_pad], mm_dt,
                           kind="ExternalInput").ap()
    bias2d = nc.dram_tensor("bias2d", [128, n_mtiles], f32,
                            kind="ExternalInput").ap()
    # cosP/sin2P are host-packed [128, S]: rows 0:40 and 64:104 hold the
    # lo/hi rope coefficients, all other rows zero (zeroes the junk rows
    # of the rotated q/k so the K=104 score matmuls see exact zeros).
    cosP = nc.dram_tensor("cosP", [128, S_core], mm_dt,
                          kind="ExternalInput").ap()
    sin2P = nc.dram_tensor("sin2P", [128, S_core], mm_dt,
                           kind="ExternalInput").ap()
    wprojT = nc.dram_tensor("wprojT", [n_h * HD, D], mm_dt,
                            kind="ExternalInput").ap()
    # per-key-tile v_aug tail init: 16 zero pad cols + ones col (f32r memset
    # fails walrus codegen, so this comes in via DMA)
    vinit = nc.dram_tensor("vinit", [128, n_tt * (VW - HD)], mm_dt,
                           kind="ExternalInput").ap()
    outT = nc.dram_tensor("outT", [D, S_core], f32, kind="ExternalOutput").ap()
    if KERNEL_DEBUG:
        dbg_qkv = nc.dram_tensor("dbg_qkv", [128, n_mtiles * S_core], f32,
                                 kind="ExternalOutput").ap()
        dbg_rot = nc.dram_tensor("dbg_rot", [128, 2 * n_h * S_core], f32,
                                 kind="ExternalOutput").ap()
        dbg_vaug = nc.dram_tensor("dbg_vaug", [128, n_h * n_tt * VW], f32,
                                  kind="ExternalOutput").ap()
        dbg_attn = nc.dram_tensor("dbg_attn", [128, n_h * S_core], f32,
                                  kind="ExternalOutput").ap()

    def r_(ap):
        return ap.bitcast(mm_dt)

    BC = 1024  # psum tile width (2 banks); matmuls stream <=512
    big_chunks = [(c, min(c + BC, S_core)) for c in range(0, S_core, BC)]

    def halves(c0, c1):
        out = []
        q = c0
        while q < c1:
            out.append((q, min(q + 512, c1)))
            q = q + 512
        return out

    with tile.TileContext(nc) as tc, ExitStack() as ctx:
        persist = ctx.enter_context(tc.tile_pool(name="persist", bufs=1))
        ident = persist.tile([128, 128], f32, tag="ident", name="ident")
        make_identity(nc, ident[:])
        bias_sb = persist.tile([128, n_mtiles], f32, tag="bias", name="bias")
        nc.sync.dma_start(bias_sb[:], bias2d[:])

        # PSUM: two 2-bank slots (t0/t1) shared by qkv/scores/proj, two
        # 1-bank slots for v-transposes, one 2-bank slot for PV accumulate
        psum_all_cm = tc.tile_pool(name="psum_all", bufs=1, space="PSUM")
        psum_all = psum_all_cm.__enter__()
        # big pool: qkvT tiles (phases 1-3), slots reused by attn (phases 4-5)
        qkv_pool = ctx.enter_context(tc.tile_pool(name="big", bufs=1))
        qkv_sb = [qkv_pool.tile([128, S_core], mm_dt, tag=f"qkvT{j}",
                                name=f"qkvT{j}") for j in range(n_mtiles)]
        # rope output (rows 0:104 live, 40:64 zeroed via cosP/sin2P pads)
        rot_cm = tc.tile_pool(name="rot", bufs=1)
        rv = rot_cm.__enter__()
        rot_sb = {}
        for h in range(n_h):
            for sec in ("q", "k"):
                rot_sb[(sec, h)] = rv.tile([128, S_core], mm_dt,
                                           tag=f"rot_{sec}{h}",
                                           name=f"rot_{sec}{h}")
        RC = 1024
        rope_cm = tc.tile_pool(name="rope_scr", bufs=2)
        rope_scr = rope_cm.__enter__()

        # ------------ phase 1: qkvT = Wpack @ hidden.T --------------
        with ExitStack() as p1:
            hidden3 = hiddenT.rearrange("p (k s) -> p k s", k=NK)
            w3 = wqkvT.rearrange("p (k m) -> p k m", k=NK)
            if resident_hidden:
                hid_pool = p1.enter_context(tc.tile_pool(name="hid", bufs=1))
                w_pool = p1.enter_context(tc.tile_pool(name="wstream", bufs=3))
                hid_sb = [hid_pool.tile([128, S_core], mm_dt, tag=f"hid{k}",
                                        name=f"hid{k}") for k in range(NK)]
                wj0 = w_pool.tile([128, NK * 128], mm_dt, tag="wj", name="wj")
                nc.sync.dma_start(hid_sb[0][:], hidden3[:, 0, :])
                nc.sync.dma_start(
                    wj0.rearrange("p (k m) -> p k m", k=NK)[:, :, :],
                    w3[:, :, 0:128])
                for k in range(1, NK):
                    nc.sync.dma_start(hid_sb[k][:], hidden3[:, k, :])
                for j in range(n_mtiles):
                    if j == 0:
                        wj = wj0
                    else:
                        wj = w_pool.tile([128, NK * 128], mm_dt, tag="wj",
                                         name="wj")
                        nc.sync.dma_start(
                            wj.rearrange("p (k m) -> p k m", k=NK)[:, :, :],
                            w3[:, :, j * 128:(j + 1) * 128])
                    for (h0, h1) in halves(0, S_core):
                        hw = h1 - h0
                        ps = psum_all.tile([128, 512], f32,
                                           tag=f"t{(h0 // 512) % 2}",
                                           name="qkvp")
                        for k in range(NK):
                            nc.tensor.matmul(
                                ps[:, :hw],
                                r_(wj[:, k * 128:(k + 1) * 128]),
                                r_(hid_sb[k][:, h0:h1]),
                                start=(k == 0), stop=(k == NK - 1))
                        nc.scalar.activation(qkv_sb[j][:, h0:h1], ps[:, :hw],
                                             AF.Identity,
                                             bias=bias_sb[:, j:j + 1])
            else:
                # k-outer streaming: two psum slots hold four j-streams
                # (columns 0:512 and 512:1024), hidden tiles are tiny
                w_pool = p1.enter_context(tc.tile_pool(name="wres", bufs=1))
                w_sb = [w_pool.tile([128, dims_pad], mm_dt, tag=f"w{k}",
                                    name=f"w{k}") for k in range(NK)]
                for k in range(NK):
                    nc.sync.dma_start(w_sb[k][:], w3[:, k, :])
                assert n_mtiles == 4
                hid_pool = p1.enter_context(tc.tile_pool(name="hidstream",
                                                         bufs=3))
                for (h0, h1) in halves(0, S_core):
                    hw = h1 - h0
                    ps01 = psum_all.tile([128, BC], f32, tag="t0", name="ps01")
                    ps23 = psum_all.tile([128, BC], f32, tag="t1", name="ps23")
                    pj_of = lambda j: (ps01 if j < 2 else ps23,
                                       (j % 2) * 512)
                    for k in range(NK):
                        ht = hid_pool.tile([128, 512], mm_dt, tag="hidc",
                                           name="hidc")
                        nc.sync.dma_start(ht[:, :hw], hidden3[:, k, h0:h1])
                        for j in range(n_mtiles):
                            psj, co = pj_of(j)
                            nc.tensor.matmul(
                                psj[:, co:co + hw],
                                r_(w_sb[k][:, j * 128:(j + 1) * 128]),
                                r_(ht[:, :hw]),
                                start=(k == 0), stop=(k == NK - 1))
                    for j in range(n_mtiles):
                        psj, co = pj_of(j)
                        nc.scalar.activation(qkv_sb[j][:, h0:h1],
                                             psj[:, co:co + hw], AF.Identity,
                                             bias=bias_sb[:, j:j + 1])

        psum_all_cm.__exit__(None, None, None)
        ps_att = ctx.enter_context(tc.tile_pool(name="ps_att", bufs=1,
                                                space="PSUM"))

        # ------------ phase 2: RoPE --------------------------------
        # DMA-stage lo/hi into 0:40 / 64:104 (stgA) and swapped (stgB),
        # then rot = stgA*cosP + stgB*sin2P as three same-base wide ops.
        # double-buffered persistent staging tensors; rows 40:64 zeroed once
        # from cosP's zero rows so the [0:104) products read defined zeros
        stg = {}
        for nm in ("sa0", "sa1", "sb0", "sb1"):
            stg[nm] = rope_scr.tile([128, RC], mm_dt, tag=nm, name=nm, bufs=1)
        pair_i = 0
        for ci, f0 in enumerate(range(0, S_core, RC)):
            f1 = min(f0 + RC, S_core)
            fs = f1 - f0
            cos_sb = rope_scr.tile([128, RC], mm_dt, tag="cos", name="cos",
                                   bufs=1)
            sin_sb = rope_scr.tile([128, RC], mm_dt, tag="sin", name="sin",
                                   bufs=1)
            nc.scalar.dma_start(cos_sb[:, :fs], cosP[:, f0:f1])
            nc.scalar.dma_start(sin_sb[:, :fs], sin2P[:, f0:f1])
            if ci == 0:
                for nm in stg:
                    nc.scalar.dma_start(stg[nm][BLK:64, :], cos_sb[BLK:64, :])
            for h in range(n_h):
                for sec in ("q", "k"):
                    lo_t, lo_r = pos[(sec, h, 0)]
                    hi_t, hi_r = pos[(sec, h, 1)]
                    assert hi_t == lo_t and hi_r == lo_r + BLK
                    x = qkv_sb[lo_t]
                    dst = rot_sb[(sec, h)]
                    stga = stg[f"sa{pair_i % 2}"]
                    stgb = stg[f"sb{pair_i % 2}"]
                    nc.scalar.dma_start(stga[0:BLK, :fs],
                                        x[lo_r:lo_r + BLK, f0:f1])
                    nc.scalar.dma_start(stga[64:64 + BLK, :fs],
                                        x[hi_r:hi_r + BLK, f0:f1])
                    nc.scalar.dma_start(stgb[0:BLK, :fs],
                                        x[hi_r:hi_r + BLK, f0:f1])
                    nc.scalar.dma_start(stgb[64:64 + BLK, :fs],
                                        x[lo_r:lo_r + BLK, f0:f1])
                    nc.vector.tensor_mul(dst[0:104, f0:f1], stga[0:104, :fs],
                                         cos_sb[0:104, :fs])
                    eng = nc.gpsimd if pair_i % 2 == 0 else nc.vector
                    eng.tensor_mul(stgb[0:104, :fs], stgb[0:104, :fs],
                                   sin_sb[0:104, :fs])
                    nc.vector.tensor_add(dst[0:104, f0:f1], dst[0:104, f0:f1],
                                         stgb[0:104, :fs])
                    pair_i += 1
        rope_cm.__exit__(None, None, None)

        # v_aug tiles + per-head emitter (invoked right after each head's
        # rope so attention unblocks head by head)
        vaug_cm = tc.tile_pool(name="vaug", bufs=1)
        vaug_pool = vaug_cm.__enter__()
        vaug_sb = [vaug_pool.tile([128, n_tt * VW], mm_dt, tag=f"vaug{h}",
                                  name=f"vaug{h}") for h in range(n_h)]
        vinit3 = vinit.rearrange("p (t c) -> p t c", c=VW - HD)
        for h in range(n_h):
            nc.sync.dma_start(
                vaug_sb[h].rearrange("p (t c) -> p t c", c=VW)[:, :, HD:VW],
                vinit3[:, :, :])
        GRP = 4  # key tiles transposed per psum tile / copy (1 psum bank)

        def emit_vaug(h):
            gi = 0
            while gi < n_tt:
                hi_g = min(gi + GRP, n_tt)
                if all(t_tiles[g][2] - t_tiles[g][1] == 128
                       for g in range(gi, hi_g)):
                    grp = list(range(gi, hi_g))
                else:
                    grp = [gi]
                ng = len(grp)
                tp = ps_att.tile([128, GRP * HD], f32, tag="tp", name="tp")
                for x, g in enumerate(grp):
                    si, t0, t1 = t_tiles[g]
                    sz = t1 - t0
                    for half in (0, 1):
                        vt, vr = pos[("v", h, half)]
                        nc.tensor.transpose(
                            tp[:sz, x * HD + half * BLK:
                               x * HD + (half + 1) * BLK],
                            qkv_sb[vt][0:BLK, t0:t1].bitcast(f32),
                            ident[:BLK, :BLK])
                sz0 = t_tiles[grp[0]][2] - t_tiles[grp[0]][1]
                dst = vaug_sb[h].rearrange("p (t c) -> p t c", c=VW)
                src_ap = tp.rearrange("p (t c) -> p t c", c=HD)
                if h % 2 == 0:
                    nc.vector.tensor_copy(dst[:sz0, grp[0]:grp[0] + ng, 0:HD],
                                          src_ap[:sz0, 0:ng, :])
                else:
                    nc.scalar.activation(dst[:sz0, grp[0]:grp[0] + ng, 0:HD],
                                         src_ap[:sz0, 0:ng, :], AF.Identity)
                gi += ng




        if KERNEL_DEBUG:
            for j in range(n_mtiles):
                nc.sync.dma_start(
                    dbg_qkv[:, j * S_core:(j + 1) * S_core],
                    qkv_sb[j][:].bitcast(f32))
            i_ = 0
            for h in range(n_h):
                for sec in ("q", "k"):
                    nc.sync.dma_start(
                        dbg_rot[:, i_ * S_core:(i_ + 1) * S_core],
                        rot_sb[(sec, h)][:].bitcast(f32))
                    i_ += 1

        # ------------ phase 4: attention ----------------------------
        # one attn tile per head (rows 0:80) so every compute access is
        # partition-0 based; tiles reuse the dead qkvT slots
        attn_sb = [qkv_pool.tile([128, S_core], mm_dt, tag=f"qkvT{h}",
                                 name=f"attnT{h}") for h in range(n_h)]

        seg_ttiles = {}
        for ti, (si, t0, t1) in enumerate(t_tiles):
            seg_ttiles.setdefault(si, []).append((ti, t0, t1))

        BA = 512  # attention query-chunk width (1-bank psum slots)
        with ExitStack() as p4:
            pt_pool = p4.enter_context(tc.tile_pool(name="pt", bufs=3))
            nrm_pool = p4.enter_context(tc.tile_pool(name="nrm", bufs=2))
            unit_box = [0]

            def emit_attention(h, si, a, e):
                qT = rot_sb[("q", h)]
                kT = rot_sb[("k", h)]
                q = a
                while q < e:
                    q0, q1 = q, min(q + BA, e)
                    qs = q1 - q0
                    po = ps_att.tile([128, BA], f32,
                                     tag=f"po{unit_box[0] % 2}", name="pv")
                    tts = seg_ttiles[si]
                    for idx, (ti, t0, t1) in enumerate(tts):
                        sz = t1 - t0
                        ps = ps_att.tile([128, BA], f32, tag=f"st{idx % 2}",
                                         name="st")
                        nc.tensor.matmul(ps[:sz, :qs], r_(kT[0:104, t0:t1]),
                                         r_(qT[0:104, q0:q1]),
                                         start=True, stop=True)
                        pt = pt_pool.tile([128, BA], mm_dt, tag="pt", name="pt")
                        nc.scalar.activation(pt[:sz, :qs], ps[:sz, :qs], AF.Exp)
                        nc.tensor.matmul(
                            po[:VW, :qs],
                            r_(vaug_sb[h][:sz, ti * VW:(ti + 1) * VW]),
                            r_(pt[:sz, :qs]),
                            start=(idx == 0), stop=(idx == len(tts) - 1))
                    # partition_broadcast ucode reads physical partition 0,
                    # so shift the denominator row 96 -> 0 via DMA
                    rc = nrm_pool.tile([128, BA], f32, tag="rc", name="rc")
                    nc.vector.tensor_copy(rc[96:97, :qs], po[96:97, :qs])
                    nc.sync.dma_start(rc[0:1, :qs], rc[96:97, :qs])
                    nc.vector.reciprocal(rc[0:1, :qs], rc[0:1, :qs])
                    bc = nrm_pool.tile([128, BA], mm_dt, tag="bc", name="bc")
                    nc.gpsimd.partition_broadcast(
                        bc[0:HD, :qs], rc[0:1, :qs].bitcast(mm_dt))
                    nc.vector.tensor_mul(attn_sb[h][0:HD, q0:q1],
                                         po[0:HD, :qs], bc[0:HD, :qs])
                    unit_box[0] += 1
                    q = q1

            if len(segs_local) == 1:
                a, e = segs_local[0]
                for h in range(n_h):
                    emit_vaug(h)
                    emit_attention(h, 0, a, e)
            else:
                for h in range(n_h):
                    emit_vaug(h)
                for si, (a, e) in enumerate(segs_local):
                    for h in range(n_h):
                        emit_attention(h, si, a, e)

        vaug_cm.__exit__(None, None, None)
        rot_cm.__exit__(None, None, None)

        # ------------ phase 5: projection partial -------------------
        with ExitStack() as p5:
            wp_pool = p5.enter_context(tc.tile_pool(name="wp", bufs=1))
            wp_sb = []
            for kt in range(k_proj):
                t = wp_pool.tile([HD, D], mm_dt, tag=f"wp{kt}", name=f"wp{kt}")
                nc.sync.dma_start(t[:], wprojT[kt * HD:(kt + 1) * HD, :])
                wp_sb.append(t)
            out_pool = p5.enter_context(tc.tile_pool(name="outsb", bufs=3))
            for (c0, c1) in big_chunks:
                cs = c1 - c0
                for j in range(D // 128):
                    ob = out_pool.tile([128, BC], f32, tag="ob", name="ob")
                    for (h0, h1) in halves(c0, c1):
                        ps = ps_att.tile([128, 512], f32, tag=f"st{j % 2}",
                                         name="pj")
                        for kt in range(k_proj):
                            nc.tensor.matmul(
                                ps[:, :h1 - h0],
                                r_(wp_sb[kt][:, j * 128:(j + 1) * 128]),
                                r_(attn_sb[kt][0:HD, h0:h1]),
                                start=(kt == 0), stop=(kt == k_proj - 1))
                        if j % 2 == 0:
                            nc.vector.tensor_copy(ob[:, h0 - c0:h1 - c0],
                                                  ps[:, :h1 - h0])
                        else:
                            nc.scalar.activation(ob[:, h0 - c0:h1 - c0],
                                                 ps[:, :h1 - h0], AF.Identity)
                    nc.sync.dma_start(outT[j * 128:(j + 1) * 128, c0:c1],
                                      ob[:, :cs])

    nc.compile()
    return nc


def _pack_w(Wqkv, bqkv, heads, n_h):
    """Per-core packed qkv weights (q rows pre-scaled).

    Returns wqkvT_tiled [128, NK*dims_pad] (k-major blocks of [128, dims_pad])
    and bias2d [128, n_mtiles]."""
    pos, n_mtiles = _pack_layout(n_h)
    dims_pad = n_mtiles * 128
    W = np.zeros((dims_pad, D), np.float32)
    b = np.zeros((dims_pad,), np.float32)
    sec_off = {"q": 0, "k": D, "v": 2 * D}
    for i, h in enumerate(heads):
        for sec in ("q", "k", "v"):
            for half in (0, 1):
                t, r = pos[(sec, i, half)]
                src = sec_off[sec] + h * HD + half * BLK
                w = Wqkv[src:src + BLK, :]
                bb = bqkv[src:src + BLK]
                if sec == "q":
                    w = w * SCALE
                    bb = bb * SCALE
                W[t * 128 + r:t * 128 + r + BLK] = w
                b[t * 128 + r:t * 128 + r + BLK] = bb
    w_tiled = _tile_rows(np.ascontiguousarray(W.T))
    bias2d = np.ascontiguousarray(b.reshape(n_mtiles, 128).T)
    return w_tiled, bias2d


def _tile_rows(x):
    """[R, C] with R = nk*128 -> [128, nk*C] k-major tiling."""
    R, C = x.shape
    nk = R // 128
    return np.ascontiguousarray(
        x.reshape(nk, 128, C).transpose(1, 0, 2).reshape(128, nk * C))


def _pack_wproj(Wproj, heads):
    """Rows of Wproj.T for this core's head dims, stacked per head."""
    W = np.zeros((len(heads) * HD, Wproj.shape[0]), np.float32)
    for i, h in enumerate(heads):
        W[i * HD:(i + 1) * HD] = Wproj[:, h * HD:(h + 1) * HD].T
    return W


def _pack_cos_sin(cos, sin):
    """cosP/sin2P [128, S]: lo coeffs at rows 0:40, hi at 64:104, rest 0.

    sin2P row signs match rot = x*cosP + swap(x)*sin2P: lo rows hold
    -sin_lo (they multiply x_hi), hi rows hold +sin_hi (they multiply x_lo).
    """
    S = cos.shape[0]
    cosP = np.zeros((128, S), np.float32)
    sinP = np.zeros((128, S), np.float32)
    cosP[0:BLK] = cos.T[0:BLK]
    cosP[64:64 + BLK] = cos.T[BLK:HD]
    sinP[0:BLK] = -sin.T[0:BLK]
    sinP[64:64 + BLK] = sin.T[BLK:HD]
    return cosP, sinP


_CACHE = {}

# ---------------------------------------------------------------------------
# v3 fast path (uniform 4x1024 segments): 2 head-groups x 4 segments SPMD.
#
# Per core: 8 heads, 1024 tokens, one segment. All on-chip data bf16 except
# PSUM (f32) and the normalization scalars (f32).
#   - q/k packed dense: 32 40-row halves, 3 per 128-row tile (11 j-tiles)
#   - v computed untransposed ([tokens, vdim]) straight into PV operand slots
#   - RoPE: DMA-stage [lo;hi]/[hi;lo] into dense [0:80] layouts, 3 DVE ops
#     at bf16 2x rate; scores contract K=80 (no zero padding rows)
#   - scores^T per (head, key-tile) into [128,1024] PSUM, one wide exp
#   - PV with ones column at slot col 96 -> denominators at PSUM row 96
#   - projection over 5 dense 128-row K-tiles (attn heads re-packed via DMA)
# ---------------------------------------------------------------------------

V3_S = 1024     # tokens per core
V3_NH = 8       # heads per core
V3_NJ = 11      # dense qk j-tiles (3 x 40-row halves each, 8 junk rows)
V3_NK = 10      # contraction tiles (D / 128)
V3_NT = 8       # token/key tiles (S / 128)
V3_VW = 97      # v slot: 80 v dims + 16 zero pad + ones col at 96
V3_NPJ = 5      # dense proj k-tiles (8 heads * 80 / 128)


def _half_pos(m):
    """Packed position of 40-row half m: (j_tile, row in {0, 40, 80})."""
    return m // 3, 40 * (m % 3)


def _build_v3():
    import concourse.mybir as mybir
    import concourse.tile as tile
    from concourse import bacc
    from contextlib import ExitStack

    f32 = mybir.dt.float32
    bf16 = mybir.dt.bfloat16
    AF = mybir.ActivationFunctionType
    S, NH, NJ, NK, NT, VW = V3_S, V3_NH, V3_NJ, V3_NK, V3_NT, V3_VW

    nc = bacc.Bacc("TRN2", target_bir_lowering=False, debug=False,
                   enable_asserts=False, num_devices=N_CORES)

    hidT = nc.dram_tensor("hidT", [128, NK * S], bf16,
                          kind="ExternalInput").ap()
    wqkT = nc.dram_tensor("wqkT", [128, NJ * NK * 128], bf16,
                          kind="ExternalInput").ap()
    bias2d = nc.dram_tensor("bias2d", [128, NJ], f32,
                            kind="ExternalInput").ap()
    wvT = nc.dram_tensor("wvT", [128, NK * 640], bf16,
                         kind="ExternalInput").ap()
    vtmpl = nc.dram_tensor("vtmpl", [128, NH * VW], bf16,
                           kind="ExternalInput").ap()
    cosP = nc.dram_tensor("cosP", [128, S], bf16, kind="ExternalInput").ap()
    sinP = nc.dram_tensor("sinP", [128, S], bf16, kind="ExternalInput").ap()
    wprojT = nc.dram_tensor("wprojT", [128, V3_NPJ * D], bf16,
                            kind="ExternalInput").ap()
    outT = nc.dram_tensor("outT", [D, S], bf16, kind="ExternalOutput").ap()
    if KERNEL_DEBUG:
        dbg_qk = nc.dram_tensor("dbg_qk", [128, NJ * S], bf16,
                                kind="ExternalOutput").ap()
        dbg_rot = nc.dram_tensor("dbg_rot", [128, 16 * S], bf16,
                                 kind="ExternalOutput").ap()
        dbg_vaug = nc.dram_tensor("dbg_vaug", [128, NT * NH * VW], bf16,
                                  kind="ExternalOutput").ap()
        dbg_attn = nc.dram_tensor("dbg_attn", [128, NH * S], bf16,
                                  kind="ExternalOutput").ap()

    hid3 = hidT.rearrange("p (k s) -> p k s", k=NK)
    wqk4 = wqkT.rearrange("p (j k c) -> p j k c", j=NJ, k=NK)
    wv3 = wvT.rearrange("p (k c) -> p k c", k=NK)

    with tile.TileContext(nc) as tc, ExitStack() as ctx:
        persist = ctx.enter_context(tc.tile_pool(name="persist", bufs=1))
        bias_sb = persist.tile([128, NJ], f32, tag="bias", name="bias")
        cos_sb = persist.tile([128, S], bf16, tag="cos", name="cos")
        sin_sb = persist.tile([128, S], bf16, tag="sin", name="sin")
        vt_sb = persist.tile([128, NH * VW], bf16, tag="vt", name="vt")
        pass

        psum = ctx.enter_context(tc.tile_pool(name="psum", bufs=1,
                                              space="PSUM"))
        unit = [0]

        def qs_tile():
            t = psum.tile([128, 1024], f32, tag=f"qs{unit[0] % 2}", name="qs")
            unit[0] += 1
            return t

        qk_pool = ctx.enter_context(tc.tile_pool(name="qk", bufs=1))
        qk_sb = qk_pool.tile([128, NJ * S], bf16, tag="qk", name="qk")
        rot_pool = ctx.enter_context(tc.tile_pool(name="rotp", bufs=1))
        rot = rot_pool.tile([128, 16 * S], bf16, tag="rot", name="rot")
        stgb_pool = ctx.enter_context(tc.tile_pool(name="stgb", bufs=6))
        vaug_pool = ctx.enter_context(tc.tile_pool(name="vaug", bufs=1))
        vaug = vaug_pool.tile([128, NT * NH * VW], bf16, tag="va", name="va")
        vaug3 = vaug.rearrange("p (t h c) -> p t h c", t=NT, h=NH)
        attn_pool = ctx.enter_context(tc.tile_pool(name="attn", bufs=1))
        # unnormalized PV output incl. bf16 denominator row at partition 96;
        # normalized in place at the end
        attn = attn_pool.tile([128, NH * S], bf16, tag="at", name="at")
        pt_pool = ctx.enter_context(tc.tile_pool(name="pt", bufs=12))
        rc_pool = ctx.enter_context(tc.tile_pool(name="rc", bufs=2))
        rcf_pool = ctx.enter_context(tc.tile_pool(name="rcf", bufs=2))
        bc_pool = ctx.enter_context(tc.tile_pool(name="bc", bufs=2))
        wp_pool = ctx.enter_context(tc.tile_pool(name="wp", bufs=1))
        wp_sb = wp_pool.tile([128, V3_NPJ * D], bf16, tag="wp", name="wp")

        # weights/hidden (dead after phase 1; proj pools reuse the space) —
        # entered last among open pools so the mid-program release is LIFO
        ph1 = tc.tile_pool(name="ph1", bufs=1)
        p1 = ph1.__enter__()
        hid_sb = p1.tile([128, NK * S], bf16, tag="hid", name="hid")
        wqk_sb = p1.tile([128, NJ * NK * 128], bf16, tag="wqk", name="wqk")
        wv_sb = p1.tile([128, NK * 640], bf16, tag="wv", name="wv")
        hid3s = hid_sb.rearrange("p (k s) -> p k s", k=NK)
        wqk4s = wqk_sb.rearrange("p (j k c) -> p j k c", j=NJ, k=NK)
        wv3s = wv_sb.rearrange("p (k c) -> p k c", k=NK)
        # PE warm-up: the cost model prices p-state at dispatch; keep the
        # PE busy on junk matmuls while the input loads stream so the real
        # matmuls dispatch against a warm (2.4 GHz) clock
        wa = p1.tile([128, 16], bf16, tag="wa", name="wa")
        wb = p1.tile([128, 256], bf16, tag="wb", name="wb")
        nc.vector.memset(wa[:, :], 0.0)
        nc.vector.memset(wb[:, :], 0.0)
        wps = psum.tile([128, 1024], f32, tag="po", name="warm")
        for _ in range(30):
            nc.tensor.matmul(wps[0:16, 0:256], wa[:, :], wb[:, :],
                             start=True, stop=True)
        nc.vector.tensor_copy(wa[0:16, 0:4], wps[0:16, 0:4])

        # loads on one queue in priority order; the tail weight loads are
        # deferred into the round loop (just-in-time) so their transfers
        # never sit ahead of the rope staging DMAs on the serial DMA device
        nc.sync.dma_start(vt_sb[:], vtmpl[:])
        nc.sync.dma_start(bias_sb[:], bias2d[:])
        nc.sync.dma_start(wqk4s[:, 0:3, :, :], wqk4[:, 0:3, :, :])
        nc.sync.dma_start(hid3s[:, 0:5, :], hid3[:, 0:5, :])
        nc.sync.dma_start(hid3s[:, 5:NK, :], hid3[:, 5:NK, :])
        nc.sync.dma_start(cos_sb[:], cosP[:])
        nc.sync.dma_start(sin_sb[:], sinP[:])
        # JIT loads must be EMITTED before their first consumer (emission
        # order defines dependency direction), while issuing late enough
        # that their transfers don't delay the rope staging DMAs
        load_at = {
            1: lambda: nc.sync.dma_start(wv3s[:, :, :], wv3[:, :, :]),
            2: lambda: nc.sync.dma_start(wqk4s[:, 3:6, :, :],
                                         wqk4[:, 3:6, :, :]),
            3: lambda: nc.sync.dma_start(wqk4s[:, 6:9, :, :],
                                         wqk4[:, 6:9, :, :]),
            4: lambda: nc.sync.dma_start(wqk4s[:, 9:NJ, :, :],
                                         wqk4[:, 9:NJ, :, :]),
            5: lambda: nc.sync.dma_start(wp_sb[:], wprojT[:]),
        }

        vt3 = vt_sb.rearrange("p (h c) -> p h c", h=NH)

        def emit_qk(j):
            ps = qs_tile()
            for half in range(2):
                c0 = half * 512
                for k in range(NK):
                    nc.tensor.matmul(ps[:, c0:c0 + 512],
                                     wqk4s[:, j, k, :],
                                     hid3s[:, k, c0:c0 + 512],
                                     start=(k == 0), stop=(k == NK - 1))
            if j < 5:
                # early copies on ACT (idle pre-exp): their rope staging
                # DMAs directly follow on the same queue, so the critical
                # startup rope chain never waits in a clogged FIFO
                nc.scalar.activation(qk_sb[:, j * S:(j + 1) * S], ps[:, :],
                                     AF.Identity, bias=bias_sb[:, j:j + 1])
            else:
                nc.vector.tensor_scalar_add(qk_sb[:, j * S:(j + 1) * S],
                                            ps[:, :], bias_sb[:, j:j + 1])

        def emit_v(tt):
            ps = qs_tile()
            for (c0, w) in ((0, 512), (512, 128)):
                for k in range(NK):
                    nc.tensor.matmul(ps[:, c0:c0 + w],
                                     hid3s[:, k, tt * 128:(tt + 1) * 128],
                                     wv3s[:, k, c0:c0 + w],
                                     start=(k == 0), stop=(k == NK - 1))
            src = ps[:, 0:NH * HD].rearrange("p (h c) -> p h c", c=HD)
            nc.vector.tensor_add(vaug3[:, tt, :, 0:HD], src[:, :, :],
                                 vt3[:, :, 0:HD])
            # pad + ones columns (disjoint from the v region written above)
            nc.vector.tensor_copy(vaug3[:, tt, :, HD:VW], vt3[:, :, HD:VW])

        qk3 = qk_sb.rearrange("p (j s) -> p j s", j=NJ)
        stgb_tiles = {}

        def emit_rope_dma(p):
            lo_t, lo_r = _half_pos(2 * p)
            hi_t, hi_r = _half_pos(2 * p + 1)
            b0 = p * S
            if p <= 6:
                eng = nc.scalar
            elif p >= 13:
                eng = nc.gpsimd
            else:
                eng = [nc.gpsimd, nc.sync][p % 2]
            if lo_t == hi_t and hi_r == lo_r + 40:
                eng.dma_start(rot[0:80, b0:b0 + S],
                              qk3[lo_r:lo_r + 80, lo_t, :])
            else:
                eng.dma_start(rot[0:40, b0:b0 + S],
                              qk3[lo_r:lo_r + 40, lo_t, :])
                eng.dma_start(rot[40:80, b0:b0 + S],
                              qk3[hi_r:hi_r + 40, hi_t, :])
            sb = stgb_pool.tile([128, S], bf16, tag="sb", name="sb")
            stgb_tiles[p] = sb
            eng.dma_start(sb[0:40, :], qk3[hi_r:hi_r + 40, hi_t, :])
            eng.dma_start(sb[40:80, :], qk3[lo_r:lo_r + 40, lo_t, :])

        def emit_rope_mul(p):
            b0 = p * S
            sb = stgb_tiles.pop(p)
            nc.vector.tensor_mul(rot[0:80, b0:b0 + S], rot[0:80, b0:b0 + S],
                                 cos_sb[0:80, :])
            meng = nc.gpsimd if p % 4 == 3 else nc.vector
            meng.tensor_mul(sb[0:80, :], sb[0:80, :], sin_sb[0:80, :])
            nc.vector.tensor_add(rot[0:80, b0:b0 + S], rot[0:80, b0:b0 + S],
                                 sb[0:80, :])

        # Attention pump. 512-wide score half-units (unit = (kt, half),
        # 16 per head) on dedicated 1-bank PSUM slots decouple the exp
        # stream from the qk/v slot rotation. Invariant: every emitted
        # instruction's dependencies (incl. slot predecessors) are emitted
        # before it — PVs of head h follow head h-1's evacuation (single po
        # slot), exps run ahead of PVs by at most PT_AHEAD pt tiles.
        PT_AHEAD = 10
        heads_q = []      # started heads, in order
        v_done = [0]
        st_unit = [0]
        outstanding = [0]

        def start_attn(h):
            heads_q.append({"h": h, "se": 0, "pv": 0, "po": None})

        def emit_st_exp(hs):
            h, u = hs["h"], hs["se"]
            kt, half = u // 2, u % 2
            pq, pk = 2 * h, 2 * h + 1
            c0 = half * 512
            st = psum.tile([128, 512], f32, tag=f"st{st_unit[0] % 2}",
                           name="st")
            st_unit[0] += 1
            nc.tensor.matmul(
                st[:, :],
                rot[0:80, pk * S + kt * 128:pk * S + (kt + 1) * 128],
                rot[0:80, pq * S + c0:pq * S + c0 + 512],
                start=True, stop=True)
            pt = pt_pool.tile([128, 512], bf16, tag="pt", name="pt")
            nc.scalar.activation(pt[:, :], st[:, :], AF.Exp)
            hs.setdefault("pts", []).append(pt)
            hs["se"] += 1
            outstanding[0] += 1

        def emit_pv(hs):
            h, u = hs["h"], hs["pv"]
            kt, half = u // 2, u % 2
            if hs["po"] is None:
                hs["po"] = psum.tile([128, 1024], f32, tag="po", name="po")
            c0 = half * 512
            nc.tensor.matmul(hs["po"][0:VW, c0:c0 + 512],
                             vaug3[:, kt, h, :],
                             hs["pts"][u][:, :],
                             start=(kt == 0), stop=(kt == NT - 1))
            hs["pv"] += 1
            outstanding[0] -= 1

        def pump_attn():
            progress = True
            while progress:
                progress = False
                if heads_q:
                    hs = heads_q[0]
                    while (hs["pv"] < hs["se"]
                           and hs["pv"] // 2 < v_done[0]):
                        emit_pv(hs)
                        progress = True
                    if hs["pv"] == 2 * NT:
                        h = hs["h"]
                        # evacuate on ACT so the po slot frees immediately
                        # and normalize never touches compute FIFOs
                        nc.scalar.activation(attn[0:VW, h * S:(h + 1) * S],
                                             hs["po"][0:VW, :], AF.Identity)
                        if h <= 5:
                            emit_norm(h)
                        heads_q.pop(0)
                        progress = True
                        continue
                for hs in heads_q:
                    while (hs["se"] < 2 * NT
                           and outstanding[0] < PT_AHEAD):
                        emit_st_exp(hs)
                        progress = True

        def emit_norm(h):
            rc = rc_pool.tile([1, S], bf16, tag="rc", name="rc")
            nc.scalar.dma_start(rc[0:1, :], attn[96:97, h * S:(h + 1) * S])
            rcf = rcf_pool.tile([1, S], f32, tag="rcf", name="rcf")
            nc.vector.reciprocal(rcf[0:1, :], rc[0:1, :])
            bc = bc_pool.tile([80, S], f32, tag="bc", name="bc")
            nc.gpsimd.partition_broadcast(bc[0:80, :], rcf[0:1, :])
            nc.vector.tensor_mul(attn[0:80, h * S:(h + 1) * S],
                                 attn[0:80, h * S:(h + 1) * S], bc[0:80, :])

        # emission driver: qk j-tiles paced with v, rope, attention
        v_at = {1: (0, 1), 2: (2, 3), 3: (4, 5), 4: (6, 7)}
        rope_at = {}
        for p in range(16):
            jmax = max(_half_pos(2 * p)[0], _half_pos(2 * p + 1)[0])
            rope_at.setdefault(jmax, []).append(p)
        pending = []
        for j in range(NJ):
            if j in load_at:
                load_at[j]()
            emit_qk(j)
            # rope muls one round behind their staging DMAs so the DVE FIFO
            # never stalls on in-flight DMA latency
            for p in pending:
                emit_rope_mul(p)
                if p % 2 == 1:
                    start_attn(p // 2)
            pending = []
            for tt in v_at.get(j, ()):
                emit_v(tt)
                v_done[0] += 1
            pump_attn()
            for p in rope_at.get(j, ()):
                emit_rope_dma(p)
                pending.append(p)
        for p in pending:
            emit_rope_mul(p)
            if p % 2 == 1:
                start_attn(p // 2)
        pump_attn()

        if KERNEL_DEBUG:
            nc.sync.dma_start(dbg_qk[:, :], qk_sb[:, :])
            nc.sync.dma_start(dbg_rot[0:80, :], rot[0:80, :])
            nc.sync.dma_start(dbg_vaug[:, :], vaug[:, :])
            nc.sync.dma_start(dbg_attn[0:97, :], attn[0:97, :])

        ph1.__exit__(None, None, None)

        # normalize + dense re-pack of attn heads + split projection:
        # kt0-3 prepass overlaps the last heads' attention, kt4 finishes
        with ExitStack() as p5:
            late = p5.enter_context(tc.tile_pool(name="late", bufs=1))
            dense = late.tile([128, V3_NPJ * S], bf16, tag="dn", name="dn")
            ob_pool = p5.enter_context(tc.tile_pool(name="ob", bufs=3))
            oa_pool = p5.enter_context(tc.tile_pool(name="oa", bufs=1))
            dense3 = dense.rearrange("p (t s) -> p t s", t=V3_NPJ)
            wp3 = wp_sb.rearrange("p (t c) -> p t c", t=V3_NPJ)

            def emit_densify(h):
                for (dt, r, n, off) in _pieces(80 * h, 80):
                    nc.sync.dma_start(
                        dense3[r:r + n, dt, :],
                        attn[off:off + n, h * S:(h + 1) * S])

            emit_norm(NH - 2)
            for h in range(NH - 1):
                emit_densify(h)
            oa_tiles = []
            for j in range(NK):
                ps = qs_tile()
                for half in range(2):
                    c0 = half * 512
                    for kt in range(4):
                        nc.tensor.matmul(
                            ps[:, c0:c0 + 512],
                            wp3[:, kt, j * 128:(j + 1) * 128],
                            dense3[:, kt, c0:c0 + 512],
                            start=(kt == 0), stop=(kt == 3))
                oa = oa_pool.tile([128, S], f32, tag=f"oa{j}", name=f"oa{j}",
                                  bufs=1)
                if j % 2:
                    nc.scalar.activation(oa[:, :], ps[:, :], AF.Identity)
                else:
                    nc.vector.tensor_copy(oa[:, :], ps[:, :])
                oa_tiles.append(oa)
            emit_norm(NH - 1)
            emit_densify(NH - 1)
            for j in range(NK):
                ps = qs_tile()
                for half in range(2):
                    c0 = half * 512
                    nc.tensor.matmul(ps[:, c0:c0 + 512],
                                     wp3[:, 4, j * 128:(j + 1) * 128],
                                     dense3[:, 4, c0:c0 + 512],
                                     start=True, stop=True)
                ob = ob_pool.tile([128, S], bf16, tag="ob", name="ob")
                nc.vector.tensor_add(ob[:, :], ps[:, :], oa_tiles[j][:, :])
                nc.sync.dma_start(outT[j * 128:(j + 1) * 128, :], ob[:, :])

    nc.compile()
    return nc


def _pack_v3(Wqkv, bqkv, Wproj, bproj, g):
    """Host-side per-head-group weight packing for the v3 program."""
    import concourse.mybir as mybir
    bf16 = mybir.dt.np(mybir.dt.bfloat16)
    NH, NJ, NK, VW = V3_NH, V3_NJ, V3_NK, V3_VW

    Wp = np.zeros((NJ * 128, D), np.float32)
    bp = np.zeros((NJ * 128,), np.float32)
    for m in range(32):
        h = m // 4
        sec = (m % 4) // 2       # 0 = q, 1 = k
        half = m % 2
        src = sec * D + (g * NH + h) * HD + half * BLK
        w = Wqkv[src:src + BLK, :]
        b = bqkv[src:src + BLK]
        if sec == 0:
            w = w * SCALE
            b = b * SCALE
        t, r = _half_pos(m)
        Wp[t * 128 + r:t * 128 + r + BLK] = w
        bp[t * 128 + r:t * 128 + r + BLK] = b
    # lhsT layout [128, j, k, 128]: wqkT[p, j, k, c] = Wp[j*128+c, k*128+p]
    wqkT = np.ascontiguousarray(
        Wp.reshape(NJ, 128, NK, 128).transpose(3, 0, 2, 1)
        .reshape(128, NJ * NK * 128)).astype(bf16)
    bias2d = np.ascontiguousarray(bp.reshape(NJ, 128).T)

    Wv = Wqkv[2 * D + g * 640:2 * D + (g + 1) * 640, :]
    wvT = _tile_rows(np.ascontiguousarray(Wv.T)).astype(bf16)
    bv = bqkv[2 * D + g * 640:2 * D + (g + 1) * 640]

    vt = np.zeros((128, NH * VW), np.float32)
    for h in range(NH):
        vt[:, h * VW:h * VW + HD] = bv[h * HD:(h + 1) * HD][None, :]
        vt[:, h * VW + 96] = 1.0
    vtmpl = vt.astype(bf16)

    Wpd = Wproj[:, g * 640:(g + 1) * 640].T  # [640, 1280] dense attn rows
    wprojT = _tile_rows(np.ascontiguousarray(Wpd)).astype(bf16)
    return wqkT, bias2d, wvT, vtmpl, wprojT


def _cos_sin_v3(cos, sin):
    """Dense [0:80] rope coefficient layouts (bf16), full sequence."""
    import concourse.mybir as mybir
    bf16 = mybir.dt.np(mybir.dt.bfloat16)
    S = cos.shape[0]
    cp = np.zeros((128, S), np.float32)
    sp = np.zeros((128, S), np.float32)
    cp[0:BLK] = cos.T[0:BLK]
    cp[BLK:HD] = cos.T[BLK:HD]
    sp[0:BLK] = -sin.T[0:BLK]
    sp[BLK:HD] = sin.T[BLK:HD]
    return cp.astype(bf16), sp.astype(bf16)


def kernel(hidden_states, cos, sin, Wqkv, bqkv, Wproj, bproj, cu_seqlens):
    sys.path.insert(0, "/opt/trn_rl_repo")
    from concourse import bass_utils

    hidden_states = np.asarray(hidden_states, np.float32)
    cos = np.asarray(cos, np.float32)
    sin = np.asarray(sin, np.float32)
    Wqkv = np.asarray(Wqkv, np.float32)
    bqkv = np.asarray(bqkv, np.float32)
    Wproj = np.asarray(Wproj, np.float32)
    bproj = np.asarray(bproj, np.float32)

    S, D_ = hidden_states.shape
    assert D_ == D
    segs = _segments(cu_seqlens, S)
    uniform = (S % 4 == 0) and segs == [(i * S // 4, (i + 1) * S // 4)
                                        for i in range(4)]

    hiddenT = np.ascontiguousarray(hidden_states.T)
    cosP, sin2P = _pack_cos_sin(cos, sin)

    def _vinit(segs_local):
        n_tt = sum(-(-(e - a) // 128) for a, e in segs_local)
        v = np.zeros((128, n_tt, 17), np.float32)
        v[:, :, 16] = 1.0
        return np.ascontiguousarray(v.reshape(128, n_tt * 17))

    if uniform:
        # v3: 2 head-groups x 4 segments, bf16 on-chip
        import concourse.mybir as mybir
        bf16 = mybir.dt.np(mybir.dt.bfloat16)
        S_core = S // 4
        key = ("V3", S)
        if key not in _CACHE:
            _CACHE[key] = _build_v3()
        nc = _CACHE[key]
        cosPd, sinPd = _cos_sin_v3(cos, sin)
        hidT_b = hiddenT.astype(bf16)
        in_maps = []
        meta = []
        for g in range(2):
            wqkT, b2, wvT, vtmpl, wprojT = _pack_v3(Wqkv, bqkv, Wproj,
                                                    bproj, g)
            for s in range(4):
                sl = slice(s * S_core, (s + 1) * S_core)
                in_maps.append({
                    "hidT": _tile_rows(hidT_b[:, sl]),
                    "wqkT": wqkT,
                    "bias2d": b2,
                    "wvT": wvT,
                    "vtmpl": vtmpl,
                    "cosP": np.ascontiguousarray(cosPd[:, sl]),
                    "sinP": np.ascontiguousarray(sinPd[:, sl]),
                    "wprojT": wprojT,
                })
                meta.append((g, s))
        res = bass_utils.run_bass_kernel_spmd(nc, in_maps,
                                              core_ids=list(range(N_CORES)))
        out = np.zeros((D, S), np.float32)
        for c, (g, s) in enumerate(meta):
            out[:, s * S_core:(s + 1) * S_core] += \
                res.results[c]["outT"].astype(np.float32)
    else:
        # mode C: 8-way head parallel, full sequence per core
        n_h, S_core = H // N_CORES, S
        key = ("C", S, tuple(np.asarray(cu_seqlens).tolist()))
        if key not in _CACHE:
            _CACHE[key] = _build_program(n_h, S_core, segs,
                                         resident_hidden=False)
        nc = _CACHE[key]
        vinit = _vinit(segs)
        hid_tiled = _tile_rows(hiddenT)
        in_maps = []
        for c in range(N_CORES):
            heads = list(range(c * n_h, (c + 1) * n_h))
            wt, b2 = _pack_w(Wqkv, bqkv, heads, n_h)
            in_maps.append({
                "hiddenT": hid_tiled,
                "wqkvT": wt,
                "bias2d": b2,
                "cosP": cosP,
                "sin2P": sin2P,
                "wprojT": _pack_wproj(Wproj, heads),
                "vinit": vinit,
            })
        res = bass_utils.run_bass_kernel_spmd(nc, in_maps,
                                              core_ids=list(range(N_CORES)))
        out = np.zeros((D, S), np.float32)
        for c in range(N_CORES):
            out += res.results[c]["outT"]

    return np.ascontiguousarray(out.T) + bproj[None, :]



# revision 98
# speedup vs baseline: 1.4522x; 1.0772x over previous
"""Trainium2 Bass kernel for Ernie4.5-VL vision attention (ragged segments).

Contract: kernel(**inputs) takes the FULL unsharded inputs (keyed as in
setup_inputs()) and returns the FULL [S, D] float32 output.

Fast path (uniform 4x1024 segments) — the v3 program, see _build_v3:
2 head-groups x 4 segments across 8 cores, bf16 on-chip, ~196us. Key
ideas: dense q/k weight packing (11 j-tiles), v computed untransposed
straight into the PV operand slots, RoPE staged via DMA into dense
[0:80] layouts (K=80 score contraction), 512-wide exp units on
dedicated PSUM slots, ones-column denominator trick, and a projection
over 5 dense re-packed K-tiles split into a kt0-3 prepass + kt4 finish.
Scheduling: emission order defines both Tile dependency direction and
scheduler priority, so readers are never emitted before their writers,
latency-bound chains (normalize) are kept off the compute-critical
FIFOs, and a PE warm-up bridges the load phase (the cost model prices
the PE p-state at dispatch time).

Fallback (any other cu_seqlens): the original fp32r program below —
8-way head parallel, every core sees all segments.

Host does only O(S*D) glue: input transposes/packing bf16 conversion,
summing the per-token partial projections, and the bias adds.
"""

import os
import sys

import numpy as np

H = 16
HD = 80
BLK = 40  # rotate_half half-width
SCALE = HD ** -0.5
N_CORES = 8
D = 1280
NK = D // 128  # contraction tiles for the qkv matmul
ATTN_STRIDE = 96  # head row pitch in the packed attention output
MM_DT_NAME = os.environ.get("KERNEL_MM_DT", "float32r")  # or "float32"
KERNEL_DEBUG = bool(int(os.environ.get("KERNEL_DEBUG", "0")))


def _segments(cu_seqlens, S):
    """Intervals matching reference's searchsorted(cu[1:], i, 'right')."""
    b = np.clip(np.sort(np.asarray(cu_seqlens, dtype=np.int64)[1:5]), 0, S)
    bounds = [0] + list(b) + [S]
    segs = []
    for a, e in zip(bounds[:-1], bounds[1:]):
        if e > a:
            segs.append((int(a), int(e)))
    return segs


def _pack_layout(n_h):
    """Pack per-core qkv dims as 40-row blocks, 3 per 128-row tile (8 pad).

    Each tile holds one v-block at row 0 (PE transpose operands must start
    at a 32-aligned partition) and two q/k blocks at rows 40 and 80.
    Returns pos[(sec, h, half)] = (tile, row) and the number of tiles.
    """
    ntiles = 2 * n_h
    pos = {}
    for h in range(n_h):
        for half in (0, 1):
            pos[("v", h, half)] = (2 * h + half, 0)
    qk = [("q", h, half) for h in range(n_h) for half in (0, 1)]
    qk += [("k", h, half) for h in range(n_h) for half in (0, 1)]
    for j, blk in enumerate(qk):
        pos[blk] = (j // 2, BLK + BLK * (j % 2))
    return pos, ntiles


def _pieces(start, length, tile_rows=128):
    """Split global row range [start, start+length) into per-tile pieces."""
    out = []
    off = 0
    while off < length:
        g = start + off
        t, r = g // tile_rows, g % tile_rows
        n = min(tile_rows - r, length - off)
        out.append((t, r, n, off))
        off += n
    return out


def _proj_k_tiles(n_h):
    rows = ATTN_STRIDE * n_h
    kt = [128] * (rows // 128)
    if rows % 128:
        kt.append(rows % 128)
    return kt


def _build_program(n_h, S_core, segs_local, resident_hidden):
    """Emit the SPMD program. Same structure for every core.

    Engine-AP partition rules on TRN2 (walrus birverifier): compute-engine
    accesses must start at a 32-aligned partition and must not cross a
    64-boundary unless they start on one; cross-partition data movement
    must go through DMA. The layout choices below all follow from this.
    """
    import concourse.mybir as mybir
    import concourse.tile as tile
    from concourse import bacc
    from concourse.masks import make_identity
    from contextlib import ExitStack

    f32 = mybir.dt.float32
    mm_dt = getattr(mybir.dt, MM_DT_NAME)
    AF = mybir.ActivationFunctionType

    k_proj = n_h
    pos, n_mtiles = _pack_layout(n_h)
    dims_pad = n_mtiles * 128
    VW = 97  # v_aug slot width: 80 v dims + 16 zero pad + ones col at 96

    # global key-tile list: (seg_idx, t0, t1)
    t_tiles = []
    for si, (a, e) in enumerate(segs_local):
        t = a
        while t < e:
            t_tiles.append((si, t, min(t + 128, e)))
            t += 128
    n_tt = len(t_tiles)

    nc = bacc.Bacc("TRN2", target_bir_lowering=False, debug=False,
                   enable_asserts=False, num_devices=N_CORES)

    # host supplies hiddenT/wqkvT pre-tiled into 128-partition-major layout
    hiddenT = nc.dram_tensor("hiddenT", [128, NK * S_core], mm_dt,
                             kind="ExternalInput").ap()
    wqkvT = nc.dram_tensor("wqkvT", [128, NK * dims_pad], mm_dt,
                           kind="ExternalInput").ap()
    bias2d = nc.dram_tensor("bias2d", [128, n_mtiles], f32,
                            kind="ExternalInput").ap()
    # cosP/sin2P are host-packed [128, S]: rows 0:40 and 64:104 hold the
    # lo/hi rope coefficients, all other rows zero (zeroes the junk rows
    # of the rotated q/k so the K=104 score matmuls see exact zeros).
    cosP = nc.dram_tensor("cosP", [128, S_core], mm_dt,
                          kind="ExternalInput").ap()
    sin2P = nc.dram_tensor("sin2P", [128, S_core], mm_dt,
                           kind="ExternalInput").ap()
    wprojT = nc.dram_tensor("wprojT", [n_h * HD, D], mm_dt,
                            kind="ExternalInput").ap()
    # per-key-tile v_aug tail init: 16 zero pad cols + ones col (f32r memset
    # fails walrus codegen, so this comes in via DMA)
    vinit = nc.dram_tensor("vinit", [128, n_tt * (VW - HD)], mm_dt,
                           kind="ExternalInput").ap()
    outT = nc.dram_tensor("outT", [D, S_core], f32, kind="ExternalOutput").ap()
    if KERNEL_DEBUG:
        dbg_qkv = nc.dram_tensor("dbg_qkv", [128, n_mtiles * S_core], f32,
                                 kind="ExternalOutput").ap()
        dbg_rot = nc.dram_tensor("dbg_rot", [128, 2 * n_h * S_core], f32,
                                 kind="ExternalOutput").ap()
        dbg_vaug = nc.dram_tensor("dbg_vaug", [128, n_h * n_tt * VW], f32,
                                  kind="ExternalOutput").ap()
        dbg_attn = nc.dram_tensor("dbg_attn", [128, n_h * S_core], f32,
                                  kind="ExternalOutput").ap()

    def r_(ap):
        return ap.bitcast(mm_dt)

    BC = 1024  # psum tile width (2 banks); matmuls stream <=512
    big_chunks = [(c, min(c + BC, S_core)) for c in range(0, S_core, BC)]

    def halves(c0, c1):
        out = []
        q = c0
        while q < c1:
            out.append((q, min(q + 512, c1)))
            q = q + 512
        return out

    with tile.TileContext(nc) as tc, ExitStack() as ctx:
        persist = ctx.enter_context(tc.tile_pool(name="persist", bufs=1))
        ident = persist.tile([128, 128], f32, tag="ident", name="ident")
        make_identity(nc, ident[:])
        bias_sb = persist.tile([128, n_mtiles], f32, tag="bias", name="bias")
        nc.sync.dma_start(bias_sb[:], bias2d[:])

        # PSUM: two 2-bank slots (t0/t1) shared by qkv/scores/proj, two
        # 1-bank slots for v-transposes, one 2-bank slot for PV accumulate
        psum_all_cm = tc.tile_pool(name="psum_all", bufs=1, space="PSUM")
        psum_all = psum_all_cm.__enter__()
        # big pool: qkvT tiles (phases 1-3), slots reused by attn (phases 4-5)
        qkv_pool = ctx.enter_context(tc.tile_pool(name="big", bufs=1))
        qkv_sb = [qkv_pool.tile([128, S_core], mm_dt, tag=f"qkvT{j}",
                                name=f"qkvT{j}") for j in range(n_mtiles)]
        # rope output (rows 0:104 live, 40:64 zeroed via cosP/sin2P pads)
        rot_cm = tc.tile_pool(name="rot", bufs=1)
        rv = rot_cm.__enter__()
        rot_sb = {}
        for h in range(n_h):
            for sec in ("q", "k"):
                rot_sb[(sec, h)] = rv.tile([128, S_core], mm_dt,
                                           tag=f"rot_{sec}{h}",
                                           name=f"rot_{sec}{h}")
        RC = 1024
        rope_cm = tc.tile_pool(name="rope_scr", bufs=2)
        rope_scr = rope_cm.__enter__()

        # ------------ phase 1: qkvT = Wpack @ hidden.T --------------
        with ExitStack() as p1:
            hidden3 = hiddenT.rearrange("p (k s) -> p k s", k=NK)
            w3 = wqkvT.rearrange("p (k m) -> p k m", k=NK)
            if resident_hidden:
                hid_pool = p1.enter_context(tc.tile_pool(name="hid", bufs=1))
                w_pool = p1.enter_context(tc.tile_pool(name="wstream", bufs=3))
                hid_sb = [hid_pool.tile([128, S_core], mm_dt, tag=f"hid{k}",
                                        name=f"hid{k}") for k in range(NK)]
                wj0 = w_pool.tile([128, NK * 128], mm_dt, tag="wj", name="wj")
                nc.sync.dma_start(hid_sb[0][:], hidden3[:, 0, :])
                nc.sync.dma_start(
                    wj0.rearrange("p (k m) -> p k m", k=NK)[:, :, :],
                    w3[:, :, 0:128])
                for k in range(1, NK):
                    nc.sync.dma_start(hid_sb[k][:], hidden3[:, k, :])
                for j in range(n_mtiles):
                    if j == 0:
                        wj = wj0
                    else:
                        wj = w_pool.tile([128, NK * 128], mm_dt, tag="wj",
                                         name="wj")
                        nc.sync.dma_start(
                            wj.rearrange("p (k m) -> p k m", k=NK)[:, :, :],
                            w3[:, :, j * 128:(j + 1) * 128])
                    for (h0, h1) in halves(0, S_core):
                        hw = h1 - h0
                        ps = psum_all.tile([128, 512], f32,
                                           tag=f"t{(h0 // 512) % 2}",
                                           name="qkvp")
                        for k in range(NK):
                            nc.tensor.matmul(
                                ps[:, :hw],
                                r_(wj[:, k * 128:(k + 1) * 128]),
                                r_(hid_sb[k][:, h0:h1]),
                                start=(k == 0), stop=(k == NK - 1))
                        nc.scalar.activation(qkv_sb[j][:, h0:h1], ps[:, :hw],
                                             AF.Identity,
                                             bias=bias_sb[:, j:j + 1])
            else:
                # k-outer streaming: two psum slots hold four j-streams
                # (columns 0:512 and 512:1024), hidden tiles are tiny
                w_pool = p1.enter_context(tc.tile_pool(name="wres", bufs=1))
                w_sb = [w_pool.tile([128, dims_pad], mm_dt, tag=f"w{k}",
                                    name=f"w{k}") for k in range(NK)]
                for k in range(NK):
                    nc.sync.dma_start(w_sb[k][:], w3[:, k, :])
                assert n_mtiles == 4
                hid_pool = p1.enter_context(tc.tile_pool(name="hidstream",
                                                         bufs=3))
                for (h0, h1) in halves(0, S_core):
                    hw = h1 - h0
                    ps01 = psum_all.tile([128, BC], f32, tag="t0", name="ps01")
                    ps23 = psum_all.tile([128, BC], f32, tag="t1", name="ps23")
                    pj_of = lambda j: (ps01 if j < 2 else ps23,
                                       (j % 2) * 512)
                    for k in range(NK):
                        ht = hid_pool.tile([128, 512], mm_dt, tag="hidc",
                                           name="hidc")
                        nc.sync.dma_start(ht[:, :hw], hidden3[:, k, h0:h1])
                        for j in range(n_mtiles):
                            psj, co = pj_of(j)
                            nc.tensor.matmul(
                                psj[:, co:co + hw],
                                r_(w_sb[k][:, j * 128:(j + 1) * 128]),
                                r_(ht[:, :hw]),
                                start=(k == 0), stop=(k == NK - 1))
                    for j in range(n_mtiles):
                        psj, co = pj_of(j)
                        nc.scalar.activation(qkv_sb[j][:, h0:h1],
                                             psj[:, co:co + hw], AF.Identity,
                                             bias=bias_sb[:, j:j + 1])

        psum_all_cm.__exit__(None, None, None)
        ps_att = ctx.enter_context(tc.tile_pool(name="ps_att", bufs=1,
                                                space="PSUM"))

        # ------------ phase 2: RoPE --------------------------------
        # DMA-stage lo/hi into 0:40 / 64:104 (stgA) and swapped (stgB),
        # then rot = stgA*cosP + stgB*sin2P as three same-base wide ops.
        # double-buffered persistent staging tensors; rows 40:64 zeroed once
        # from cosP's zero rows so the [0:104) products read defined zeros
        stg = {}
        for nm in ("sa0", "sa1", "sb0", "sb1"):
            stg[nm] = rope_scr.tile([128, RC], mm_dt, tag=nm, name=nm, bufs=1)
        pair_i = 0
        for ci, f0 in enumerate(range(0, S_core, RC)):
            f1 = min(f0 + RC, S_core)
            fs = f1 - f0
            cos_sb = rope_scr.tile([128, RC], mm_dt, tag="cos", name="cos",
                                   bufs=1)
            sin_sb = rope_scr.tile([128, RC], mm_dt, tag="sin", name="sin",
                                   bufs=1)
            nc.scalar.dma_start(cos_sb[:, :fs], cosP[:, f0:f1])
            nc.scalar.dma_start(sin_sb[:, :fs], sin2P[:, f0:f1])
            if ci == 0:
                for nm in stg:
                    nc.scalar.dma_start(stg[nm][BLK:64, :], cos_sb[BLK:64, :])
            for h in range(n_h):
                for sec in ("q", "k"):
                    lo_t, lo_r = pos[(sec, h, 0)]
                    hi_t, hi_r = pos[(sec, h, 1)]
                    assert hi_t == lo_t and hi_r == lo_r + BLK
                    x = qkv_sb[lo_t]
                    dst = rot_sb[(sec, h)]
                    stga = stg[f"sa{pair_i % 2}"]
                    stgb = stg[f"sb{pair_i % 2}"]
                    nc.scalar.dma_start(stga[0:BLK, :fs],
                                        x[lo_r:lo_r + BLK, f0:f1])
                    nc.scalar.dma_start(stga[64:64 + BLK, :fs],
                                        x[hi_r:hi_r + BLK, f0:f1])
                    nc.scalar.dma_start(stgb[0:BLK, :fs],
                                        x[hi_r:hi_r + BLK, f0:f1])
                    nc.scalar.dma_start(stgb[64:64 + BLK, :fs],
                                        x[lo_r:lo_r + BLK, f0:f1])
                    nc.vector.tensor_mul(dst[0:104, f0:f1], stga[0:104, :fs],
                                         cos_sb[0:104, :fs])
                    eng = nc.gpsimd if pair_i % 2 == 0 else nc.vector
                    eng.tensor_mul(stgb[0:104, :fs], stgb[0:104, :fs],
                                   sin_sb[0:104, :fs])
                    nc.vector.tensor_add(dst[0:104, f0:f1], dst[0:104, f0:f1],
                                         stgb[0:104, :fs])
                    pair_i += 1
        rope_cm.__exit__(None, None, None)

        # v_aug tiles + per-head emitter (invoked right after each head's
        # rope so attention unblocks head by head)
        vaug_cm = tc.tile_pool(name="vaug", bufs=1)
        vaug_pool = vaug_cm.__enter__()
        vaug_sb = [vaug_pool.tile([128, n_tt * VW], mm_dt, tag=f"vaug{h}",
                                  name=f"vaug{h}") for h in range(n_h)]
        vinit3 = vinit.rearrange("p (t c) -> p t c", c=VW - HD)
        for h in range(n_h):
            nc.sync.dma_start(
                vaug_sb[h].rearrange("p (t c) -> p t c", c=VW)[:, :, HD:VW],
                vinit3[:, :, :])
        GRP = 4  # key tiles transposed per psum tile / copy (1 psum bank)

        def emit_vaug(h):
            gi = 0
            while gi < n_tt:
                hi_g = min(gi + GRP, n_tt)
                if all(t_tiles[g][2] - t_tiles[g][1] == 128
                       for g in range(gi, hi_g)):
                    grp = list(range(gi, hi_g))
                else:
                    grp = [gi]
                ng = len(grp)
                tp = ps_att.tile([128, GRP * HD], f32, tag="tp", name="tp")
                for x, g in enumerate(grp):
                    si, t0, t1 = t_tiles[g]
                    sz = t1 - t0
                    for half in (0, 1):
                        vt, vr = pos[("v", h, half)]
                        nc.tensor.transpose(
                            tp[:sz, x * HD + half * BLK:
                               x * HD + (half + 1) * BLK],
                            qkv_sb[vt][0:BLK, t0:t1].bitcast(f32),
                            ident[:BLK, :BLK])
                sz0 = t_tiles[grp[0]][2] - t_tiles[grp[0]][1]
                dst = vaug_sb[h].rearrange("p (t c) -> p t c", c=VW)
                src_ap = tp.rearrange("p (t c) -> p t c", c=HD)
                if h % 2 == 0:
                    nc.vector.tensor_copy(dst[:sz0, grp[0]:grp[0] + ng, 0:HD],
                                          src_ap[:sz0, 0:ng, :])
                else:
                    nc.scalar.activation(dst[:sz0, grp[0]:grp[0] + ng, 0:HD],
                                         src_ap[:sz0, 0:ng, :], AF.Identity)
                gi += ng




        if KERNEL_DEBUG:
            for j in range(n_mtiles):
                nc.sync.dma_start(
                    dbg_qkv[:, j * S_core:(j + 1) * S_core],
                    qkv_sb[j][:].bitcast(f32))
            i_ = 0
            for h in range(n_h):
                for sec in ("q", "k"):
                    nc.sync.dma_start(
                        dbg_rot[:, i_ * S_core:(i_ + 1) * S_core],
                        rot_sb[(sec, h)][:].bitcast(f32))
                    i_ += 1

        # ------------ phase 4: attention ----------------------------
        # one attn tile per head (rows 0:80) so every compute access is
        # partition-0 based; tiles reuse the dead qkvT slots
        attn_sb = [qkv_pool.tile([128, S_core], mm_dt, tag=f"qkvT{h}",
                                 name=f"attnT{h}") for h in range(n_h)]

        seg_ttiles = {}
        for ti, (si, t0, t1) in enumerate(t_tiles):
            seg_ttiles.setdefault(si, []).append((ti, t0, t1))

        BA = 512  # attention query-chunk width (1-bank psum slots)
        with ExitStack() as p4:
            pt_pool = p4.enter_context(tc.tile_pool(name="pt", bufs=3))
            nrm_pool = p4.enter_context(tc.tile_pool(name="nrm", bufs=2))
            unit_box = [0]

            def emit_attention(h, si, a, e):
                qT = rot_sb[("q", h)]
                kT = rot_sb[("k", h)]
                q = a
                while q < e:
                    q0, q1 = q, min(q + BA, e)
                    qs = q1 - q0
                    po = ps_att.tile([128, BA], f32,
                                     tag=f"po{unit_box[0] % 2}", name="pv")
                    tts = seg_ttiles[si]
                    for idx, (ti, t0, t1) in enumerate(tts):
                        sz = t1 - t0
                        ps = ps_att.tile([128, BA], f32, tag=f"st{idx % 2}",
                                         name="st")
                        nc.tensor.matmul(ps[:sz, :qs], r_(kT[0:104, t0:t1]),
                                         r_(qT[0:104, q0:q1]),
                                         start=True, stop=True)
                        pt = pt_pool.tile([128, BA], mm_dt, tag="pt", name="pt")
                        nc.scalar.activation(pt[:sz, :qs], ps[:sz, :qs], AF.Exp)
                        nc.tensor.matmul(
                            po[:VW, :qs],
                            r_(vaug_sb[h][:sz, ti * VW:(ti + 1) * VW]),
                            r_(pt[:sz, :qs]),
                            start=(idx == 0), stop=(idx == len(tts) - 1))
                    # partition_broadcast ucode reads physical partition 0,
                    # so shift the denominator row 96 -> 0 via DMA
                    rc = nrm_pool.tile([128, BA], f32, tag="rc", name="rc")
                    nc.vector.tensor_copy(rc[96:97, :qs], po[96:97, :qs])
                    nc.sync.dma_start(rc[0:1, :qs], rc[96:97, :qs])
                    nc.vector.reciprocal(rc[0:1, :qs], rc[0:1, :qs])
                    bc = nrm_pool.tile([128, BA], mm_dt, tag="bc", name="bc")
                    nc.gpsimd.partition_broadcast(
                        bc[0:HD, :qs], rc[0:1, :qs].bitcast(mm_dt))
                    nc.vector.tensor_mul(attn_sb[h][0:HD, q0:q1],
                                         po[0:HD, :qs], bc[0:HD, :qs])
                    unit_box[0] += 1
                    q = q1

            if len(segs_local) == 1:
                a, e = segs_local[0]
                for h in range(n_h):
                    emit_vaug(h)
                    emit_attention(h, 0, a, e)
            else:
                for h in range(n_h):
                    emit_vaug(h)
                for si, (a, e) in enumerate(segs_local):
                    for h in range(n_h):
                        emit_attention(h, si, a, e)

        vaug_cm.__exit__(None, None, None)
        rot_cm.__exit__(None, None, None)

        # ------------ phase 5: projection partial -------------------
        with ExitStack() as p5:
            wp_pool = p5.enter_context(tc.tile_pool(name="wp", bufs=1))
            wp_sb = []
            for kt in range(k_proj):
                t = wp_pool.tile([HD, D], mm_dt, tag=f"wp{kt}", name=f"wp{kt}")
                nc.sync.dma_start(t[:], wprojT[kt * HD:(kt + 1) * HD, :])
                wp_sb.append(t)
            out_pool = p5.enter_context(tc.tile_pool(name="outsb", bufs=3))
            for (c0, c1) in big_chunks:
                cs = c1 - c0
                for j in range(D // 128):
                    ob = out_pool.tile([128, BC], f32, tag="ob", name="ob")
                    for (h0, h1) in halves(c0, c1):
                        ps = ps_att.tile([128, 512], f32, tag=f"st{j % 2}",
                                         name="pj")
                        for kt in range(k_proj):
                            nc.tensor.matmul(
                                ps[:, :h1 - h0],
                                r_(wp_sb[kt][:, j * 128:(j + 1) * 128]),
                                r_(attn_sb[kt][0:HD, h0:h1]),
                                start=(kt == 0), stop=(kt == k_proj - 1))
                        if j % 2 == 0:
                            nc.vector.tensor_copy(ob[:, h0 - c0:h1 - c0],
                                                  ps[:, :h1 - h0])
                        else:
                            nc.scalar.activation(ob[:, h0 - c0:h1 - c0],
                                                 ps[:, :h1 - h0], AF.Identity)
                    nc.sync.dma_start(outT[j * 128:(j + 1) * 128, c0:c1],
                                      ob[:, :cs])

    nc.compile()
    return nc


def _pack_w(Wqkv, bqkv, heads, n_h):
    """Per-core packed qkv weights (q rows pre-scaled).

    Returns wqkvT_tiled [128, NK*dims_pad] (k-major blocks of [128, dims_pad])
    and bias2d [128, n_mtiles]."""
    pos, n_mtiles = _pack_layout(n_h)
    dims_pad = n_mtiles * 128
    W = np.zeros((dims_pad, D), np.float32)
    b = np.zeros((dims_pad,), np.float32)
    sec_off = {"q": 0, "k": D, "v": 2 * D}
    for i, h in enumerate(heads):
        for sec in ("q", "k", "v"):
            for half in (0, 1):
                t, r = pos[(sec, i, half)]
                src = sec_off[sec] + h * HD + half * BLK
                w = Wqkv[src:src + BLK, :]
                bb = bqkv[src:src + BLK]
                if sec == "q":
                    w = w * SCALE
                    bb = bb * SCALE
                W[t * 128 + r:t * 128 + r + BLK] = w
                b[t * 128 + r:t * 128 + r + BLK] = bb
    w_tiled = _tile_rows(np.ascontiguousarray(W.T))
    bias2d = np.ascontiguousarray(b.reshape(n_mtiles, 128).T)
    return w_tiled, bias2d


def _tile_rows(x):
    """[R, C] with R = nk*128 -> [128, nk*C] k-major tiling."""
    R, C = x.shape
    nk = R // 128
    return np.ascontiguousarray(
        x.reshape(nk, 128, C).transpose(1, 0, 2).reshape(128, nk * C))


def _pack_wproj(Wproj, heads):
    """Rows of Wproj.T for this core's head dims, stacked per head."""
    W = np.zeros((len(heads) * HD, Wproj.shape[0]), np.float32)
    for i, h in enumerate(heads):
        W[i * HD:(i + 1) * HD] = Wproj[:, h * HD:(h + 1) * HD].T
    return W


def _pack_cos_sin(cos, sin):
    """cosP/sin2P [128, S]: lo coeffs at rows 0:40, hi at 64:104, rest 0.

    sin2P row signs match rot = x*cosP + swap(x)*sin2P: lo rows hold
    -sin_lo (they multiply x_hi), hi rows hold +sin_hi (they multiply x_lo).
    """
    S = cos.shape[0]
    cosP = np.zeros((128, S), np.float32)
    sinP = np.zeros((128, S), np.float32)
    cosP[0:BLK] = cos.T[0:BLK]
    cosP[64:64 + BLK] = cos.T[BLK:HD]
    sinP[0:BLK] = -sin.T[0:BLK]
    sinP[64:64 + BLK] = sin.T[BLK:HD]
    return cosP, sinP


_CACHE = {}

# ---------------------------------------------------------------------------
# v3 fast path (uniform 4x1024 segments): 2 head-groups x 4 segments SPMD.
#
# Per core: 8 heads, 1024 tokens, one segment. All on-chip data bf16 except
# PSUM (f32) and the normalization scalars (f32).
#   - q/k packed dense: 32 40-row halves, 3 per 128-row tile (11 j-tiles)
#   - v computed untransposed ([tokens, vdim]) straight into PV operand slots
#   - RoPE: DMA-stage [lo;hi]/[hi;lo] into dense [0:80] layouts, 3 DVE ops
#     at bf16 2x rate; scores contract K=80 (no zero padding rows)
#   - scores^T per (head, key-tile) into [128,1024] PSUM, one wide exp
#   - PV with ones column at slot col 96 -> denominators at PSUM row 96
#   - projection over 5 dense 128-row K-tiles (attn heads re-packed via DMA)
# ---------------------------------------------------------------------------

V3_S = 1024     # tokens per core
V3_NH = 8       # heads per core
V3_NJ = 11      # dense qk j-tiles (3 x 40-row halves each, 8 junk rows)
V3_NK = 10      # contraction tiles (D / 128)
V3_NT = 8       # token/key tiles (S / 128)
V3_VW = 97      # v slot: 80 v dims + 16 zero pad + ones col at 96
V3_NPJ = 5      # dense proj k-tiles (8 heads * 80 / 128)


def _half_pos(m):
    """Packed position of 40-row half m: (j_tile, row in {0, 40, 80})."""
    return m // 3, 40 * (m % 3)


def _build_v3():
    import concourse.mybir as mybir
    import concourse.tile as tile
    from concourse import bacc
    from contextlib import ExitStack

    f32 = mybir.dt.float32
    bf16 = mybir.dt.bfloat16
    AF = mybir.ActivationFunctionType
    S, NH, NJ, NK, NT, VW = V3_S, V3_NH, V3_NJ, V3_NK, V3_NT, V3_VW

    nc = bacc.Bacc("TRN2", target_bir_lowering=False, debug=False,
                   enable_asserts=False, num_devices=N_CORES)

    hidT = nc.dram_tensor("hidT", [128, NK * S], bf16,
                          kind="ExternalInput").ap()
    wqkT = nc.dram_tensor("wqkT", [128, NJ * NK * 128], bf16,
                          kind="ExternalInput").ap()
    bias2d = nc.dram_tensor("bias2d", [128, NJ], f32,
                            kind="ExternalInput").ap()
    wvT = nc.dram_tensor("wvT", [128, NK * 640], bf16,
                         kind="ExternalInput").ap()
    vtmpl = nc.dram_tensor("vtmpl", [128, NH * VW], bf16,
                           kind="ExternalInput").ap()
    cosP = nc.dram_tensor("cosP", [128, S], bf16, kind="ExternalInput").ap()
    sinP = nc.dram_tensor("sinP", [128, S], bf16, kind="ExternalInput").ap()
    wprojT = nc.dram_tensor("wprojT", [128, V3_NPJ * D], bf16,
                            kind="ExternalInput").ap()
    outT = nc.dram_tensor("outT", [D, S], bf16, kind="ExternalOutput").ap()
    if KERNEL_DEBUG:
        dbg_qk = nc.dram_tensor("dbg_qk", [128, NJ * S], bf16,
                                kind="ExternalOutput").ap()
        dbg_rot = nc.dram_tensor("dbg_rot", [128, 16 * S], bf16,
                                 kind="ExternalOutput").ap()
        dbg_vaug = nc.dram_tensor("dbg_vaug", [128, NT * NH * VW], bf16,
                                  kind="ExternalOutput").ap()
        dbg_attn = nc.dram_tensor("dbg_attn", [128, NH * S], bf16,
                                  kind="ExternalOutput").ap()

    hid3 = hidT.rearrange("p (k s) -> p k s", k=NK)
    wqk4 = wqkT.rearrange("p (j k c) -> p j k c", j=NJ, k=NK)
    wv3 = wvT.rearrange("p (k c) -> p k c", k=NK)

    with tile.TileContext(nc) as tc, ExitStack() as ctx:
        persist = ctx.enter_context(tc.tile_pool(name="persist", bufs=1))
        bias_sb = persist.tile([128, NJ], f32, tag="bias", name="bias")
        cos_sb = persist.tile([128, S], bf16, tag="cos", name="cos")
        sin_sb = persist.tile([128, S], bf16, tag="sin", name="sin")
        vt_sb = persist.tile([128, NH * VW], bf16, tag="vt", name="vt")
        pass

        psum = ctx.enter_context(tc.tile_pool(name="psum", bufs=1,
                                              space="PSUM"))
        unit = [0]

        def qs_tile():
            t = psum.tile([128, 1024], f32, tag=f"qs{unit[0] % 2}", name="qs")
            unit[0] += 1
            return t

        qk_pool = ctx.enter_context(tc.tile_pool(name="qk", bufs=1))
        qk_sb = qk_pool.tile([128, NJ * S], bf16, tag="qk", name="qk")
        rot_pool = ctx.enter_context(tc.tile_pool(name="rotp", bufs=1))
        rot = rot_pool.tile([128, 16 * S], bf16, tag="rot", name="rot")
        stgb_pool = ctx.enter_context(tc.tile_pool(name="stgb", bufs=6))
        vaug_pool = ctx.enter_context(tc.tile_pool(name="vaug", bufs=1))
        vaug = vaug_pool.tile([128, NT * NH * VW], bf16, tag="va", name="va")
        vaug3 = vaug.rearrange("p (t h c) -> p t h c", t=NT, h=NH)
        attn_pool = ctx.enter_context(tc.tile_pool(name="attn", bufs=1))
        # unnormalized PV output incl. bf16 denominator row at partition 96;
        # normalized in place at the end
        attn = attn_pool.tile([128, NH * S], bf16, tag="at", name="at")
        pt_pool = ctx.enter_context(tc.tile_pool(name="pt", bufs=12))
        rc_pool = ctx.enter_context(tc.tile_pool(name="rc", bufs=2))
        rcf_pool = ctx.enter_context(tc.tile_pool(name="rcf", bufs=2))
        bc_pool = ctx.enter_context(tc.tile_pool(name="bc", bufs=2))
        wp_pool = ctx.enter_context(tc.tile_pool(name="wp", bufs=1))
        wp_sb = wp_pool.tile([128, V3_NPJ * D], bf16, tag="wp", name="wp")

        # weights/hidden (dead after phase 1; proj pools reuse the space) —
        # entered last among open pools so the mid-program release is LIFO
        ph1 = tc.tile_pool(name="ph1", bufs=1)
        p1 = ph1.__enter__()
        hid_sb = p1.tile([128, NK * S], bf16, tag="hid", name="hid")
        wqk_sb = p1.tile([128, NJ * NK * 128], bf16, tag="wqk", name="wqk")
        wv_sb = p1.tile([128, NK * 640], bf16, tag="wv", name="wv")
        hid3s = hid_sb.rearrange("p (k s) -> p k s", k=NK)
        wqk4s = wqk_sb.rearrange("p (j k c) -> p j k c", j=NJ, k=NK)
        wv3s = wv_sb.rearrange("p (k c) -> p k c", k=NK)
        # PE warm-up: the cost model prices p-state at dispatch; keep the
        # PE busy on junk matmuls while the input loads stream so the real
        # matmuls dispatch against a warm (2.4 GHz) clock
        wa = p1.tile([128, 16], bf16, tag="wa", name="wa")
        wb = p1.tile([128, 256], bf16, tag="wb", name="wb")
        nc.vector.memset(wa[:, :], 0.0)
        nc.vector.memset(wb[:, :], 0.0)
        wps = psum.tile([128, 1024], f32, tag="po", name="warm")
        for _ in range(30):
            nc.tensor.matmul(wps[0:16, 0:256], wa[:, :], wb[:, :],
                             start=True, stop=True)
        nc.vector.tensor_copy(wa[0:16, 0:4], wps[0:16, 0:4])

        # loads on one queue in priority order; the tail weight loads are
        # deferred into the round loop (just-in-time) so their transfers
        # never sit ahead of the rope staging DMAs on the serial DMA device
        nc.sync.dma_start(vt_sb[:], vtmpl[:])
        nc.sync.dma_start(bias_sb[:], bias2d[:])
        nc.sync.dma_start(wqk4s[:, 0:3, :, :], wqk4[:, 0:3, :, :])
        nc.sync.dma_start(hid3s[:, 0:5, :], hid3[:, 0:5, :])
        nc.sync.dma_start(hid3s[:, 5:NK, :], hid3[:, 5:NK, :])
        nc.sync.dma_start(cos_sb[:], cosP[:])
        nc.sync.dma_start(sin_sb[:], sinP[:])
        # JIT loads must be EMITTED before their first consumer (emission
        # order defines dependency direction), while issuing late enough
        # that their transfers don't delay the rope staging DMAs
        load_at = {
            1: lambda: nc.sync.dma_start(wv3s[:, :, :], wv3[:, :, :]),
            2: lambda: nc.sync.dma_start(wqk4s[:, 3:6, :, :],
                                         wqk4[:, 3:6, :, :]),
            3: lambda: nc.sync.dma_start(wqk4s[:, 6:9, :, :],
                                         wqk4[:, 6:9, :, :]),
            4: lambda: nc.sync.dma_start(wqk4s[:, 9:NJ, :, :],
                                         wqk4[:, 9:NJ, :, :]),
            5: lambda: nc.sync.dma_start(wp_sb[:], wprojT[:]),
        }

        vt3 = vt_sb.rearrange("p (h c) -> p h c", h=NH)

        def emit_qk(j):
            ps = qs_tile()
            for half in range(2):
                c0 = half * 512
                for k in range(NK):
                    nc.tensor.matmul(ps[:, c0:c0 + 512],
                                     wqk4s[:, j, k, :],
                                     hid3s[:, k, c0:c0 + 512],
                                     start=(k == 0), stop=(k == NK - 1))
            if j < 5:
                # early copies on ACT (idle pre-exp): their rope staging
                # DMAs directly follow on the same queue, so the critical
                # startup rope chain never waits in a clogged FIFO
                nc.scalar.activation(qk_sb[:, j * S:(j + 1) * S], ps[:, :],
                                     AF.Identity, bias=bias_sb[:, j:j + 1])
            else:
                nc.vector.tensor_scalar_add(qk_sb[:, j * S:(j + 1) * S],
                                            ps[:, :], bias_sb[:, j:j + 1])

        def emit_v(tt):
            ps = qs_tile()
            for (c0, w) in ((0, 512), (512, 128)):
                for k in range(NK):
                    nc.tensor.matmul(ps[:, c0:c0 + w],
                                     hid3s[:, k, tt * 128:(tt + 1) * 128],
                                     wv3s[:, k, c0:c0 + w],
                                     start=(k == 0), stop=(k == NK - 1))
            src = ps[:, 0:NH * HD].rearrange("p (h c) -> p h c", c=HD)
            nc.vector.tensor_add(vaug3[:, tt, :, 0:HD], src[:, :, :],
                                 vt3[:, :, 0:HD])
            # pad + ones columns (disjoint from the v region written above)
            nc.vector.tensor_copy(vaug3[:, tt, :, HD:VW], vt3[:, :, HD:VW])

        qk3 = qk_sb.rearrange("p (j s) -> p j s", j=NJ)
        stgb_tiles = {}

        def emit_rope_dma(p):
            lo_t, lo_r = _half_pos(2 * p)
            hi_t, hi_r = _half_pos(2 * p + 1)
            b0 = p * S
            if p <= 6:
                eng = nc.scalar
            elif p >= 13:
                eng = nc.gpsimd
            else:
                eng = [nc.gpsimd, nc.sync][p % 2]
            if lo_t == hi_t and hi_r == lo_r + 40:
                eng.dma_start(rot[0:80, b0:b0 + S],
                              qk3[lo_r:lo_r + 80, lo_t, :])
            else:
                eng.dma_start(rot[0:40, b0:b0 + S],
                              qk3[lo_r:lo_r + 40, lo_t, :])
                eng.dma_start(rot[40:80, b0:b0 + S],
                              qk3[hi_r:hi_r + 40, hi_t, :])
            sb = stgb_pool.tile([128, S], bf16, tag="sb", name="sb")
            stgb_tiles[p] = sb
            eng.dma_start(sb[0:40, :], qk3[hi_r:hi_r + 40, hi_t, :])
            eng.dma_start(sb[40:80, :], qk3[lo_r:lo_r + 40, lo_t, :])

        def emit_rope_mul(p):
            b0 = p * S
            sb = stgb_tiles.pop(p)
            nc.vector.tensor_mul(rot[0:80, b0:b0 + S], rot[0:80, b0:b0 + S],
                                 cos_sb[0:80, :])
            meng = nc.gpsimd if p % 4 == 3 else nc.vector
            meng.tensor_mul(sb[0:80, :], sb[0:80, :], sin_sb[0:80, :])
            nc.vector.tensor_add(rot[0:80, b0:b0 + S], rot[0:80, b0:b0 + S],
                                 sb[0:80, :])

        # Attention pump. 512-wide score half-units (unit = (kt, half),
        # 16 per head) on dedicated 1-bank PSUM slots decouple the exp
        # stream from the qk/v slot rotation. Invariant: every emitted
        # instruction's dependencies (incl. slot predecessors) are emitted
        # before it — PVs of head h follow head h-1's evacuation (single po
        # slot), exps run ahead of PVs by at most PT_AHEAD pt tiles.
        PT_AHEAD = 10
        heads_q = []      # started heads, in order
        v_done = [0]
        st_unit = [0]
        outstanding = [0]

        def start_attn(h):
            heads_q.append({"h": h, "se": 0, "pv": 0, "po": None})

        def emit_st_exp(hs):
            h, u = hs["h"], hs["se"]
            kt, half = u // 2, u % 2
            pq, pk = 2 * h, 2 * h + 1
            c0 = half * 512
            st = psum.tile([128, 512], f32, tag=f"st{st_unit[0] % 2}",
                           name="st")
            st_unit[0] += 1
            nc.tensor.matmul(
                st[:, :],
                rot[0:80, pk * S + kt * 128:pk * S + (kt + 1) * 128],
                rot[0:80, pq * S + c0:pq * S + c0 + 512],
                start=True, stop=True)
            pt = pt_pool.tile([128, 512], bf16, tag="pt", name="pt")
            nc.scalar.activation(pt[:, :], st[:, :], AF.Exp)
            hs.setdefault("pts", []).append(pt)
            hs["se"] += 1
            outstanding[0] += 1

        def emit_pv(hs):
            h, u = hs["h"], hs["pv"]
            kt, half = u // 2, u % 2
            if hs["po"] is None:
                hs["po"] = psum.tile([128, 1024], f32, tag="po", name="po")
            c0 = half * 512
            nc.tensor.matmul(hs["po"][0:VW, c0:c0 + 512],
                             vaug3[:, kt, h, :],
                             hs["pts"][u][:, :],
                             start=(kt == 0), stop=(kt == NT - 1))
            hs["pv"] += 1
            outstanding[0] -= 1

        def pump_attn():
            progress = True
            while progress:
                progress = False
                if heads_q:
                    hs = heads_q[0]
                    while (hs["pv"] < hs["se"]
                           and hs["pv"] // 2 < v_done[0]):
                        emit_pv(hs)
                        progress = True
                    if hs["pv"] == 2 * NT:
                        h = hs["h"]
                        # evacuate on ACT so the po slot frees immediately
                        # and normalize never touches compute FIFOs
                        nc.scalar.activation(attn[0:VW, h * S:(h + 1) * S],
                                             hs["po"][0:VW, :], AF.Identity)
                        if h <= 5:
                            emit_norm(h)
                        heads_q.pop(0)
                        progress = True
                        continue
                for hs in heads_q:
                    while (hs["se"] < 2 * NT
                           and outstanding[0] < PT_AHEAD):
                        emit_st_exp(hs)
                        progress = True

        def emit_norm(h):
            rc = rc_pool.tile([1, S], bf16, tag="rc", name="rc")
            nc.scalar.dma_start(rc[0:1, :], attn[96:97, h * S:(h + 1) * S])
            rcf = rcf_pool.tile([1, S], f32, tag="rcf", name="rcf")
            nc.vector.reciprocal(rcf[0:1, :], rc[0:1, :])
            bc = bc_pool.tile([80, S], f32, tag="bc", name="bc")
            nc.gpsimd.partition_broadcast(bc[0:80, :], rcf[0:1, :])
            nc.vector.tensor_mul(attn[0:80, h * S:(h + 1) * S],
                                 attn[0:80, h * S:(h + 1) * S], bc[0:80, :])

        # emission driver: qk j-tiles paced with v, rope, attention
        v_at = {1: (0, 1), 2: (2, 3), 3: (4, 5), 4: (6, 7)}
        rope_at = {}
        for p in range(16):
            jmax = max(_half_pos(2 * p)[0], _half_pos(2 * p + 1)[0])
            rope_at.setdefault(jmax, []).append(p)
        pending = []
        for j in range(NJ):
            if j in load_at:
                load_at[j]()
            emit_qk(j)
            # rope muls one round behind their staging DMAs so the DVE FIFO
            # never stalls on in-flight DMA latency
            for p in pending:
                emit_rope_mul(p)
                if p % 2 == 1:
                    start_attn(p // 2)
            pending = []
            for tt in v_at.get(j, ()):
                emit_v(tt)
                v_done[0] += 1
            pump_attn()
            for p in rope_at.get(j, ()):
                emit_rope_dma(p)
                pending.append(p)
        for p in pending:
            emit_rope_mul(p)
            if p % 2 == 1:
                start_attn(p // 2)
        pump_attn()

        if KERNEL_DEBUG:
            nc.sync.dma_start(dbg_qk[:, :], qk_sb[:, :])
            nc.sync.dma_start(dbg_rot[0:80, :], rot[0:80, :])
            nc.sync.dma_start(dbg_vaug[:, :], vaug[:, :])
            nc.sync.dma_start(dbg_attn[0:97, :], attn[0:97, :])

        ph1.__exit__(None, None, None)

        # normalize + dense re-pack of attn heads + split projection:
        # kt0-3 prepass overlaps the last heads' attention, kt4 finishes
        with ExitStack() as p5:
            late = p5.enter_context(tc.tile_pool(name="late", bufs=1))
            dense = late.tile([128, V3_NPJ * S], bf16, tag="dn", name="dn")
            ob_pool = p5.enter_context(tc.tile_pool(name="ob", bufs=3))
            oa_pool = p5.enter_context(tc.tile_pool(name="oa", bufs=1))
            dense3 = dense.rearrange("p (t s) -> p t s", t=V3_NPJ)
            wp3 = wp_sb.rearrange("p (t c) -> p t c", t=V3_NPJ)

            def emit_densify(h):
                for (dt, r, n, off) in _pieces(80 * h, 80):
                    nc.sync.dma_start(
                        dense3[r:r + n, dt, :],
                        attn[off:off + n, h * S:(h + 1) * S])

            emit_norm(NH - 2)
            for h in range(NH - 1):
                emit_densify(h)
            KA = 3   # prepass contracts kt0..KA-1 (early heads), final the rest
            oa_tiles = []
            for j in range(NK):
                ps = qs_tile()
                for half in range(2):
                    c0 = half * 512
                    for kt in range(KA):
                        nc.tensor.matmul(
                            ps[:, c0:c0 + 512],
                            wp3[:, kt, j * 128:(j + 1) * 128],
                            dense3[:, kt, c0:c0 + 512],
                            start=(kt == 0), stop=(kt == KA - 1))
                oa = oa_pool.tile([128, S], f32, tag=f"oa{j}", name=f"oa{j}",
                                  bufs=1)
                if j % 2:
                    nc.scalar.activation(oa[:, :], ps[:, :], AF.Identity)
                else:
                    nc.vector.tensor_copy(oa[:, :], ps[:, :])
                oa_tiles.append(oa)
            emit_norm(NH - 1)
            emit_densify(NH - 1)
            for j in range(NK):
                ps = qs_tile()
                for half in range(2):
                    c0 = half * 512
                    for kt in range(KA, V3_NPJ):
                        nc.tensor.matmul(ps[:, c0:c0 + 512],
                                         wp3[:, kt, j * 128:(j + 1) * 128],
                                         dense3[:, kt, c0:c0 + 512],
                                         start=(kt == KA),
                                         stop=(kt == V3_NPJ - 1))
                ob = ob_pool.tile([128, S], bf16, tag="ob", name="ob")
                nc.vector.tensor_add(ob[:, :], ps[:, :], oa_tiles[j][:, :])
                nc.sync.dma_start(outT[j * 128:(j + 1) * 128, :], ob[:, :])

    nc.compile()
    return nc


def _pack_v3(Wqkv, bqkv, Wproj, bproj, g):
    """Host-side per-head-group weight packing for the v3 program."""
    import concourse.mybir as mybir
    bf16 = mybir.dt.np(mybir.dt.bfloat16)
    NH, NJ, NK, VW = V3_NH, V3_NJ, V3_NK, V3_VW

    Wp = np.zeros((NJ * 128, D), np.float32)
    bp = np.zeros((NJ * 128,), np.float32)
    for m in range(32):
        h = m // 4
        sec = (m % 4) // 2       # 0 = q, 1 = k
        half = m % 2
        src = sec * D + (g * NH + h) * HD + half * BLK
        w = Wqkv[src:src + BLK, :]
        b = bqkv[src:src + BLK]
        if sec == 0:
            w = w * SCALE
            b = b * SCALE
        t, r = _half_pos(m)
        Wp[t * 128 + r:t * 128 + r + BLK] = w
        bp[t * 128 + r:t * 128 + r + BLK] = b
    # lhsT layout [128, j, k, 128]: wqkT[p, j, k, c] = Wp[j*128+c, k*128+p]
    wqkT = np.ascontiguousarray(
        Wp.reshape(NJ, 128, NK, 128).transpose(3, 0, 2, 1)
        .reshape(128, NJ * NK * 128)).astype(bf16)
    bias2d = np.ascontiguousarray(bp.reshape(NJ, 128).T)

    Wv = Wqkv[2 * D + g * 640:2 * D + (g + 1) * 640, :]
    wvT = _tile_rows(np.ascontiguousarray(Wv.T)).astype(bf16)
    bv = bqkv[2 * D + g * 640:2 * D + (g + 1) * 640]

    vt = np.zeros((128, NH * VW), np.float32)
    for h in range(NH):
        vt[:, h * VW:h * VW + HD] = bv[h * HD:(h + 1) * HD][None, :]
        vt[:, h * VW + 96] = 1.0
    vtmpl = vt.astype(bf16)

    Wpd = Wproj[:, g * 640:(g + 1) * 640].T  # [640, 1280] dense attn rows
    wprojT = _tile_rows(np.ascontiguousarray(Wpd)).astype(bf16)
    return wqkT, bias2d, wvT, vtmpl, wprojT


def _cos_sin_v3(cos, sin):
    """Dense [0:80] rope coefficient layouts (bf16), full sequence."""
    import concourse.mybir as mybir
    bf16 = mybir.dt.np(mybir.dt.bfloat16)
    S = cos.shape[0]
    cp = np.zeros((128, S), np.float32)
    sp = np.zeros((128, S), np.float32)
    cp[0:BLK] = cos.T[0:BLK]
    cp[BLK:HD] = cos.T[BLK:HD]
    sp[0:BLK] = -sin.T[0:BLK]
    sp[BLK:HD] = sin.T[BLK:HD]
    return cp.astype(bf16), sp.astype(bf16)


def kernel(hidden_states, cos, sin, Wqkv, bqkv, Wproj, bproj, cu_seqlens):
    sys.path.insert(0, "/opt/trn_rl_repo")
    from concourse import bass_utils

    hidden_states = np.asarray(hidden_states, np.float32)
    cos = np.asarray(cos, np.float32)
    sin = np.asarray(sin, np.float32)
    Wqkv = np.asarray(Wqkv, np.float32)
    bqkv = np.asarray(bqkv, np.float32)
    Wproj = np.asarray(Wproj, np.float32)
    bproj = np.asarray(bproj, np.float32)

    S, D_ = hidden_states.shape
    assert D_ == D
    segs = _segments(cu_seqlens, S)
    uniform = (S % 4 == 0) and segs == [(i * S // 4, (i + 1) * S // 4)
                                        for i in range(4)]

    hiddenT = np.ascontiguousarray(hidden_states.T)
    cosP, sin2P = _pack_cos_sin(cos, sin)

    def _vinit(segs_local):
        n_tt = sum(-(-(e - a) // 128) for a, e in segs_local)
        v = np.zeros((128, n_tt, 17), np.float32)
        v[:, :, 16] = 1.0
        return np.ascontiguousarray(v.reshape(128, n_tt * 17))

    if uniform:
        # v3: 2 head-groups x 4 segments, bf16 on-chip
        import concourse.mybir as mybir
        bf16 = mybir.dt.np(mybir.dt.bfloat16)
        S_core = S // 4
        key = ("V3", S)
        if key not in _CACHE:
            _CACHE[key] = _build_v3()
        nc = _CACHE[key]
        cosPd, sinPd = _cos_sin_v3(cos, sin)
        hidT_b = hiddenT.astype(bf16)
        in_maps = []
        meta = []
        for g in range(2):
            wqkT, b2, wvT, vtmpl, wprojT = _pack_v3(Wqkv, bqkv, Wproj,
                                                    bproj, g)
            for s in range(4):
                sl = slice(s * S_core, (s + 1) * S_core)
                in_maps.append({
                    "hidT": _tile_rows(hidT_b[:, sl]),
                    "wqkT": wqkT,
                    "bias2d": b2,
                    "wvT": wvT,
                    "vtmpl": vtmpl,
                    "cosP": np.ascontiguousarray(cosPd[:, sl]),
                    "sinP": np.ascontiguousarray(sinPd[:, sl]),
                    "wprojT": wprojT,
                })
                meta.append((g, s))
        res = bass_utils.run_bass_kernel_spmd(nc, in_maps,
                                              core_ids=list(range(N_CORES)))
        out = np.zeros((D, S), np.float32)
        for c, (g, s) in enumerate(meta):
            out[:, s * S_core:(s + 1) * S_core] += \
                res.results[c]["outT"].astype(np.float32)
    else:
        # mode C: 8-way head parallel, full sequence per core
        n_h, S_core = H // N_CORES, S
        key = ("C", S, tuple(np.asarray(cu_seqlens).tolist()))
        if key not in _CACHE:
            _CACHE[key] = _build_program(n_h, S_core, segs,
                                         resident_hidden=False)
        nc = _CACHE[key]
        vinit = _vinit(segs)
        hid_tiled = _tile_rows(hiddenT)
        in_maps = []
        for c in range(N_CORES):
            heads = list(range(c * n_h, (c + 1) * n_h))
            wt, b2 = _pack_w(Wqkv, bqkv, heads, n_h)
            in_maps.append({
                "hiddenT": hid_tiled,
                "wqkvT": wt,
                "bias2d": b2,
                "cosP": cosP,
                "sin2P": sin2P,
                "wprojT": _pack_wproj(Wproj, heads),
                "vinit": vinit,
            })
        res = bass_utils.run_bass_kernel_spmd(nc, in_maps,
                                              core_ids=list(range(N_CORES)))
        out = np.zeros((D, S), np.float32)
        for c in range(N_CORES):
            out += res.results[c]["outT"]

    return np.ascontiguousarray(out.T) + bproj[None, :]



# revision 108
# speedup vs baseline: 1.4525x; 1.0002x over previous
"""Trainium2 Bass kernel for Ernie4.5-VL vision attention (ragged segments).

Contract: kernel(**inputs) takes the FULL unsharded inputs (keyed as in
setup_inputs()) and returns the FULL [S, D] float32 output.

Fast path (uniform 4x1024 segments) — the v3 program, see _build_v3:
2 head-groups x 4 segments across 8 cores, bf16 on-chip, ~179us. Key
ideas: dense q/k weight packing (11 j-tiles), v computed untransposed
straight into the PV operand slots, RoPE staged via DMA into dense
[0:80] layouts (K=80 score contraction), 512-wide exp units on
dedicated PSUM slots, ones-column denominator trick, and a projection
over 5 dense re-packed K-tiles split into a kt0-2 prepass (overlaps
the last heads' attention) + kt3-4 finish.
Scheduling: emission order defines both Tile dependency direction and
scheduler priority, so readers are never emitted before their writers,
latency-bound chains (normalize) are kept off the compute-critical
FIFOs, and a PE warm-up bridges the load phase (the cost model prices
the PE p-state at dispatch time).

Fallback (any other cu_seqlens): the original fp32r program below —
8-way head parallel, every core sees all segments.

Host does only O(S*D) glue: input transposes/packing bf16 conversion,
summing the per-token partial projections, and the bias adds.
"""

import os
import sys

import numpy as np

H = 16
HD = 80
BLK = 40  # rotate_half half-width
SCALE = HD ** -0.5
N_CORES = 8
D = 1280
NK = D // 128  # contraction tiles for the qkv matmul
ATTN_STRIDE = 96  # head row pitch in the packed attention output
MM_DT_NAME = os.environ.get("KERNEL_MM_DT", "float32r")  # or "float32"
KERNEL_DEBUG = bool(int(os.environ.get("KERNEL_DEBUG", "0")))


def _segments(cu_seqlens, S):
    """Intervals matching reference's searchsorted(cu[1:], i, 'right')."""
    b = np.clip(np.sort(np.asarray(cu_seqlens, dtype=np.int64)[1:5]), 0, S)
    bounds = [0] + list(b) + [S]
    segs = []
    for a, e in zip(bounds[:-1], bounds[1:]):
        if e > a:
            segs.append((int(a), int(e)))
    return segs


def _pack_layout(n_h):
    """Pack per-core qkv dims as 40-row blocks, 3 per 128-row tile (8 pad).

    Each tile holds one v-block at row 0 (PE transpose operands must start
    at a 32-aligned partition) and two q/k blocks at rows 40 and 80.
    Returns pos[(sec, h, half)] = (tile, row) and the number of tiles.
    """
    ntiles = 2 * n_h
    pos = {}
    for h in range(n_h):
        for half in (0, 1):
            pos[("v", h, half)] = (2 * h + half, 0)
    qk = [("q", h, half) for h in range(n_h) for half in (0, 1)]
    qk += [("k", h, half) for h in range(n_h) for half in (0, 1)]
    for j, blk in enumerate(qk):
        pos[blk] = (j // 2, BLK + BLK * (j % 2))
    return pos, ntiles


def _pieces(start, length, tile_rows=128):
    """Split global row range [start, start+length) into per-tile pieces."""
    out = []
    off = 0
    while off < length:
        g = start + off
        t, r = g // tile_rows, g % tile_rows
        n = min(tile_rows - r, length - off)
        out.append((t, r, n, off))
        off += n
    return out


def _proj_k_tiles(n_h):
    rows = ATTN_STRIDE * n_h
    kt = [128] * (rows // 128)
    if rows % 128:
        kt.append(rows % 128)
    return kt


def _build_program(n_h, S_core, segs_local, resident_hidden):
    """Emit the SPMD program. Same structure for every core.

    Engine-AP partition rules on TRN2 (walrus birverifier): compute-engine
    accesses must start at a 32-aligned partition and must not cross a
    64-boundary unless they start on one; cross-partition data movement
    must go through DMA. The layout choices below all follow from this.
    """
    import concourse.mybir as mybir
    import concourse.tile as tile
    from concourse import bacc
    from concourse.masks import make_identity
    from contextlib import ExitStack

    f32 = mybir.dt.float32
    mm_dt = getattr(mybir.dt, MM_DT_NAME)
    AF = mybir.ActivationFunctionType

    k_proj = n_h
    pos, n_mtiles = _pack_layout(n_h)
    dims_pad = n_mtiles * 128
    VW = 97  # v_aug slot width: 80 v dims + 16 zero pad + ones col at 96

    # global key-tile list: (seg_idx, t0, t1)
    t_tiles = []
    for si, (a, e) in enumerate(segs_local):
        t = a
        while t < e:
            t_tiles.append((si, t, min(t + 128, e)))
            t += 128
    n_tt = len(t_tiles)

    nc = bacc.Bacc("TRN2", target_bir_lowering=False, debug=False,
                   enable_asserts=False, num_devices=N_CORES)

    # host supplies hiddenT/wqkvT pre-tiled into 128-partition-major layout
    hiddenT = nc.dram_tensor("hiddenT", [128, NK * S_core], mm_dt,
                             kind="ExternalInput").ap()
    wqkvT = nc.dram_tensor("wqkvT", [128, NK * dims_pad], mm_dt,
                           kind="ExternalInput").ap()
    bias2d = nc.dram_tensor("bias2d", [128, n_mtiles], f32,
                            kind="ExternalInput").ap()
    # cosP/sin2P are host-packed [128, S]: rows 0:40 and 64:104 hold the
    # lo/hi rope coefficients, all other rows zero (zeroes the junk rows
    # of the rotated q/k so the K=104 score matmuls see exact zeros).
    cosP = nc.dram_tensor("cosP", [128, S_core], mm_dt,
                          kind="ExternalInput").ap()
    sin2P = nc.dram_tensor("sin2P", [128, S_core], mm_dt,
                           kind="ExternalInput").ap()
    wprojT = nc.dram_tensor("wprojT", [n_h * HD, D], mm_dt,
                            kind="ExternalInput").ap()
    # per-key-tile v_aug tail init: 16 zero pad cols + ones col (f32r memset
    # fails walrus codegen, so this comes in via DMA)
    vinit = nc.dram_tensor("vinit", [128, n_tt * (VW - HD)], mm_dt,
                           kind="ExternalInput").ap()
    outT = nc.dram_tensor("outT", [D, S_core], f32, kind="ExternalOutput").ap()
    if KERNEL_DEBUG:
        dbg_qkv = nc.dram_tensor("dbg_qkv", [128, n_mtiles * S_core], f32,
                                 kind="ExternalOutput").ap()
        dbg_rot = nc.dram_tensor("dbg_rot", [128, 2 * n_h * S_core], f32,
                                 kind="ExternalOutput").ap()
        dbg_vaug = nc.dram_tensor("dbg_vaug", [128, n_h * n_tt * VW], f32,
                                  kind="ExternalOutput").ap()
        dbg_attn = nc.dram_tensor("dbg_attn", [128, n_h * S_core], f32,
                                  kind="ExternalOutput").ap()

    def r_(ap):
        return ap.bitcast(mm_dt)

    BC = 1024  # psum tile width (2 banks); matmuls stream <=512
    big_chunks = [(c, min(c + BC, S_core)) for c in range(0, S_core, BC)]

    def halves(c0, c1):
        out = []
        q = c0
        while q < c1:
            out.append((q, min(q + 512, c1)))
            q = q + 512
        return out

    with tile.TileContext(nc) as tc, ExitStack() as ctx:
        persist = ctx.enter_context(tc.tile_pool(name="persist", bufs=1))
        ident = persist.tile([128, 128], f32, tag="ident", name="ident")
        make_identity(nc, ident[:])
        bias_sb = persist.tile([128, n_mtiles], f32, tag="bias", name="bias")
        nc.sync.dma_start(bias_sb[:], bias2d[:])

        # PSUM: two 2-bank slots (t0/t1) shared by qkv/scores/proj, two
        # 1-bank slots for v-transposes, one 2-bank slot for PV accumulate
        psum_all_cm = tc.tile_pool(name="psum_all", bufs=1, space="PSUM")
        psum_all = psum_all_cm.__enter__()
        # big pool: qkvT tiles (phases 1-3), slots reused by attn (phases 4-5)
        qkv_pool = ctx.enter_context(tc.tile_pool(name="big", bufs=1))
        qkv_sb = [qkv_pool.tile([128, S_core], mm_dt, tag=f"qkvT{j}",
                                name=f"qkvT{j}") for j in range(n_mtiles)]
        # rope output (rows 0:104 live, 40:64 zeroed via cosP/sin2P pads)
        rot_cm = tc.tile_pool(name="rot", bufs=1)
        rv = rot_cm.__enter__()
        rot_sb = {}
        for h in range(n_h):
            for sec in ("q", "k"):
                rot_sb[(sec, h)] = rv.tile([128, S_core], mm_dt,
                                           tag=f"rot_{sec}{h}",
                                           name=f"rot_{sec}{h}")
        RC = 1024
        rope_cm = tc.tile_pool(name="rope_scr", bufs=2)
        rope_scr = rope_cm.__enter__()

        # ------------ phase 1: qkvT = Wpack @ hidden.T --------------
        with ExitStack() as p1:
            hidden3 = hiddenT.rearrange("p (k s) -> p k s", k=NK)
            w3 = wqkvT.rearrange("p (k m) -> p k m", k=NK)
            if resident_hidden:
                hid_pool = p1.enter_context(tc.tile_pool(name="hid", bufs=1))
                w_pool = p1.enter_context(tc.tile_pool(name="wstream", bufs=3))
                hid_sb = [hid_pool.tile([128, S_core], mm_dt, tag=f"hid{k}",
                                        name=f"hid{k}") for k in range(NK)]
                wj0 = w_pool.tile([128, NK * 128], mm_dt, tag="wj", name="wj")
                nc.sync.dma_start(hid_sb[0][:], hidden3[:, 0, :])
                nc.sync.dma_start(
                    wj0.rearrange("p (k m) -> p k m", k=NK)[:, :, :],
                    w3[:, :, 0:128])
                for k in range(1, NK):
                    nc.sync.dma_start(hid_sb[k][:], hidden3[:, k, :])
                for j in range(n_mtiles):
                    if j == 0:
                        wj = wj0
                    else:
                        wj = w_pool.tile([128, NK * 128], mm_dt, tag="wj",
                                         name="wj")
                        nc.sync.dma_start(
                            wj.rearrange("p (k m) -> p k m", k=NK)[:, :, :],
                            w3[:, :, j * 128:(j + 1) * 128])
                    for (h0, h1) in halves(0, S_core):
                        hw = h1 - h0
                        ps = psum_all.tile([128, 512], f32,
                                           tag=f"t{(h0 // 512) % 2}",
                                           name="qkvp")
                        for k in range(NK):
                            nc.tensor.matmul(
                                ps[:, :hw],
                                r_(wj[:, k * 128:(k + 1) * 128]),
                                r_(hid_sb[k][:, h0:h1]),
                                start=(k == 0), stop=(k == NK - 1))
                        nc.scalar.activation(qkv_sb[j][:, h0:h1], ps[:, :hw],
                                             AF.Identity,
                                             bias=bias_sb[:, j:j + 1])
            else:
                # k-outer streaming: two psum slots hold four j-streams
                # (columns 0:512 and 512:1024), hidden tiles are tiny
                w_pool = p1.enter_context(tc.tile_pool(name="wres", bufs=1))
                w_sb = [w_pool.tile([128, dims_pad], mm_dt, tag=f"w{k}",
                                    name=f"w{k}") for k in range(NK)]
                for k in range(NK):
                    nc.sync.dma_start(w_sb[k][:], w3[:, k, :])
                assert n_mtiles == 4
                hid_pool = p1.enter_context(tc.tile_pool(name="hidstream",
                                                         bufs=3))
                for (h0, h1) in halves(0, S_core):
                    hw = h1 - h0
                    ps01 = psum_all.tile([128, BC], f32, tag="t0", name="ps01")
                    ps23 = psum_all.tile([128, BC], f32, tag="t1", name="ps23")
                    pj_of = lambda j: (ps01 if j < 2 else ps23,
                                       (j % 2) * 512)
                    for k in range(NK):
                        ht = hid_pool.tile([128, 512], mm_dt, tag="hidc",
                                           name="hidc")
                        nc.sync.dma_start(ht[:, :hw], hidden3[:, k, h0:h1])
                        for j in range(n_mtiles):
                            psj, co = pj_of(j)
                            nc.tensor.matmul(
                                psj[:, co:co + hw],
                                r_(w_sb[k][:, j * 128:(j + 1) * 128]),
                                r_(ht[:, :hw]),
                                start=(k == 0), stop=(k == NK - 1))
                    for j in range(n_mtiles):
                        psj, co = pj_of(j)
                        nc.scalar.activation(qkv_sb[j][:, h0:h1],
                                             psj[:, co:co + hw], AF.Identity,
                                             bias=bias_sb[:, j:j + 1])

        psum_all_cm.__exit__(None, None, None)
        ps_att = ctx.enter_context(tc.tile_pool(name="ps_att", bufs=1,
                                                space="PSUM"))

        # ------------ phase 2: RoPE --------------------------------
        # DMA-stage lo/hi into 0:40 / 64:104 (stgA) and swapped (stgB),
        # then rot = stgA*cosP + stgB*sin2P as three same-base wide ops.
        # double-buffered persistent staging tensors; rows 40:64 zeroed once
        # from cosP's zero rows so the [0:104) products read defined zeros
        stg = {}
        for nm in ("sa0", "sa1", "sb0", "sb1"):
            stg[nm] = rope_scr.tile([128, RC], mm_dt, tag=nm, name=nm, bufs=1)
        pair_i = 0
        for ci, f0 in enumerate(range(0, S_core, RC)):
            f1 = min(f0 + RC, S_core)
            fs = f1 - f0
            cos_sb = rope_scr.tile([128, RC], mm_dt, tag="cos", name="cos",
                                   bufs=1)
            sin_sb = rope_scr.tile([128, RC], mm_dt, tag="sin", name="sin",
                                   bufs=1)
            nc.scalar.dma_start(cos_sb[:, :fs], cosP[:, f0:f1])
            nc.scalar.dma_start(sin_sb[:, :fs], sin2P[:, f0:f1])
            if ci == 0:
                for nm in stg:
                    nc.scalar.dma_start(stg[nm][BLK:64, :], cos_sb[BLK:64, :])
            for h in range(n_h):
                for sec in ("q", "k"):
                    lo_t, lo_r = pos[(sec, h, 0)]
                    hi_t, hi_r = pos[(sec, h, 1)]
                    assert hi_t == lo_t and hi_r == lo_r + BLK
                    x = qkv_sb[lo_t]
                    dst = rot_sb[(sec, h)]
                    stga = stg[f"sa{pair_i % 2}"]
                    stgb = stg[f"sb{pair_i % 2}"]
                    nc.scalar.dma_start(stga[0:BLK, :fs],
                                        x[lo_r:lo_r + BLK, f0:f1])
                    nc.scalar.dma_start(stga[64:64 + BLK, :fs],
                                        x[hi_r:hi_r + BLK, f0:f1])
                    nc.scalar.dma_start(stgb[0:BLK, :fs],
                                        x[hi_r:hi_r + BLK, f0:f1])
                    nc.scalar.dma_start(stgb[64:64 + BLK, :fs],
                                        x[lo_r:lo_r + BLK, f0:f1])
                    nc.vector.tensor_mul(dst[0:104, f0:f1], stga[0:104, :fs],
                                         cos_sb[0:104, :fs])
                    eng = nc.gpsimd if pair_i % 2 == 0 else nc.vector
                    eng.tensor_mul(stgb[0:104, :fs], stgb[0:104, :fs],
                                   sin_sb[0:104, :fs])
                    nc.vector.tensor_add(dst[0:104, f0:f1], dst[0:104, f0:f1],
                                         stgb[0:104, :fs])
                    pair_i += 1
        rope_cm.__exit__(None, None, None)

        # v_aug tiles + per-head emitter (invoked right after each head's
        # rope so attention unblocks head by head)
        vaug_cm = tc.tile_pool(name="vaug", bufs=1)
        vaug_pool = vaug_cm.__enter__()
        vaug_sb = [vaug_pool.tile([128, n_tt * VW], mm_dt, tag=f"vaug{h}",
                                  name=f"vaug{h}") for h in range(n_h)]
        vinit3 = vinit.rearrange("p (t c) -> p t c", c=VW - HD)
        for h in range(n_h):
            nc.sync.dma_start(
                vaug_sb[h].rearrange("p (t c) -> p t c", c=VW)[:, :, HD:VW],
                vinit3[:, :, :])
        GRP = 4  # key tiles transposed per psum tile / copy (1 psum bank)

        def emit_vaug(h):
            gi = 0
            while gi < n_tt:
                hi_g = min(gi + GRP, n_tt)
                if all(t_tiles[g][2] - t_tiles[g][1] == 128
                       for g in range(gi, hi_g)):
                    grp = list(range(gi, hi_g))
                else:
                    grp = [gi]
                ng = len(grp)
                tp = ps_att.tile([128, GRP * HD], f32, tag="tp", name="tp")
                for x, g in enumerate(grp):
                    si, t0, t1 = t_tiles[g]
                    sz = t1 - t0
                    for half in (0, 1):
                        vt, vr = pos[("v", h, half)]
                        nc.tensor.transpose(
                            tp[:sz, x * HD + half * BLK:
                               x * HD + (half + 1) * BLK],
                            qkv_sb[vt][0:BLK, t0:t1].bitcast(f32),
                            ident[:BLK, :BLK])
                sz0 = t_tiles[grp[0]][2] - t_tiles[grp[0]][1]
                dst = vaug_sb[h].rearrange("p (t c) -> p t c", c=VW)
                src_ap = tp.rearrange("p (t c) -> p t c", c=HD)
                if h % 2 == 0:
                    nc.vector.tensor_copy(dst[:sz0, grp[0]:grp[0] + ng, 0:HD],
                                          src_ap[:sz0, 0:ng, :])
                else:
                    nc.scalar.activation(dst[:sz0, grp[0]:grp[0] + ng, 0:HD],
                                         src_ap[:sz0, 0:ng, :], AF.Identity)
                gi += ng




        if KERNEL_DEBUG:
            for j in range(n_mtiles):
                nc.sync.dma_start(
                    dbg_qkv[:, j * S_core:(j + 1) * S_core],
                    qkv_sb[j][:].bitcast(f32))
            i_ = 0
            for h in range(n_h):
                for sec in ("q", "k"):
                    nc.sync.dma_start(
                        dbg_rot[:, i_ * S_core:(i_ + 1) * S_core],
                        rot_sb[(sec, h)][:].bitcast(f32))
                    i_ += 1

        # ------------ phase 4: attention ----------------------------
        # one attn tile per head (rows 0:80) so every compute access is
        # partition-0 based; tiles reuse the dead qkvT slots
        attn_sb = [qkv_pool.tile([128, S_core], mm_dt, tag=f"qkvT{h}",
                                 name=f"attnT{h}") for h in range(n_h)]

        seg_ttiles = {}
        for ti, (si, t0, t1) in enumerate(t_tiles):
            seg_ttiles.setdefault(si, []).append((ti, t0, t1))

        BA = 512  # attention query-chunk width (1-bank psum slots)
        with ExitStack() as p4:
            pt_pool = p4.enter_context(tc.tile_pool(name="pt", bufs=3))
            nrm_pool = p4.enter_context(tc.tile_pool(name="nrm", bufs=2))
            unit_box = [0]

            def emit_attention(h, si, a, e):
                qT = rot_sb[("q", h)]
                kT = rot_sb[("k", h)]
                q = a
                while q < e:
                    q0, q1 = q, min(q + BA, e)
                    qs = q1 - q0
                    po = ps_att.tile([128, BA], f32,
                                     tag=f"po{unit_box[0] % 2}", name="pv")
                    tts = seg_ttiles[si]
                    for idx, (ti, t0, t1) in enumerate(tts):
                        sz = t1 - t0
                        ps = ps_att.tile([128, BA], f32, tag=f"st{idx % 2}",
                                         name="st")
                        nc.tensor.matmul(ps[:sz, :qs], r_(kT[0:104, t0:t1]),
                                         r_(qT[0:104, q0:q1]),
                                         start=True, stop=True)
                        pt = pt_pool.tile([128, BA], mm_dt, tag="pt", name="pt")
                        nc.scalar.activation(pt[:sz, :qs], ps[:sz, :qs], AF.Exp)
                        nc.tensor.matmul(
                            po[:VW, :qs],
                            r_(vaug_sb[h][:sz, ti * VW:(ti + 1) * VW]),
                            r_(pt[:sz, :qs]),
                            start=(idx == 0), stop=(idx == len(tts) - 1))
                    # partition_broadcast ucode reads physical partition 0,
                    # so shift the denominator row 96 -> 0 via DMA
                    rc = nrm_pool.tile([128, BA], f32, tag="rc", name="rc")
                    nc.vector.tensor_copy(rc[96:97, :qs], po[96:97, :qs])
                    nc.sync.dma_start(rc[0:1, :qs], rc[96:97, :qs])
                    nc.vector.reciprocal(rc[0:1, :qs], rc[0:1, :qs])
                    bc = nrm_pool.tile([128, BA], mm_dt, tag="bc", name="bc")
                    nc.gpsimd.partition_broadcast(
                        bc[0:HD, :qs], rc[0:1, :qs].bitcast(mm_dt))
                    nc.vector.tensor_mul(attn_sb[h][0:HD, q0:q1],
                                         po[0:HD, :qs], bc[0:HD, :qs])
                    unit_box[0] += 1
                    q = q1

            if len(segs_local) == 1:
                a, e = segs_local[0]
                for h in range(n_h):
                    emit_vaug(h)
                    emit_attention(h, 0, a, e)
            else:
                for h in range(n_h):
                    emit_vaug(h)
                for si, (a, e) in enumerate(segs_local):
                    for h in range(n_h):
                        emit_attention(h, si, a, e)

        vaug_cm.__exit__(None, None, None)
        rot_cm.__exit__(None, None, None)

        # ------------ phase 5: projection partial -------------------
        with ExitStack() as p5:
            wp_pool = p5.enter_context(tc.tile_pool(name="wp", bufs=1))
            wp_sb = []
            for kt in range(k_proj):
                t = wp_pool.tile([HD, D], mm_dt, tag=f"wp{kt}", name=f"wp{kt}")
                nc.sync.dma_start(t[:], wprojT[kt * HD:(kt + 1) * HD, :])
                wp_sb.append(t)
            out_pool = p5.enter_context(tc.tile_pool(name="outsb", bufs=3))
            for (c0, c1) in big_chunks:
                cs = c1 - c0
                for j in range(D // 128):
                    ob = out_pool.tile([128, BC], f32, tag="ob", name="ob")
                    for (h0, h1) in halves(c0, c1):
                        ps = ps_att.tile([128, 512], f32, tag=f"st{j % 2}",
                                         name="pj")
                        for kt in range(k_proj):
                            nc.tensor.matmul(
                                ps[:, :h1 - h0],
                                r_(wp_sb[kt][:, j * 128:(j + 1) * 128]),
                                r_(attn_sb[kt][0:HD, h0:h1]),
                                start=(kt == 0), stop=(kt == k_proj - 1))
                        if j % 2 == 0:
                            nc.vector.tensor_copy(ob[:, h0 - c0:h1 - c0],
                                                  ps[:, :h1 - h0])
                        else:
                            nc.scalar.activation(ob[:, h0 - c0:h1 - c0],
                                                 ps[:, :h1 - h0], AF.Identity)
                    nc.sync.dma_start(outT[j * 128:(j + 1) * 128, c0:c1],
                                      ob[:, :cs])

    nc.compile()
    return nc


def _pack_w(Wqkv, bqkv, heads, n_h):
    """Per-core packed qkv weights (q rows pre-scaled).

    Returns wqkvT_tiled [128, NK*dims_pad] (k-major blocks of [128, dims_pad])
    and bias2d [128, n_mtiles]."""
    pos, n_mtiles = _pack_layout(n_h)
    dims_pad = n_mtiles * 128
    W = np.zeros((dims_pad, D), np.float32)
    b = np.zeros((dims_pad,), np.float32)
    sec_off = {"q": 0, "k": D, "v": 2 * D}
    for i, h in enumerate(heads):
        for sec in ("q", "k", "v"):
            for half in (0, 1):
                t, r = pos[(sec, i, half)]
                src = sec_off[sec] + h * HD + half * BLK
                w = Wqkv[src:src + BLK, :]
                bb = bqkv[src:src + BLK]
                if sec == "q":
                    w = w * SCALE
                    bb = bb * SCALE
                W[t * 128 + r:t * 128 + r + BLK] = w
                b[t * 128 + r:t * 128 + r + BLK] = bb
    w_tiled = _tile_rows(np.ascontiguousarray(W.T))
    bias2d = np.ascontiguousarray(b.reshape(n_mtiles, 128).T)
    return w_tiled, bias2d


def _tile_rows(x):
    """[R, C] with R = nk*128 -> [128, nk*C] k-major tiling."""
    R, C = x.shape
    nk = R // 128
    return np.ascontiguousarray(
        x.reshape(nk, 128, C).transpose(1, 0, 2).reshape(128, nk * C))


def _pack_wproj(Wproj, heads):
    """Rows of Wproj.T for this core's head dims, stacked per head."""
    W = np.zeros((len(heads) * HD, Wproj.shape[0]), np.float32)
    for i, h in enumerate(heads):
        W[i * HD:(i + 1) * HD] = Wproj[:, h * HD:(h + 1) * HD].T
    return W


def _pack_cos_sin(cos, sin):
    """cosP/sin2P [128, S]: lo coeffs at rows 0:40, hi at 64:104, rest 0.

    sin2P row signs match rot = x*cosP + swap(x)*sin2P: lo rows hold
    -sin_lo (they multiply x_hi), hi rows hold +sin_hi (they multiply x_lo).
    """
    S = cos.shape[0]
    cosP = np.zeros((128, S), np.float32)
    sinP = np.zeros((128, S), np.float32)
    cosP[0:BLK] = cos.T[0:BLK]
    cosP[64:64 + BLK] = cos.T[BLK:HD]
    sinP[0:BLK] = -sin.T[0:BLK]
    sinP[64:64 + BLK] = sin.T[BLK:HD]
    return cosP, sinP


_CACHE = {}

# ---------------------------------------------------------------------------
# v3 fast path (uniform 4x1024 segments): 2 head-groups x 4 segments SPMD.
#
# Per core: 8 heads, 1024 tokens, one segment. All on-chip data bf16 except
# PSUM (f32) and the normalization scalars (f32).
#   - q/k packed dense: 32 40-row halves, 3 per 128-row tile (11 j-tiles)
#   - v computed untransposed ([tokens, vdim]) straight into PV operand slots
#   - RoPE: DMA-stage [lo;hi]/[hi;lo] into dense [0:80] layouts, 3 DVE ops
#     at bf16 2x rate; scores contract K=80 (no zero padding rows)
#   - scores^T per (head, key-tile) into [128,1024] PSUM, one wide exp
#   - PV with ones column at slot col 96 -> denominators at PSUM row 96
#   - projection over 5 dense 128-row K-tiles (attn heads re-packed via DMA)
# ---------------------------------------------------------------------------

V3_S = 1024     # tokens per core
V3_NH = 8       # heads per core
V3_NJ = 11      # dense qk j-tiles (3 x 40-row halves each, 8 junk rows)
V3_NK = 10      # contraction tiles (D / 128)
V3_NT = 8       # token/key tiles (S / 128)
V3_VW = 97      # v slot: 80 v dims + 16 zero pad + ones col at 96
V3_NPJ = 5      # dense proj k-tiles (8 heads * 80 / 128)


def _half_pos(m):
    """Packed position of 40-row half m: (j_tile, row in {0, 40, 80})."""
    return m // 3, 40 * (m % 3)


def _build_v3():
    import concourse.mybir as mybir
    import concourse.tile as tile
    from concourse import bacc
    from contextlib import ExitStack

    f32 = mybir.dt.float32
    bf16 = mybir.dt.bfloat16
    AF = mybir.ActivationFunctionType
    S, NH, NJ, NK, NT, VW = V3_S, V3_NH, V3_NJ, V3_NK, V3_NT, V3_VW

    nc = bacc.Bacc("TRN2", target_bir_lowering=False, debug=False,
                   enable_asserts=False, num_devices=N_CORES)

    hidT = nc.dram_tensor("hidT", [128, NK * S], bf16,
                          kind="ExternalInput").ap()
    wqkT = nc.dram_tensor("wqkT", [128, NJ * NK * 128], bf16,
                          kind="ExternalInput").ap()
    bias2d = nc.dram_tensor("bias2d", [128, NJ], f32,
                            kind="ExternalInput").ap()
    wvT = nc.dram_tensor("wvT", [128, NK * 640], bf16,
                         kind="ExternalInput").ap()
    vtmpl = nc.dram_tensor("vtmpl", [128, NH * VW], bf16,
                           kind="ExternalInput").ap()
    cosP = nc.dram_tensor("cosP", [128, S], bf16, kind="ExternalInput").ap()
    sinP = nc.dram_tensor("sinP", [128, S], bf16, kind="ExternalInput").ap()
    wprojT = nc.dram_tensor("wprojT", [128, V3_NPJ * D], bf16,
                            kind="ExternalInput").ap()
    outT = nc.dram_tensor("outT", [D, S], bf16, kind="ExternalOutput").ap()
    if KERNEL_DEBUG:
        dbg_qk = nc.dram_tensor("dbg_qk", [128, NJ * S], bf16,
                                kind="ExternalOutput").ap()
        dbg_rot = nc.dram_tensor("dbg_rot", [128, 16 * S], bf16,
                                 kind="ExternalOutput").ap()
        dbg_vaug = nc.dram_tensor("dbg_vaug", [128, NT * NH * VW], bf16,
                                  kind="ExternalOutput").ap()
        dbg_attn = nc.dram_tensor("dbg_attn", [128, NH * S], bf16,
                                  kind="ExternalOutput").ap()

    hid3 = hidT.rearrange("p (k s) -> p k s", k=NK)
    wqk4 = wqkT.rearrange("p (j k c) -> p j k c", j=NJ, k=NK)
    wv3 = wvT.rearrange("p (k c) -> p k c", k=NK)

    with tile.TileContext(nc) as tc, ExitStack() as ctx:
        persist = ctx.enter_context(tc.tile_pool(name="persist", bufs=1))
        bias_sb = persist.tile([128, NJ], f32, tag="bias", name="bias")
        cos_sb = persist.tile([128, S], bf16, tag="cos", name="cos")
        sin_sb = persist.tile([128, S], bf16, tag="sin", name="sin")
        vt_sb = persist.tile([128, NH * VW], bf16, tag="vt", name="vt")
        pass

        psum = ctx.enter_context(tc.tile_pool(name="psum", bufs=1,
                                              space="PSUM"))
        unit = [0]

        def qs_tile():
            t = psum.tile([128, 1024], f32, tag=f"qs{unit[0] % 2}", name="qs")
            unit[0] += 1
            return t

        qk_pool = ctx.enter_context(tc.tile_pool(name="qk", bufs=1))
        qk_sb = qk_pool.tile([128, NJ * S], bf16, tag="qk", name="qk")
        rot_pool = ctx.enter_context(tc.tile_pool(name="rotp", bufs=1))
        rot = rot_pool.tile([128, 16 * S], bf16, tag="rot", name="rot")
        stgb_pool = ctx.enter_context(tc.tile_pool(name="stgb", bufs=6))
        vaug_pool = ctx.enter_context(tc.tile_pool(name="vaug", bufs=1))
        vaug = vaug_pool.tile([128, NT * NH * VW], bf16, tag="va", name="va")
        vaug3 = vaug.rearrange("p (t h c) -> p t h c", t=NT, h=NH)
        attn_pool = ctx.enter_context(tc.tile_pool(name="attn", bufs=1))
        # unnormalized PV output incl. bf16 denominator row at partition 96;
        # normalized in place at the end
        attn = attn_pool.tile([128, NH * S], bf16, tag="at", name="at")
        pt_pool = ctx.enter_context(tc.tile_pool(name="pt", bufs=12))
        rc_pool = ctx.enter_context(tc.tile_pool(name="rc", bufs=2))
        rcf_pool = ctx.enter_context(tc.tile_pool(name="rcf", bufs=2))
        bc_pool = ctx.enter_context(tc.tile_pool(name="bc", bufs=2))
        wp_pool = ctx.enter_context(tc.tile_pool(name="wp", bufs=1))
        wp_sb = wp_pool.tile([128, V3_NPJ * D], bf16, tag="wp", name="wp")

        # weights/hidden (dead after phase 1; proj pools reuse the space) —
        # entered last among open pools so the mid-program release is LIFO
        ph1 = tc.tile_pool(name="ph1", bufs=1)
        p1 = ph1.__enter__()
        hid_sb = p1.tile([128, NK * S], bf16, tag="hid", name="hid")
        wqk_sb = p1.tile([128, NJ * NK * 128], bf16, tag="wqk", name="wqk")
        wv_sb = p1.tile([128, NK * 640], bf16, tag="wv", name="wv")
        hid3s = hid_sb.rearrange("p (k s) -> p k s", k=NK)
        wqk4s = wqk_sb.rearrange("p (j k c) -> p j k c", j=NJ, k=NK)
        wv3s = wv_sb.rearrange("p (k c) -> p k c", k=NK)
        # PE warm-up: the cost model prices p-state at dispatch; keep the
        # PE busy on junk matmuls while the input loads stream so the real
        # matmuls dispatch against a warm (2.4 GHz) clock
        wa = p1.tile([128, 16], bf16, tag="wa", name="wa")
        wb = p1.tile([128, 256], bf16, tag="wb", name="wb")
        nc.vector.memset(wa[:, :], 0.0)
        nc.vector.memset(wb[:, :], 0.0)
        wps = psum.tile([128, 1024], f32, tag="po", name="warm")
        for _ in range(30):
            nc.tensor.matmul(wps[0:16, 0:256], wa[:, :], wb[:, :],
                             start=True, stop=True)
        nc.vector.tensor_copy(wa[0:16, 0:4], wps[0:16, 0:4])

        # loads on one queue in priority order; the tail weight loads are
        # deferred into the round loop (just-in-time) so their transfers
        # never sit ahead of the rope staging DMAs on the serial DMA device
        nc.sync.dma_start(vt_sb[:], vtmpl[:])
        nc.sync.dma_start(bias_sb[:], bias2d[:])
        nc.sync.dma_start(wqk4s[:, 0:3, :, :], wqk4[:, 0:3, :, :])
        nc.sync.dma_start(hid3s[:, 0:5, :], hid3[:, 0:5, :])
        nc.sync.dma_start(hid3s[:, 5:NK, :], hid3[:, 5:NK, :])
        nc.sync.dma_start(cos_sb[:], cosP[:])
        nc.sync.dma_start(sin_sb[:], sinP[:])
        # JIT loads must be EMITTED before their first consumer (emission
        # order defines dependency direction), while issuing late enough
        # that their transfers don't delay the rope staging DMAs
        load_at = {
            1: lambda: nc.sync.dma_start(wv3s[:, :, :], wv3[:, :, :]),
            2: lambda: nc.sync.dma_start(wqk4s[:, 3:6, :, :],
                                         wqk4[:, 3:6, :, :]),
            3: lambda: nc.sync.dma_start(wqk4s[:, 6:9, :, :],
                                         wqk4[:, 6:9, :, :]),
            4: lambda: nc.sync.dma_start(wqk4s[:, 9:NJ, :, :],
                                         wqk4[:, 9:NJ, :, :]),
            5: lambda: nc.sync.dma_start(wp_sb[:], wprojT[:]),
        }

        vt3 = vt_sb.rearrange("p (h c) -> p h c", h=NH)

        def emit_qk(j):
            ps = qs_tile()
            for half in range(2):
                c0 = half * 512
                for k in range(NK):
                    nc.tensor.matmul(ps[:, c0:c0 + 512],
                                     wqk4s[:, j, k, :],
                                     hid3s[:, k, c0:c0 + 512],
                                     start=(k == 0), stop=(k == NK - 1))
            if j < 5:
                # early copies on ACT (idle pre-exp): their rope staging
                # DMAs directly follow on the same queue, so the critical
                # startup rope chain never waits in a clogged FIFO
                nc.scalar.activation(qk_sb[:, j * S:(j + 1) * S], ps[:, :],
                                     AF.Identity, bias=bias_sb[:, j:j + 1])
            else:
                nc.vector.tensor_scalar_add(qk_sb[:, j * S:(j + 1) * S],
                                            ps[:, :], bias_sb[:, j:j + 1])

        def emit_v(tt):
            ps = qs_tile()
            for (c0, w) in ((0, 512), (512, 128)):
                for k in range(NK):
                    nc.tensor.matmul(ps[:, c0:c0 + w],
                                     hid3s[:, k, tt * 128:(tt + 1) * 128],
                                     wv3s[:, k, c0:c0 + w],
                                     start=(k == 0), stop=(k == NK - 1))
            src = ps[:, 0:NH * HD].rearrange("p (h c) -> p h c", c=HD)
            nc.vector.tensor_add(vaug3[:, tt, :, 0:HD], src[:, :, :],
                                 vt3[:, :, 0:HD])
            # pad + ones columns (disjoint from the v region written above)
            nc.vector.tensor_copy(vaug3[:, tt, :, HD:VW], vt3[:, :, HD:VW])

        qk3 = qk_sb.rearrange("p (j s) -> p j s", j=NJ)
        stgb_tiles = {}

        def emit_rope_dma(p):
            lo_t, lo_r = _half_pos(2 * p)
            hi_t, hi_r = _half_pos(2 * p + 1)
            b0 = p * S
            if p <= 6:
                eng = nc.scalar
            elif p >= 13:
                eng = nc.gpsimd
            else:
                eng = [nc.gpsimd, nc.sync][p % 2]
            if lo_t == hi_t and hi_r == lo_r + 40:
                eng.dma_start(rot[0:80, b0:b0 + S],
                              qk3[lo_r:lo_r + 80, lo_t, :])
            else:
                eng.dma_start(rot[0:40, b0:b0 + S],
                              qk3[lo_r:lo_r + 40, lo_t, :])
                eng.dma_start(rot[40:80, b0:b0 + S],
                              qk3[hi_r:hi_r + 40, hi_t, :])
            sb = stgb_pool.tile([128, S], bf16, tag="sb", name="sb")
            stgb_tiles[p] = sb
            eng.dma_start(sb[0:40, :], qk3[hi_r:hi_r + 40, hi_t, :])
            eng.dma_start(sb[40:80, :], qk3[lo_r:lo_r + 40, lo_t, :])

        def emit_rope_mul(p):
            b0 = p * S
            sb = stgb_tiles.pop(p)
            nc.vector.tensor_mul(rot[0:80, b0:b0 + S], rot[0:80, b0:b0 + S],
                                 cos_sb[0:80, :])
            meng = nc.gpsimd if p % 4 == 3 else nc.vector
            meng.tensor_mul(sb[0:80, :], sb[0:80, :], sin_sb[0:80, :])
            nc.vector.tensor_add(rot[0:80, b0:b0 + S], rot[0:80, b0:b0 + S],
                                 sb[0:80, :])

        # Attention pump. 512-wide score half-units (unit = (kt, half),
        # 16 per head) on dedicated 1-bank PSUM slots decouple the exp
        # stream from the qk/v slot rotation. Invariant: every emitted
        # instruction's dependencies (incl. slot predecessors) are emitted
        # before it — PVs of head h follow head h-1's evacuation (single po
        # slot), exps run ahead of PVs by at most PT_AHEAD pt tiles.
        PT_AHEAD = 10
        heads_q = []      # started heads, in order
        v_done = [0]
        st_unit = [0]
        outstanding = [0]

        def start_attn(h):
            heads_q.append({"h": h, "se": 0, "pv": 0, "po": None})

        def emit_st_exp(hs):
            h, u = hs["h"], hs["se"]
            kt, half = u // 2, u % 2
            pq, pk = 2 * h, 2 * h + 1
            c0 = half * 512
            st = psum.tile([128, 512], f32, tag=f"st{st_unit[0] % 2}",
                           name="st")
            st_unit[0] += 1
            nc.tensor.matmul(
                st[:, :],
                rot[0:80, pk * S + kt * 128:pk * S + (kt + 1) * 128],
                rot[0:80, pq * S + c0:pq * S + c0 + 512],
                start=True, stop=True)
            pt = pt_pool.tile([128, 512], bf16, tag="pt", name="pt")
            nc.scalar.activation(pt[:, :], st[:, :], AF.Exp)
            hs.setdefault("pts", []).append(pt)
            hs["se"] += 1
            outstanding[0] += 1

        def emit_pv(hs):
            h, u = hs["h"], hs["pv"]
            kt, half = u // 2, u % 2
            if hs["po"] is None:
                hs["po"] = psum.tile([128, 1024], f32, tag="po", name="po")
            c0 = half * 512
            nc.tensor.matmul(hs["po"][0:VW, c0:c0 + 512],
                             vaug3[:, kt, h, :],
                             hs["pts"][u][:, :],
                             start=(kt == 0), stop=(kt == NT - 1))
            hs["pv"] += 1
            outstanding[0] -= 1

        def pump_attn():
            progress = True
            while progress:
                progress = False
                if heads_q:
                    hs = heads_q[0]
                    while (hs["pv"] < hs["se"]
                           and hs["pv"] // 2 < v_done[0]):
                        emit_pv(hs)
                        progress = True
                    if hs["pv"] == 2 * NT:
                        h = hs["h"]
                        # evacuate on ACT so the po slot frees immediately
                        # and normalize never touches compute FIFOs
                        nc.scalar.activation(attn[0:VW, h * S:(h + 1) * S],
                                             hs["po"][0:VW, :], AF.Identity)
                        if h <= 5:
                            emit_norm(h)
                        heads_q.pop(0)
                        progress = True
                        continue
                for hs in heads_q:
                    while (hs["se"] < 2 * NT
                           and outstanding[0] < PT_AHEAD):
                        emit_st_exp(hs)
                        progress = True

        def emit_norm(h):
            rc = rc_pool.tile([1, S], bf16, tag="rc", name="rc")
            nc.scalar.dma_start(rc[0:1, :], attn[96:97, h * S:(h + 1) * S])
            rcf = rcf_pool.tile([1, S], f32, tag="rcf", name="rcf")
            nc.vector.reciprocal(rcf[0:1, :], rc[0:1, :])
            bc = bc_pool.tile([80, S], f32, tag="bc", name="bc")
            nc.gpsimd.partition_broadcast(bc[0:80, :], rcf[0:1, :])
            # late heads' muls on Pool: DVE is busy with the output adds then
            meng = nc.gpsimd if h >= 6 else nc.vector
            meng.tensor_mul(attn[0:80, h * S:(h + 1) * S],
                            attn[0:80, h * S:(h + 1) * S], bc[0:80, :])

        # emission driver: qk j-tiles paced with v, rope, attention
        v_at = {1: (0, 1), 2: (2, 3), 3: (4, 5), 4: (6, 7)}
        rope_at = {}
        for p in range(16):
            jmax = max(_half_pos(2 * p)[0], _half_pos(2 * p + 1)[0])
            rope_at.setdefault(jmax, []).append(p)
        pending = []
        for j in range(NJ):
            if j in load_at:
                load_at[j]()
            emit_qk(j)
            # rope muls one round behind their staging DMAs so the DVE FIFO
            # never stalls on in-flight DMA latency
            for p in pending:
                emit_rope_mul(p)
                if p % 2 == 1:
                    start_attn(p // 2)
            pending = []
            for tt in v_at.get(j, ()):
                emit_v(tt)
                v_done[0] += 1
            pump_attn()
            for p in rope_at.get(j, ()):
                emit_rope_dma(p)
                pending.append(p)
        for p in pending:
            emit_rope_mul(p)
            if p % 2 == 1:
                start_attn(p // 2)
        pump_attn()

        if KERNEL_DEBUG:
            nc.sync.dma_start(dbg_qk[:, :], qk_sb[:, :])
            nc.sync.dma_start(dbg_rot[0:80, :], rot[0:80, :])
            nc.sync.dma_start(dbg_vaug[:, :], vaug[:, :])
            nc.sync.dma_start(dbg_attn[0:97, :], attn[0:97, :])

        ph1.__exit__(None, None, None)

        # normalize + dense re-pack of attn heads + split projection:
        # kt0-3 prepass overlaps the last heads' attention, kt4 finishes
        with ExitStack() as p5:
            late = p5.enter_context(tc.tile_pool(name="late", bufs=1))
            dense = late.tile([128, V3_NPJ * S], bf16, tag="dn", name="dn")
            ob_pool = p5.enter_context(tc.tile_pool(name="ob", bufs=3))
            oa_pool = p5.enter_context(tc.tile_pool(name="oa", bufs=1))
            dense3 = dense.rearrange("p (t s) -> p t s", t=V3_NPJ)
            wp3 = wp_sb.rearrange("p (t c) -> p t c", t=V3_NPJ)

            def emit_densify(h):
                for (dt, r, n, off) in _pieces(80 * h, 80):
                    nc.sync.dma_start(
                        dense3[r:r + n, dt, :],
                        attn[off:off + n, h * S:(h + 1) * S])

            emit_norm(NH - 2)
            for h in range(NH - 1):
                emit_densify(h)
            KA = 3   # prepass contracts kt0..KA-1 (early heads), final the rest
            oa_tiles = []
            for j in range(NK):
                ps = qs_tile()
                for half in range(2):
                    c0 = half * 512
                    for kt in range(KA):
                        nc.tensor.matmul(
                            ps[:, c0:c0 + 512],
                            wp3[:, kt, j * 128:(j + 1) * 128],
                            dense3[:, kt, c0:c0 + 512],
                            start=(kt == 0), stop=(kt == KA - 1))
                oa = oa_pool.tile([128, S], f32, tag=f"oa{j}", name=f"oa{j}",
                                  bufs=1)
                if j % 2:
                    nc.scalar.activation(oa[:, :], ps[:, :], AF.Identity)
                else:
                    nc.vector.tensor_copy(oa[:, :], ps[:, :])
                oa_tiles.append(oa)
            emit_norm(NH - 1)
            emit_densify(NH - 1)
            for j in range(NK):
                ps = qs_tile()
                for half in range(2):
                    c0 = half * 512
                    for kt in range(KA, V3_NPJ):
                        nc.tensor.matmul(ps[:, c0:c0 + 512],
                                         wp3[:, kt, j * 128:(j + 1) * 128],
                                         dense3[:, kt, c0:c0 + 512],
                                         start=(kt == KA),
                                         stop=(kt == V3_NPJ - 1))
                ob = ob_pool.tile([128, S], bf16, tag="ob", name="ob")
                nc.vector.tensor_add(ob[:, :], ps[:, :], oa_tiles[j][:, :])
                nc.sync.dma_start(outT[j * 128:(j + 1) * 128, :], ob[:, :])

    nc.compile()
    return nc


def _pack_v3(Wqkv, bqkv, Wproj, bproj, g):
    """Host-side per-head-group weight packing for the v3 program."""
    import concourse.mybir as mybir
    bf16 = mybir.dt.np(mybir.dt.bfloat16)
    NH, NJ, NK, VW = V3_NH, V3_NJ, V3_NK, V3_VW

    Wp = np.zeros((NJ * 128, D), np.float32)
    bp = np.zeros((NJ * 128,), np.float32)
    for m in range(32):
        h = m // 4
        sec = (m % 4) // 2       # 0 = q, 1 = k
        half = m % 2
        src = sec * D + (g * NH + h) * HD + half * BLK
        w = Wqkv[src:src + BLK, :]
        b = bqkv[src:src + BLK]
        if sec == 0:
            w = w * SCALE
            b = b * SCALE
        t, r = _half_pos(m)
        Wp[t * 128 + r:t * 128 + r + BLK] = w
        bp[t * 128 + r:t * 128 + r + BLK] = b
    # lhsT layout [128, j, k, 128]: wqkT[p, j, k, c] = Wp[j*128+c, k*128+p]
    wqkT = np.ascontiguousarray(
        Wp.reshape(NJ, 128, NK, 128).transpose(3, 0, 2, 1)
        .reshape(128, NJ * NK * 128)).astype(bf16)
    bias2d = np.ascontiguousarray(bp.reshape(NJ, 128).T)

    Wv = Wqkv[2 * D + g * 640:2 * D + (g + 1) * 640, :]
    wvT = _tile_rows(np.ascontiguousarray(Wv.T)).astype(bf16)
    bv = bqkv[2 * D + g * 640:2 * D + (g + 1) * 640]

    vt = np.zeros((128, NH * VW), np.float32)
    for h in range(NH):
        vt[:, h * VW:h * VW + HD] = bv[h * HD:(h + 1) * HD][None, :]
        vt[:, h * VW + 96] = 1.0
    vtmpl = vt.astype(bf16)

    Wpd = Wproj[:, g * 640:(g + 1) * 640].T  # [640, 1280] dense attn rows
    wprojT = _tile_rows(np.ascontiguousarray(Wpd)).astype(bf16)
    return wqkT, bias2d, wvT, vtmpl, wprojT


def _cos_sin_v3(cos, sin):
    """Dense [0:80] rope coefficient layouts (bf16), full sequence."""
    import concourse.mybir as mybir
    bf16 = mybir.dt.np(mybir.dt.bfloat16)
    S = cos.shape[0]
    cp = np.zeros((128, S), np.float32)
    sp = np.zeros((128, S), np.float32)
    cp[0:BLK] = cos.T[0:BLK]
    cp[BLK:HD] = cos.T[BLK:HD]
    sp[0:BLK] = -sin.T[0:BLK]
    sp[BLK:HD] = sin.T[BLK:HD]
    return cp.astype(bf16), sp.astype(bf16)


def kernel(hidden_states, cos, sin, Wqkv, bqkv, Wproj, bproj, cu_seqlens):
    sys.path.insert(0, "/opt/trn_rl_repo")
    from concourse import bass_utils

    hidden_states = np.asarray(hidden_states, np.float32)
    cos = np.asarray(cos, np.float32)
    sin = np.asarray(sin, np.float32)
    Wqkv = np.asarray(Wqkv, np.float32)
    bqkv = np.asarray(bqkv, np.float32)
    Wproj = np.asarray(Wproj, np.float32)
    bproj = np.asarray(bproj, np.float32)

    S, D_ = hidden_states.shape
    assert D_ == D
    segs = _segments(cu_seqlens, S)
    uniform = (S % 4 == 0) and segs == [(i * S // 4, (i + 1) * S // 4)
                                        for i in range(4)]

    hiddenT = np.ascontiguousarray(hidden_states.T)
    cosP, sin2P = _pack_cos_sin(cos, sin)

    def _vinit(segs_local):
        n_tt = sum(-(-(e - a) // 128) for a, e in segs_local)
        v = np.zeros((128, n_tt, 17), np.float32)
        v[:, :, 16] = 1.0
        return np.ascontiguousarray(v.reshape(128, n_tt * 17))

    if uniform:
        # v3: 2 head-groups x 4 segments, bf16 on-chip
        import concourse.mybir as mybir
        bf16 = mybir.dt.np(mybir.dt.bfloat16)
        S_core = S // 4
        key = ("V3", S)
        if key not in _CACHE:
            _CACHE[key] = _build_v3()
        nc = _CACHE[key]
        cosPd, sinPd = _cos_sin_v3(cos, sin)
        hidT_b = hiddenT.astype(bf16)
        in_maps = []
        meta = []
        for g in range(2):
            wqkT, b2, wvT, vtmpl, wprojT = _pack_v3(Wqkv, bqkv, Wproj,
                                                    bproj, g)
            for s in range(4):
                sl = slice(s * S_core, (s + 1) * S_core)
                in_maps.append({
                    "hidT": _tile_rows(hidT_b[:, sl]),
                    "wqkT": wqkT,
                    "bias2d": b2,
                    "wvT": wvT,
                    "vtmpl": vtmpl,
                    "cosP": np.ascontiguousarray(cosPd[:, sl]),
                    "sinP": np.ascontiguousarray(sinPd[:, sl]),
                    "wprojT": wprojT,
                })
                meta.append((g, s))
        res = bass_utils.run_bass_kernel_spmd(nc, in_maps,
                                              core_ids=list(range(N_CORES)))
        out = np.zeros((D, S), np.float32)
        for c, (g, s) in enumerate(meta):
            out[:, s * S_core:(s + 1) * S_core] += \
                res.results[c]["outT"].astype(np.float32)
    else:
        # mode C: 8-way head parallel, full sequence per core
        n_h, S_core = H // N_CORES, S
        key = ("C", S, tuple(np.asarray(cu_seqlens).tolist()))
        if key not in _CACHE:
            _CACHE[key] = _build_program(n_h, S_core, segs,
                                         resident_hidden=False)
        nc = _CACHE[key]
        vinit = _vinit(segs)
        hid_tiled = _tile_rows(hiddenT)
        in_maps = []
        for c in range(N_CORES):
            heads = list(range(c * n_h, (c + 1) * n_h))
            wt, b2 = _pack_w(Wqkv, bqkv, heads, n_h)
            in_maps.append({
                "hiddenT": hid_tiled,
                "wqkvT": wt,
                "bias2d": b2,
                "cosP": cosP,
                "sin2P": sin2P,
                "wprojT": _pack_wproj(Wproj, heads),
                "vinit": vinit,
            })
        res = bass_utils.run_bass_kernel_spmd(nc, in_maps,
                                              core_ids=list(range(N_CORES)))
        out = np.zeros((D, S), np.float32)
        for c in range(N_CORES):
            out += res.results[c]["outT"]

    return np.ascontiguousarray(out.T) + bproj[None, :]



# revision 112
# speedup vs baseline: 1.4529x; 1.0003x over previous
"""Trainium2 Bass kernel for Ernie4.5-VL vision attention (ragged segments).

Contract: kernel(**inputs) takes the FULL unsharded inputs (keyed as in
setup_inputs()) and returns the FULL [S, D] float32 output.

Fast path (uniform 4x1024 segments) — the v3 program, see _build_v3:
2 head-groups x 4 segments across 8 cores, bf16 on-chip, ~179us. Key
ideas: dense q/k weight packing (11 j-tiles), v computed untransposed
straight into the PV operand slots, RoPE staged via DMA into dense
[0:80] layouts (K=80 score contraction), 512-wide exp units on
dedicated PSUM slots, ones-column denominator trick, and a projection
over 5 dense re-packed K-tiles split into a kt0-2 prepass (overlaps
the last heads' attention) + kt3-4 finish.
Scheduling: emission order defines both Tile dependency direction and
scheduler priority, so readers are never emitted before their writers,
latency-bound chains (normalize) are kept off the compute-critical
FIFOs, and a PE warm-up bridges the load phase (the cost model prices
the PE p-state at dispatch time).

Fallback (any other cu_seqlens): the original fp32r program below —
8-way head parallel, every core sees all segments.

Host does only O(S*D) glue: input transposes/packing bf16 conversion,
summing the per-token partial projections, and the bias adds.
"""

import os
import sys

import numpy as np

H = 16
HD = 80
BLK = 40  # rotate_half half-width
SCALE = HD ** -0.5
N_CORES = 8
D = 1280
NK = D // 128  # contraction tiles for the qkv matmul
ATTN_STRIDE = 96  # head row pitch in the packed attention output
MM_DT_NAME = os.environ.get("KERNEL_MM_DT", "float32r")  # or "float32"
KERNEL_DEBUG = bool(int(os.environ.get("KERNEL_DEBUG", "0")))


def _segments(cu_seqlens, S):
    """Intervals matching reference's searchsorted(cu[1:], i, 'right')."""
    b = np.clip(np.sort(np.asarray(cu_seqlens, dtype=np.int64)[1:5]), 0, S)
    bounds = [0] + list(b) + [S]
    segs = []
    for a, e in zip(bounds[:-1], bounds[1:]):
        if e > a:
            segs.append((int(a), int(e)))
    return segs


def _pack_layout(n_h):
    """Pack per-core qkv dims as 40-row blocks, 3 per 128-row tile (8 pad).

    Each tile holds one v-block at row 0 (PE transpose operands must start
    at a 32-aligned partition) and two q/k blocks at rows 40 and 80.
    Returns pos[(sec, h, half)] = (tile, row) and the number of tiles.
    """
    ntiles = 2 * n_h
    pos = {}
    for h in range(n_h):
        for half in (0, 1):
            pos[("v", h, half)] = (2 * h + half, 0)
    qk = [("q", h, half) for h in range(n_h) for half in (0, 1)]
    qk += [("k", h, half) for h in range(n_h) for half in (0, 1)]
    for j, blk in enumerate(qk):
        pos[blk] = (j // 2, BLK + BLK * (j % 2))
    return pos, ntiles


def _pieces(start, length, tile_rows=128):
    """Split global row range [start, start+length) into per-tile pieces."""
    out = []
    off = 0
    while off < length:
        g = start + off
        t, r = g // tile_rows, g % tile_rows
        n = min(tile_rows - r, length - off)
        out.append((t, r, n, off))
        off += n
    return out


def _proj_k_tiles(n_h):
    rows = ATTN_STRIDE * n_h
    kt = [128] * (rows // 128)
    if rows % 128:
        kt.append(rows % 128)
    return kt


def _build_program(n_h, S_core, segs_local, resident_hidden):
    """Emit the SPMD program. Same structure for every core.

    Engine-AP partition rules on TRN2 (walrus birverifier): compute-engine
    accesses must start at a 32-aligned partition and must not cross a
    64-boundary unless they start on one; cross-partition data movement
    must go through DMA. The layout choices below all follow from this.
    """
    import concourse.mybir as mybir
    import concourse.tile as tile
    from concourse import bacc
    from concourse.masks import make_identity
    from contextlib import ExitStack

    f32 = mybir.dt.float32
    mm_dt = getattr(mybir.dt, MM_DT_NAME)
    AF = mybir.ActivationFunctionType

    k_proj = n_h
    pos, n_mtiles = _pack_layout(n_h)
    dims_pad = n_mtiles * 128
    VW = 97  # v_aug slot width: 80 v dims + 16 zero pad + ones col at 96

    # global key-tile list: (seg_idx, t0, t1)
    t_tiles = []
    for si, (a, e) in enumerate(segs_local):
        t = a
        while t < e:
            t_tiles.append((si, t, min(t + 128, e)))
            t += 128
    n_tt = len(t_tiles)

    nc = bacc.Bacc("TRN2", target_bir_lowering=False, debug=False,
                   enable_asserts=False, num_devices=N_CORES)

    # host supplies hiddenT/wqkvT pre-tiled into 128-partition-major layout
    hiddenT = nc.dram_tensor("hiddenT", [128, NK * S_core], mm_dt,
                             kind="ExternalInput").ap()
    wqkvT = nc.dram_tensor("wqkvT", [128, NK * dims_pad], mm_dt,
                           kind="ExternalInput").ap()
    bias2d = nc.dram_tensor("bias2d", [128, n_mtiles], f32,
                            kind="ExternalInput").ap()
    # cosP/sin2P are host-packed [128, S]: rows 0:40 and 64:104 hold the
    # lo/hi rope coefficients, all other rows zero (zeroes the junk rows
    # of the rotated q/k so the K=104 score matmuls see exact zeros).
    cosP = nc.dram_tensor("cosP", [128, S_core], mm_dt,
                          kind="ExternalInput").ap()
    sin2P = nc.dram_tensor("sin2P", [128, S_core], mm_dt,
                           kind="ExternalInput").ap()
    wprojT = nc.dram_tensor("wprojT", [n_h * HD, D], mm_dt,
                            kind="ExternalInput").ap()
    # per-key-tile v_aug tail init: 16 zero pad cols + ones col (f32r memset
    # fails walrus codegen, so this comes in via DMA)
    vinit = nc.dram_tensor("vinit", [128, n_tt * (VW - HD)], mm_dt,
                           kind="ExternalInput").ap()
    outT = nc.dram_tensor("outT", [D, S_core], f32, kind="ExternalOutput").ap()
    if KERNEL_DEBUG:
        dbg_qkv = nc.dram_tensor("dbg_qkv", [128, n_mtiles * S_core], f32,
                                 kind="ExternalOutput").ap()
        dbg_rot = nc.dram_tensor("dbg_rot", [128, 2 * n_h * S_core], f32,
                                 kind="ExternalOutput").ap()
        dbg_vaug = nc.dram_tensor("dbg_vaug", [128, n_h * n_tt * VW], f32,
                                  kind="ExternalOutput").ap()
        dbg_attn = nc.dram_tensor("dbg_attn", [128, n_h * S_core], f32,
                                  kind="ExternalOutput").ap()

    def r_(ap):
        return ap.bitcast(mm_dt)

    BC = 1024  # psum tile width (2 banks); matmuls stream <=512
    big_chunks = [(c, min(c + BC, S_core)) for c in range(0, S_core, BC)]

    def halves(c0, c1):
        out = []
        q = c0
        while q < c1:
            out.append((q, min(q + 512, c1)))
            q = q + 512
        return out

    with tile.TileContext(nc) as tc, ExitStack() as ctx:
        persist = ctx.enter_context(tc.tile_pool(name="persist", bufs=1))
        ident = persist.tile([128, 128], f32, tag="ident", name="ident")
        make_identity(nc, ident[:])
        bias_sb = persist.tile([128, n_mtiles], f32, tag="bias", name="bias")
        nc.sync.dma_start(bias_sb[:], bias2d[:])

        # PSUM: two 2-bank slots (t0/t1) shared by qkv/scores/proj, two
        # 1-bank slots for v-transposes, one 2-bank slot for PV accumulate
        psum_all_cm = tc.tile_pool(name="psum_all", bufs=1, space="PSUM")
        psum_all = psum_all_cm.__enter__()
        # big pool: qkvT tiles (phases 1-3), slots reused by attn (phases 4-5)
        qkv_pool = ctx.enter_context(tc.tile_pool(name="big", bufs=1))
        qkv_sb = [qkv_pool.tile([128, S_core], mm_dt, tag=f"qkvT{j}",
                                name=f"qkvT{j}") for j in range(n_mtiles)]
        # rope output (rows 0:104 live, 40:64 zeroed via cosP/sin2P pads)
        rot_cm = tc.tile_pool(name="rot", bufs=1)
        rv = rot_cm.__enter__()
        rot_sb = {}
        for h in range(n_h):
            for sec in ("q", "k"):
                rot_sb[(sec, h)] = rv.tile([128, S_core], mm_dt,
                                           tag=f"rot_{sec}{h}",
                                           name=f"rot_{sec}{h}")
        RC = 1024
        rope_cm = tc.tile_pool(name="rope_scr", bufs=2)
        rope_scr = rope_cm.__enter__()

        # ------------ phase 1: qkvT = Wpack @ hidden.T --------------
        with ExitStack() as p1:
            hidden3 = hiddenT.rearrange("p (k s) -> p k s", k=NK)
            w3 = wqkvT.rearrange("p (k m) -> p k m", k=NK)
            if resident_hidden:
                hid_pool = p1.enter_context(tc.tile_pool(name="hid", bufs=1))
                w_pool = p1.enter_context(tc.tile_pool(name="wstream", bufs=3))
                hid_sb = [hid_pool.tile([128, S_core], mm_dt, tag=f"hid{k}",
                                        name=f"hid{k}") for k in range(NK)]
                wj0 = w_pool.tile([128, NK * 128], mm_dt, tag="wj", name="wj")
                nc.sync.dma_start(hid_sb[0][:], hidden3[:, 0, :])
                nc.sync.dma_start(
                    wj0.rearrange("p (k m) -> p k m", k=NK)[:, :, :],
                    w3[:, :, 0:128])
                for k in range(1, NK):
                    nc.sync.dma_start(hid_sb[k][:], hidden3[:, k, :])
                for j in range(n_mtiles):
                    if j == 0:
                        wj = wj0
                    else:
                        wj = w_pool.tile([128, NK * 128], mm_dt, tag="wj",
                                         name="wj")
                        nc.sync.dma_start(
                            wj.rearrange("p (k m) -> p k m", k=NK)[:, :, :],
                            w3[:, :, j * 128:(j + 1) * 128])
                    for (h0, h1) in halves(0, S_core):
                        hw = h1 - h0
                        ps = psum_all.tile([128, 512], f32,
                                           tag=f"t{(h0 // 512) % 2}",
                                           name="qkvp")
                        for k in range(NK):
                            nc.tensor.matmul(
                                ps[:, :hw],
                                r_(wj[:, k * 128:(k + 1) * 128]),
                                r_(hid_sb[k][:, h0:h1]),
                                start=(k == 0), stop=(k == NK - 1))
                        nc.scalar.activation(qkv_sb[j][:, h0:h1], ps[:, :hw],
                                             AF.Identity,
                                             bias=bias_sb[:, j:j + 1])
            else:
                # k-outer streaming: two psum slots hold four j-streams
                # (columns 0:512 and 512:1024), hidden tiles are tiny
                w_pool = p1.enter_context(tc.tile_pool(name="wres", bufs=1))
                w_sb = [w_pool.tile([128, dims_pad], mm_dt, tag=f"w{k}",
                                    name=f"w{k}") for k in range(NK)]
                for k in range(NK):
                    nc.sync.dma_start(w_sb[k][:], w3[:, k, :])
                assert n_mtiles == 4
                hid_pool = p1.enter_context(tc.tile_pool(name="hidstream",
                                                         bufs=3))
                for (h0, h1) in halves(0, S_core):
                    hw = h1 - h0
                    ps01 = psum_all.tile([128, BC], f32, tag="t0", name="ps01")
                    ps23 = psum_all.tile([128, BC], f32, tag="t1", name="ps23")
                    pj_of = lambda j: (ps01 if j < 2 else ps23,
                                       (j % 2) * 512)
                    for k in range(NK):
                        ht = hid_pool.tile([128, 512], mm_dt, tag="hidc",
                                           name="hidc")
                        nc.sync.dma_start(ht[:, :hw], hidden3[:, k, h0:h1])
                        for j in range(n_mtiles):
                            psj, co = pj_of(j)
                            nc.tensor.matmul(
                                psj[:, co:co + hw],
                                r_(w_sb[k][:, j * 128:(j + 1) * 128]),
                                r_(ht[:, :hw]),
                                start=(k == 0), stop=(k == NK - 1))
                    for j in range(n_mtiles):
                        psj, co = pj_of(j)
                        nc.scalar.activation(qkv_sb[j][:, h0:h1],
                                             psj[:, co:co + hw], AF.Identity,
                                             bias=bias_sb[:, j:j + 1])

        psum_all_cm.__exit__(None, None, None)
        ps_att = ctx.enter_context(tc.tile_pool(name="ps_att", bufs=1,
                                                space="PSUM"))

        # ------------ phase 2: RoPE --------------------------------
        # DMA-stage lo/hi into 0:40 / 64:104 (stgA) and swapped (stgB),
        # then rot = stgA*cosP + stgB*sin2P as three same-base wide ops.
        # double-buffered persistent staging tensors; rows 40:64 zeroed once
        # from cosP's zero rows so the [0:104) products read defined zeros
        stg = {}
        for nm in ("sa0", "sa1", "sb0", "sb1"):
            stg[nm] = rope_scr.tile([128, RC], mm_dt, tag=nm, name=nm, bufs=1)
        pair_i = 0
        for ci, f0 in enumerate(range(0, S_core, RC)):
            f1 = min(f0 + RC, S_core)
            fs = f1 - f0
            cos_sb = rope_scr.tile([128, RC], mm_dt, tag="cos", name="cos",
                                   bufs=1)
            sin_sb = rope_scr.tile([128, RC], mm_dt, tag="sin", name="sin",
                                   bufs=1)
            nc.scalar.dma_start(cos_sb[:, :fs], cosP[:, f0:f1])
            nc.scalar.dma_start(sin_sb[:, :fs], sin2P[:, f0:f1])
            if ci == 0:
                for nm in stg:
                    nc.scalar.dma_start(stg[nm][BLK:64, :], cos_sb[BLK:64, :])
            for h in range(n_h):
                for sec in ("q", "k"):
                    lo_t, lo_r = pos[(sec, h, 0)]
                    hi_t, hi_r = pos[(sec, h, 1)]
                    assert hi_t == lo_t and hi_r == lo_r + BLK
                    x = qkv_sb[lo_t]
                    dst = rot_sb[(sec, h)]
                    stga = stg[f"sa{pair_i % 2}"]
                    stgb = stg[f"sb{pair_i % 2}"]
                    nc.scalar.dma_start(stga[0:BLK, :fs],
                                        x[lo_r:lo_r + BLK, f0:f1])
                    nc.scalar.dma_start(stga[64:64 + BLK, :fs],
                                        x[hi_r:hi_r + BLK, f0:f1])
                    nc.scalar.dma_start(stgb[0:BLK, :fs],
                                        x[hi_r:hi_r + BLK, f0:f1])
                    nc.scalar.dma_start(stgb[64:64 + BLK, :fs],
                                        x[lo_r:lo_r + BLK, f0:f1])
                    nc.vector.tensor_mul(dst[0:104, f0:f1], stga[0:104, :fs],
                                         cos_sb[0:104, :fs])
                    eng = nc.gpsimd if pair_i % 2 == 0 else nc.vector
                    eng.tensor_mul(stgb[0:104, :fs], stgb[0:104, :fs],
                                   sin_sb[0:104, :fs])
                    nc.vector.tensor_add(dst[0:104, f0:f1], dst[0:104, f0:f1],
                                         stgb[0:104, :fs])
                    pair_i += 1
        rope_cm.__exit__(None, None, None)

        # v_aug tiles + per-head emitter (invoked right after each head's
        # rope so attention unblocks head by head)
        vaug_cm = tc.tile_pool(name="vaug", bufs=1)
        vaug_pool = vaug_cm.__enter__()
        vaug_sb = [vaug_pool.tile([128, n_tt * VW], mm_dt, tag=f"vaug{h}",
                                  name=f"vaug{h}") for h in range(n_h)]
        vinit3 = vinit.rearrange("p (t c) -> p t c", c=VW - HD)
        for h in range(n_h):
            nc.sync.dma_start(
                vaug_sb[h].rearrange("p (t c) -> p t c", c=VW)[:, :, HD:VW],
                vinit3[:, :, :])
        GRP = 4  # key tiles transposed per psum tile / copy (1 psum bank)

        def emit_vaug(h):
            gi = 0
            while gi < n_tt:
                hi_g = min(gi + GRP, n_tt)
                if all(t_tiles[g][2] - t_tiles[g][1] == 128
                       for g in range(gi, hi_g)):
                    grp = list(range(gi, hi_g))
                else:
                    grp = [gi]
                ng = len(grp)
                tp = ps_att.tile([128, GRP * HD], f32, tag="tp", name="tp")
                for x, g in enumerate(grp):
                    si, t0, t1 = t_tiles[g]
                    sz = t1 - t0
                    for half in (0, 1):
                        vt, vr = pos[("v", h, half)]
                        nc.tensor.transpose(
                            tp[:sz, x * HD + half * BLK:
                               x * HD + (half + 1) * BLK],
                            qkv_sb[vt][0:BLK, t0:t1].bitcast(f32),
                            ident[:BLK, :BLK])
                sz0 = t_tiles[grp[0]][2] - t_tiles[grp[0]][1]
                dst = vaug_sb[h].rearrange("p (t c) -> p t c", c=VW)
                src_ap = tp.rearrange("p (t c) -> p t c", c=HD)
                if h % 2 == 0:
                    nc.vector.tensor_copy(dst[:sz0, grp[0]:grp[0] + ng, 0:HD],
                                          src_ap[:sz0, 0:ng, :])
                else:
                    nc.scalar.activation(dst[:sz0, grp[0]:grp[0] + ng, 0:HD],
                                         src_ap[:sz0, 0:ng, :], AF.Identity)
                gi += ng




        if KERNEL_DEBUG:
            for j in range(n_mtiles):
                nc.sync.dma_start(
                    dbg_qkv[:, j * S_core:(j + 1) * S_core],
                    qkv_sb[j][:].bitcast(f32))
            i_ = 0
            for h in range(n_h):
                for sec in ("q", "k"):
                    nc.sync.dma_start(
                        dbg_rot[:, i_ * S_core:(i_ + 1) * S_core],
                        rot_sb[(sec, h)][:].bitcast(f32))
                    i_ += 1

        # ------------ phase 4: attention ----------------------------
        # one attn tile per head (rows 0:80) so every compute access is
        # partition-0 based; tiles reuse the dead qkvT slots
        attn_sb = [qkv_pool.tile([128, S_core], mm_dt, tag=f"qkvT{h}",
                                 name=f"attnT{h}") for h in range(n_h)]

        seg_ttiles = {}
        for ti, (si, t0, t1) in enumerate(t_tiles):
            seg_ttiles.setdefault(si, []).append((ti, t0, t1))

        BA = 512  # attention query-chunk width (1-bank psum slots)
        with ExitStack() as p4:
            pt_pool = p4.enter_context(tc.tile_pool(name="pt", bufs=3))
            nrm_pool = p4.enter_context(tc.tile_pool(name="nrm", bufs=2))
            unit_box = [0]

            def emit_attention(h, si, a, e):
                qT = rot_sb[("q", h)]
                kT = rot_sb[("k", h)]
                q = a
                while q < e:
                    q0, q1 = q, min(q + BA, e)
                    qs = q1 - q0
                    po = ps_att.tile([128, BA], f32,
                                     tag=f"po{unit_box[0] % 2}", name="pv")
                    tts = seg_ttiles[si]
                    for idx, (ti, t0, t1) in enumerate(tts):
                        sz = t1 - t0
                        ps = ps_att.tile([128, BA], f32, tag=f"st{idx % 2}",
                                         name="st")
                        nc.tensor.matmul(ps[:sz, :qs], r_(kT[0:104, t0:t1]),
                                         r_(qT[0:104, q0:q1]),
                                         start=True, stop=True)
                        pt = pt_pool.tile([128, BA], mm_dt, tag="pt", name="pt")
                        nc.scalar.activation(pt[:sz, :qs], ps[:sz, :qs], AF.Exp)
                        nc.tensor.matmul(
                            po[:VW, :qs],
                            r_(vaug_sb[h][:sz, ti * VW:(ti + 1) * VW]),
                            r_(pt[:sz, :qs]),
                            start=(idx == 0), stop=(idx == len(tts) - 1))
                    # partition_broadcast ucode reads physical partition 0,
                    # so shift the denominator row 96 -> 0 via DMA
                    rc = nrm_pool.tile([128, BA], f32, tag="rc", name="rc")
                    nc.vector.tensor_copy(rc[96:97, :qs], po[96:97, :qs])
                    nc.sync.dma_start(rc[0:1, :qs], rc[96:97, :qs])
                    nc.vector.reciprocal(rc[0:1, :qs], rc[0:1, :qs])
                    bc = nrm_pool.tile([128, BA], mm_dt, tag="bc", name="bc")
                    nc.gpsimd.partition_broadcast(
                        bc[0:HD, :qs], rc[0:1, :qs].bitcast(mm_dt))
                    nc.vector.tensor_mul(attn_sb[h][0:HD, q0:q1],
                                         po[0:HD, :qs], bc[0:HD, :qs])
                    unit_box[0] += 1
                    q = q1

            if len(segs_local) == 1:
                a, e = segs_local[0]
                for h in range(n_h):
                    emit_vaug(h)
                    emit_attention(h, 0, a, e)
            else:
                for h in range(n_h):
                    emit_vaug(h)
                for si, (a, e) in enumerate(segs_local):
                    for h in range(n_h):
                        emit_attention(h, si, a, e)

        vaug_cm.__exit__(None, None, None)
        rot_cm.__exit__(None, None, None)

        # ------------ phase 5: projection partial -------------------
        with ExitStack() as p5:
            wp_pool = p5.enter_context(tc.tile_pool(name="wp", bufs=1))
            wp_sb = []
            for kt in range(k_proj):
                t = wp_pool.tile([HD, D], mm_dt, tag=f"wp{kt}", name=f"wp{kt}")
                nc.sync.dma_start(t[:], wprojT[kt * HD:(kt + 1) * HD, :])
                wp_sb.append(t)
            out_pool = p5.enter_context(tc.tile_pool(name="outsb", bufs=3))
            for (c0, c1) in big_chunks:
                cs = c1 - c0
                for j in range(D // 128):
                    ob = out_pool.tile([128, BC], f32, tag="ob", name="ob")
                    for (h0, h1) in halves(c0, c1):
                        ps = ps_att.tile([128, 512], f32, tag=f"st{j % 2}",
                                         name="pj")
                        for kt in range(k_proj):
                            nc.tensor.matmul(
                                ps[:, :h1 - h0],
                                r_(wp_sb[kt][:, j * 128:(j + 1) * 128]),
                                r_(attn_sb[kt][0:HD, h0:h1]),
                                start=(kt == 0), stop=(kt == k_proj - 1))
                        if j % 2 == 0:
                            nc.vector.tensor_copy(ob[:, h0 - c0:h1 - c0],
                                                  ps[:, :h1 - h0])
                        else:
                            nc.scalar.activation(ob[:, h0 - c0:h1 - c0],
                                                 ps[:, :h1 - h0], AF.Identity)
                    nc.sync.dma_start(outT[j * 128:(j + 1) * 128, c0:c1],
                                      ob[:, :cs])

    nc.compile()
    return nc


def _pack_w(Wqkv, bqkv, heads, n_h):
    """Per-core packed qkv weights (q rows pre-scaled).

    Returns wqkvT_tiled [128, NK*dims_pad] (k-major blocks of [128, dims_pad])
    and bias2d [128, n_mtiles]."""
    pos, n_mtiles = _pack_layout(n_h)
    dims_pad = n_mtiles * 128
    W = np.zeros((dims_pad, D), np.float32)
    b = np.zeros((dims_pad,), np.float32)
    sec_off = {"q": 0, "k": D, "v": 2 * D}
    for i, h in enumerate(heads):
        for sec in ("q", "k", "v"):
            for half in (0, 1):
                t, r = pos[(sec, i, half)]
                src = sec_off[sec] + h * HD + half * BLK
                w = Wqkv[src:src + BLK, :]
                bb = bqkv[src:src + BLK]
                if sec == "q":
                    w = w * SCALE
                    bb = bb * SCALE
                W[t * 128 + r:t * 128 + r + BLK] = w
                b[t * 128 + r:t * 128 + r + BLK] = bb
    w_tiled = _tile_rows(np.ascontiguousarray(W.T))
    bias2d = np.ascontiguousarray(b.reshape(n_mtiles, 128).T)
    return w_tiled, bias2d


def _tile_rows(x):
    """[R, C] with R = nk*128 -> [128, nk*C] k-major tiling."""
    R, C = x.shape
    nk = R // 128
    return np.ascontiguousarray(
        x.reshape(nk, 128, C).transpose(1, 0, 2).reshape(128, nk * C))


def _pack_wproj(Wproj, heads):
    """Rows of Wproj.T for this core's head dims, stacked per head."""
    W = np.zeros((len(heads) * HD, Wproj.shape[0]), np.float32)
    for i, h in enumerate(heads):
        W[i * HD:(i + 1) * HD] = Wproj[:, h * HD:(h + 1) * HD].T
    return W


def _pack_cos_sin(cos, sin):
    """cosP/sin2P [128, S]: lo coeffs at rows 0:40, hi at 64:104, rest 0.

    sin2P row signs match rot = x*cosP + swap(x)*sin2P: lo rows hold
    -sin_lo (they multiply x_hi), hi rows hold +sin_hi (they multiply x_lo).
    """
    S = cos.shape[0]
    cosP = np.zeros((128, S), np.float32)
    sinP = np.zeros((128, S), np.float32)
    cosP[0:BLK] = cos.T[0:BLK]
    cosP[64:64 + BLK] = cos.T[BLK:HD]
    sinP[0:BLK] = -sin.T[0:BLK]
    sinP[64:64 + BLK] = sin.T[BLK:HD]
    return cosP, sinP


_CACHE = {}

# ---------------------------------------------------------------------------
# v3 fast path (uniform 4x1024 segments): 2 head-groups x 4 segments SPMD.
#
# Per core: 8 heads, 1024 tokens, one segment. All on-chip data bf16 except
# PSUM (f32) and the normalization scalars (f32).
#   - q/k packed dense: 32 40-row halves, 3 per 128-row tile (11 j-tiles)
#   - v computed untransposed ([tokens, vdim]) straight into PV operand slots
#   - RoPE: DMA-stage [lo;hi]/[hi;lo] into dense [0:80] layouts, 3 DVE ops
#     at bf16 2x rate; scores contract K=80 (no zero padding rows)
#   - scores^T per (head, key-tile) into [128,1024] PSUM, one wide exp
#   - PV with ones column at slot col 96 -> denominators at PSUM row 96
#   - projection over 5 dense 128-row K-tiles (attn heads re-packed via DMA)
# ---------------------------------------------------------------------------

V3_S = 1024     # tokens per core
V3_NH = 8       # heads per core
V3_NJ = 11      # dense qk j-tiles (3 x 40-row halves each, 8 junk rows)
V3_NK = 10      # contraction tiles (D / 128)
V3_NT = 8       # token/key tiles (S / 128)
V3_VW = 97      # v slot: 80 v dims + 16 zero pad + ones col at 96
V3_NPJ = 5      # dense proj k-tiles (8 heads * 80 / 128)


def _half_pos(m):
    """Packed position of 40-row half m: (j_tile, row in {0, 40, 80})."""
    return m // 3, 40 * (m % 3)


def _build_v3():
    import concourse.mybir as mybir
    import concourse.tile as tile
    from concourse import bacc
    from contextlib import ExitStack

    f32 = mybir.dt.float32
    bf16 = mybir.dt.bfloat16
    AF = mybir.ActivationFunctionType
    S, NH, NJ, NK, NT, VW = V3_S, V3_NH, V3_NJ, V3_NK, V3_NT, V3_VW

    nc = bacc.Bacc("TRN2", target_bir_lowering=False, debug=False,
                   enable_asserts=False, num_devices=N_CORES)

    hidT = nc.dram_tensor("hidT", [128, NK * S], bf16,
                          kind="ExternalInput").ap()
    wqkT = nc.dram_tensor("wqkT", [128, NJ * NK * 128], bf16,
                          kind="ExternalInput").ap()
    bias2d = nc.dram_tensor("bias2d", [128, NJ], f32,
                            kind="ExternalInput").ap()
    wvT = nc.dram_tensor("wvT", [128, NK * 640], bf16,
                         kind="ExternalInput").ap()
    vtmpl = nc.dram_tensor("vtmpl", [128, NH * VW], bf16,
                           kind="ExternalInput").ap()
    cosP = nc.dram_tensor("cosP", [128, S], bf16, kind="ExternalInput").ap()
    sinP = nc.dram_tensor("sinP", [128, S], bf16, kind="ExternalInput").ap()
    wprojT = nc.dram_tensor("wprojT", [128, V3_NPJ * D], bf16,
                            kind="ExternalInput").ap()
    outT = nc.dram_tensor("outT", [D, S], bf16, kind="ExternalOutput").ap()
    if KERNEL_DEBUG:
        dbg_qk = nc.dram_tensor("dbg_qk", [128, NJ * S], bf16,
                                kind="ExternalOutput").ap()
        dbg_rot = nc.dram_tensor("dbg_rot", [128, 16 * S], bf16,
                                 kind="ExternalOutput").ap()
        dbg_vaug = nc.dram_tensor("dbg_vaug", [128, NT * NH * VW], bf16,
                                  kind="ExternalOutput").ap()
        dbg_attn = nc.dram_tensor("dbg_attn", [128, NH * S], bf16,
                                  kind="ExternalOutput").ap()

    hid3 = hidT.rearrange("p (k s) -> p k s", k=NK)
    wqk4 = wqkT.rearrange("p (j k c) -> p j k c", j=NJ, k=NK)
    wv3 = wvT.rearrange("p (k c) -> p k c", k=NK)

    with tile.TileContext(nc) as tc, ExitStack() as ctx:
        persist = ctx.enter_context(tc.tile_pool(name="persist", bufs=1))
        bias_sb = persist.tile([128, NJ], f32, tag="bias", name="bias")
        cos_sb = persist.tile([128, S], bf16, tag="cos", name="cos")
        sin_sb = persist.tile([128, S], bf16, tag="sin", name="sin")
        vt_sb = persist.tile([128, NH * VW], bf16, tag="vt", name="vt")
        pass

        psum = ctx.enter_context(tc.tile_pool(name="psum", bufs=1,
                                              space="PSUM"))
        unit = [0]

        def qs_tile():
            t = psum.tile([128, 1024], f32, tag=f"qs{unit[0] % 2}", name="qs")
            unit[0] += 1
            return t

        qk_pool = ctx.enter_context(tc.tile_pool(name="qk", bufs=1))
        qk_sb = qk_pool.tile([128, NJ * S], bf16, tag="qk", name="qk")
        rot_pool = ctx.enter_context(tc.tile_pool(name="rotp", bufs=1))
        rot = rot_pool.tile([128, 16 * S], bf16, tag="rot", name="rot")
        stgb_pool = ctx.enter_context(tc.tile_pool(name="stgb", bufs=6))
        vaug_pool = ctx.enter_context(tc.tile_pool(name="vaug", bufs=1))
        vaug = vaug_pool.tile([128, NT * NH * VW], bf16, tag="va", name="va")
        vaug3 = vaug.rearrange("p (t h c) -> p t h c", t=NT, h=NH)
        attn_pool = ctx.enter_context(tc.tile_pool(name="attn", bufs=1))
        # unnormalized PV output incl. bf16 denominator row at partition 96;
        # normalized in place at the end
        attn = attn_pool.tile([128, NH * S], bf16, tag="at", name="at")
        pt_pool = ctx.enter_context(tc.tile_pool(name="pt", bufs=12))
        rc_pool = ctx.enter_context(tc.tile_pool(name="rc", bufs=2))
        rcf_pool = ctx.enter_context(tc.tile_pool(name="rcf", bufs=2))
        bc_pool = ctx.enter_context(tc.tile_pool(name="bc", bufs=2))
        wp_pool = ctx.enter_context(tc.tile_pool(name="wp", bufs=1))
        wp_sb = wp_pool.tile([128, V3_NPJ * D], bf16, tag="wp", name="wp")

        # weights/hidden (dead after phase 1; proj pools reuse the space) —
        # entered last among open pools so the mid-program release is LIFO
        ph1 = tc.tile_pool(name="ph1", bufs=1)
        p1 = ph1.__enter__()
        hid_sb = p1.tile([128, NK * S], bf16, tag="hid", name="hid")
        wqk_sb = p1.tile([128, NJ * NK * 128], bf16, tag="wqk", name="wqk")
        wv_sb = p1.tile([128, NK * 640], bf16, tag="wv", name="wv")
        hid3s = hid_sb.rearrange("p (k s) -> p k s", k=NK)
        wqk4s = wqk_sb.rearrange("p (j k c) -> p j k c", j=NJ, k=NK)
        wv3s = wv_sb.rearrange("p (k c) -> p k c", k=NK)
        # PE warm-up: the cost model prices p-state at dispatch; keep the
        # PE busy on junk matmuls while the input loads stream so the real
        # matmuls dispatch against a warm (2.4 GHz) clock
        wa = p1.tile([128, 16], bf16, tag="wa", name="wa")
        wb = p1.tile([128, 256], bf16, tag="wb", name="wb")
        nc.vector.memset(wa[:, :], 0.0)
        nc.vector.memset(wb[:, :], 0.0)
        wps = psum.tile([128, 1024], f32, tag="po", name="warm")
        for _ in range(30):
            nc.tensor.matmul(wps[0:16, 0:256], wa[:, :], wb[:, :],
                             start=True, stop=True)
        nc.vector.tensor_copy(wa[0:16, 0:4], wps[0:16, 0:4])

        # loads on one queue in priority order; the tail weight loads are
        # deferred into the round loop (just-in-time) so their transfers
        # never sit ahead of the rope staging DMAs on the serial DMA device
        nc.sync.dma_start(vt_sb[:], vtmpl[:])
        nc.sync.dma_start(bias_sb[:], bias2d[:])
        nc.sync.dma_start(wqk4s[:, 0:3, :, :], wqk4[:, 0:3, :, :])
        nc.sync.dma_start(hid3s[:, 0:5, :], hid3[:, 0:5, :])
        nc.sync.dma_start(hid3s[:, 5:NK, :], hid3[:, 5:NK, :])
        nc.sync.dma_start(cos_sb[:], cosP[:])
        nc.sync.dma_start(sin_sb[:], sinP[:])
        # JIT loads must be EMITTED before their first consumer (emission
        # order defines dependency direction), while issuing late enough
        # that their transfers don't delay the rope staging DMAs
        def _load1():
            nc.sync.dma_start(wv3s[:, :, :], wv3[:, :, :])
            nc.sync.dma_start(wqk4s[:, 3:4, :, :], wqk4[:, 3:4, :, :])

        load_at = {
            1: _load1,
            2: lambda: nc.sync.dma_start(wqk4s[:, 4:6, :, :],
                                         wqk4[:, 4:6, :, :]),
            3: lambda: nc.sync.dma_start(wqk4s[:, 6:9, :, :],
                                         wqk4[:, 6:9, :, :]),
            4: lambda: nc.sync.dma_start(wqk4s[:, 9:NJ, :, :],
                                         wqk4[:, 9:NJ, :, :]),
            5: lambda: nc.sync.dma_start(wp_sb[:], wprojT[:]),
        }

        vt3 = vt_sb.rearrange("p (h c) -> p h c", h=NH)

        def emit_qk(j):
            ps = qs_tile()
            for half in range(2):
                c0 = half * 512
                for k in range(NK):
                    nc.tensor.matmul(ps[:, c0:c0 + 512],
                                     wqk4s[:, j, k, :],
                                     hid3s[:, k, c0:c0 + 512],
                                     start=(k == 0), stop=(k == NK - 1))
            if j < 5:
                # early copies on ACT (idle pre-exp): their rope staging
                # DMAs directly follow on the same queue, so the critical
                # startup rope chain never waits in a clogged FIFO
                nc.scalar.activation(qk_sb[:, j * S:(j + 1) * S], ps[:, :],
                                     AF.Identity, bias=bias_sb[:, j:j + 1])
            else:
                nc.vector.tensor_scalar_add(qk_sb[:, j * S:(j + 1) * S],
                                            ps[:, :], bias_sb[:, j:j + 1])

        def emit_v(tt):
            ps = qs_tile()
            for (c0, w) in ((0, 512), (512, 128)):
                for k in range(NK):
                    nc.tensor.matmul(ps[:, c0:c0 + w],
                                     hid3s[:, k, tt * 128:(tt + 1) * 128],
                                     wv3s[:, k, c0:c0 + w],
                                     start=(k == 0), stop=(k == NK - 1))
            src = ps[:, 0:NH * HD].rearrange("p (h c) -> p h c", c=HD)
            nc.vector.tensor_add(vaug3[:, tt, :, 0:HD], src[:, :, :],
                                 vt3[:, :, 0:HD])
            # pad + ones columns (disjoint from the v region written above)
            nc.vector.tensor_copy(vaug3[:, tt, :, HD:VW], vt3[:, :, HD:VW])

        qk3 = qk_sb.rearrange("p (j s) -> p j s", j=NJ)
        stgb_tiles = {}

        def emit_rope_dma(p):
            lo_t, lo_r = _half_pos(2 * p)
            hi_t, hi_r = _half_pos(2 * p + 1)
            b0 = p * S
            if p <= 6:
                eng = nc.scalar
            elif p >= 13:
                eng = nc.gpsimd
            else:
                eng = [nc.gpsimd, nc.sync][p % 2]
            if lo_t == hi_t and hi_r == lo_r + 40:
                eng.dma_start(rot[0:80, b0:b0 + S],
                              qk3[lo_r:lo_r + 80, lo_t, :])
            else:
                eng.dma_start(rot[0:40, b0:b0 + S],
                              qk3[lo_r:lo_r + 40, lo_t, :])
                eng.dma_start(rot[40:80, b0:b0 + S],
                              qk3[hi_r:hi_r + 40, hi_t, :])
            sb = stgb_pool.tile([128, S], bf16, tag="sb", name="sb")
            stgb_tiles[p] = sb
            eng.dma_start(sb[0:40, :], qk3[hi_r:hi_r + 40, hi_t, :])
            eng.dma_start(sb[40:80, :], qk3[lo_r:lo_r + 40, lo_t, :])

        def emit_rope_mul(p):
            b0 = p * S
            sb = stgb_tiles.pop(p)
            nc.vector.tensor_mul(rot[0:80, b0:b0 + S], rot[0:80, b0:b0 + S],
                                 cos_sb[0:80, :])
            meng = nc.gpsimd if p % 4 == 3 else nc.vector
            meng.tensor_mul(sb[0:80, :], sb[0:80, :], sin_sb[0:80, :])
            nc.vector.tensor_add(rot[0:80, b0:b0 + S], rot[0:80, b0:b0 + S],
                                 sb[0:80, :])

        # Attention pump. 512-wide score half-units (unit = (kt, half),
        # 16 per head) on dedicated 1-bank PSUM slots decouple the exp
        # stream from the qk/v slot rotation. Invariant: every emitted
        # instruction's dependencies (incl. slot predecessors) are emitted
        # before it — PVs of head h follow head h-1's evacuation (single po
        # slot), exps run ahead of PVs by at most PT_AHEAD pt tiles.
        PT_AHEAD = 10
        heads_q = []      # started heads, in order
        v_done = [0]
        st_unit = [0]
        outstanding = [0]

        def start_attn(h):
            heads_q.append({"h": h, "se": 0, "pv": 0, "po": None})

        def emit_st_exp(hs):
            h, u = hs["h"], hs["se"]
            kt, half = u // 2, u % 2
            pq, pk = 2 * h, 2 * h + 1
            c0 = half * 512
            st = psum.tile([128, 512], f32, tag=f"st{st_unit[0] % 2}",
                           name="st")
            st_unit[0] += 1
            nc.tensor.matmul(
                st[:, :],
                rot[0:80, pk * S + kt * 128:pk * S + (kt + 1) * 128],
                rot[0:80, pq * S + c0:pq * S + c0 + 512],
                start=True, stop=True)
            pt = pt_pool.tile([128, 512], bf16, tag="pt", name="pt")
            nc.scalar.activation(pt[:, :], st[:, :], AF.Exp)
            hs.setdefault("pts", []).append(pt)
            hs["se"] += 1
            outstanding[0] += 1

        def emit_pv(hs):
            h, u = hs["h"], hs["pv"]
            kt, half = u // 2, u % 2
            if hs["po"] is None:
                hs["po"] = psum.tile([128, 1024], f32, tag="po", name="po")
            c0 = half * 512
            nc.tensor.matmul(hs["po"][0:VW, c0:c0 + 512],
                             vaug3[:, kt, h, :],
                             hs["pts"][u][:, :],
                             start=(kt == 0), stop=(kt == NT - 1))
            hs["pv"] += 1
            outstanding[0] -= 1

        def pump_attn():
            progress = True
            while progress:
                progress = False
                if heads_q:
                    hs = heads_q[0]
                    while (hs["pv"] < hs["se"]
                           and hs["pv"] // 2 < v_done[0]):
                        emit_pv(hs)
                        progress = True
                    if hs["pv"] == 2 * NT:
                        h = hs["h"]
                        # evacuate on ACT so the po slot frees immediately
                        # and normalize never touches compute FIFOs
                        nc.scalar.activation(attn[0:VW, h * S:(h + 1) * S],
                                             hs["po"][0:VW, :], AF.Identity)
                        if h <= 5:
                            emit_norm(h)
                        heads_q.pop(0)
                        progress = True
                        continue
                for hs in heads_q:
                    while (hs["se"] < 2 * NT
                           and outstanding[0] < PT_AHEAD):
                        emit_st_exp(hs)
                        progress = True

        def emit_norm(h):
            rc = rc_pool.tile([1, S], bf16, tag="rc", name="rc")
            nc.scalar.dma_start(rc[0:1, :], attn[96:97, h * S:(h + 1) * S])
            rcf = rcf_pool.tile([1, S], f32, tag="rcf", name="rcf")
            nc.vector.reciprocal(rcf[0:1, :], rc[0:1, :])
            bc = bc_pool.tile([80, S], f32, tag="bc", name="bc")
            nc.gpsimd.partition_broadcast(bc[0:80, :], rcf[0:1, :])
            # late heads' muls on Pool: DVE is busy with the output adds then
            meng = nc.gpsimd if h >= 6 else nc.vector
            meng.tensor_mul(attn[0:80, h * S:(h + 1) * S],
                            attn[0:80, h * S:(h + 1) * S], bc[0:80, :])

        # emission driver: qk j-tiles paced with v, rope, attention
        v_at = {1: (0, 1), 2: (2, 3), 3: (4, 5), 4: (6, 7)}
        rope_at = {}
        for p in range(16):
            jmax = max(_half_pos(2 * p)[0], _half_pos(2 * p + 1)[0])
            rope_at.setdefault(jmax, []).append(p)
        pending = []
        for j in range(NJ):
            if j in load_at:
                load_at[j]()
            emit_qk(j)
            # rope muls one round behind their staging DMAs so the DVE FIFO
            # never stalls on in-flight DMA latency
            for p in pending:
                emit_rope_mul(p)
                if p % 2 == 1:
                    start_attn(p // 2)
            pending = []
            for tt in v_at.get(j, ()):
                emit_v(tt)
                v_done[0] += 1
            pump_attn()
            for p in rope_at.get(j, ()):
                emit_rope_dma(p)
                pending.append(p)
        for p in pending:
            emit_rope_mul(p)
            if p % 2 == 1:
                start_attn(p // 2)
        pump_attn()

        if KERNEL_DEBUG:
            nc.sync.dma_start(dbg_qk[:, :], qk_sb[:, :])
            nc.sync.dma_start(dbg_rot[0:80, :], rot[0:80, :])
            nc.sync.dma_start(dbg_vaug[:, :], vaug[:, :])
            nc.sync.dma_start(dbg_attn[0:97, :], attn[0:97, :])

        ph1.__exit__(None, None, None)

        # normalize + dense re-pack of attn heads + split projection:
        # kt0-3 prepass overlaps the last heads' attention, kt4 finishes
        with ExitStack() as p5:
            late = p5.enter_context(tc.tile_pool(name="late", bufs=1))
            dense = late.tile([128, V3_NPJ * S], bf16, tag="dn", name="dn")
            ob_pool = p5.enter_context(tc.tile_pool(name="ob", bufs=3))
            oa_pool = p5.enter_context(tc.tile_pool(name="oa", bufs=1))
            dense3 = dense.rearrange("p (t s) -> p t s", t=V3_NPJ)
            wp3 = wp_sb.rearrange("p (t c) -> p t c", t=V3_NPJ)

            def emit_densify(h):
                for (dt, r, n, off) in _pieces(80 * h, 80):
                    nc.sync.dma_start(
                        dense3[r:r + n, dt, :],
                        attn[off:off + n, h * S:(h + 1) * S])

            emit_norm(NH - 2)
            for h in range(NH - 1):
                emit_densify(h)
            KA = 3   # prepass contracts kt0..KA-1 (early heads), final the rest
            oa_tiles = []
            for j in range(NK):
                ps = qs_tile()
                for half in range(2):
                    c0 = half * 512
                    for kt in range(KA):
                        nc.tensor.matmul(
                            ps[:, c0:c0 + 512],
                            wp3[:, kt, j * 128:(j + 1) * 128],
                            dense3[:, kt, c0:c0 + 512],
                            start=(kt == 0), stop=(kt == KA - 1))
                oa = oa_pool.tile([128, S], f32, tag=f"oa{j}", name=f"oa{j}",
                                  bufs=1)
                if j % 2:
                    nc.scalar.activation(oa[:, :], ps[:, :], AF.Identity)
                else:
                    nc.vector.tensor_copy(oa[:, :], ps[:, :])
                oa_tiles.append(oa)
            emit_norm(NH - 1)
            emit_densify(NH - 1)
            for j in range(NK):
                ps = qs_tile()
                for half in range(2):
                    c0 = half * 512
                    for kt in range(KA, V3_NPJ):
                        nc.tensor.matmul(ps[:, c0:c0 + 512],
                                         wp3[:, kt, j * 128:(j + 1) * 128],
                                         dense3[:, kt, c0:c0 + 512],
                                         start=(kt == KA),
                                         stop=(kt == V3_NPJ - 1))
                ob = ob_pool.tile([128, S], bf16, tag="ob", name="ob")
                nc.vector.tensor_add(ob[:, :], ps[:, :], oa_tiles[j][:, :])
                nc.sync.dma_start(outT[j * 128:(j + 1) * 128, :], ob[:, :])

    nc.compile()
    return nc


def _pack_v3(Wqkv, bqkv, Wproj, bproj, g):
    """Host-side per-head-group weight packing for the v3 program."""
    import concourse.mybir as mybir
    bf16 = mybir.dt.np(mybir.dt.bfloat16)
    NH, NJ, NK, VW = V3_NH, V3_NJ, V3_NK, V3_VW

    Wp = np.zeros((NJ * 128, D), np.float32)
    bp = np.zeros((NJ * 128,), np.float32)
    for m in range(32):
        h = m // 4
        sec = (m % 4) // 2       # 0 = q, 1 = k
        half = m % 2
        src = sec * D + (g * NH + h) * HD + half * BLK
        w = Wqkv[src:src + BLK, :]
        b = bqkv[src:src + BLK]
        if sec == 0:
            w = w * SCALE
            b = b * SCALE
        t, r = _half_pos(m)
        Wp[t * 128 + r:t * 128 + r + BLK] = w
        bp[t * 128 + r:t * 128 + r + BLK] = b
    # lhsT layout [128, j, k, 128]: wqkT[p, j, k, c] = Wp[j*128+c, k*128+p]
    wqkT = np.ascontiguousarray(
        Wp.reshape(NJ, 128, NK, 128).transpose(3, 0, 2, 1)
        .reshape(128, NJ * NK * 128)).astype(bf16)
    bias2d = np.ascontiguousarray(bp.reshape(NJ, 128).T)

    Wv = Wqkv[2 * D + g * 640:2 * D + (g + 1) * 640, :]
    wvT = _tile_rows(np.ascontiguousarray(Wv.T)).astype(bf16)
    bv = bqkv[2 * D + g * 640:2 * D + (g + 1) * 640]

    vt = np.zeros((128, NH * VW), np.float32)
    for h in range(NH):
        vt[:, h * VW:h * VW + HD] = bv[h * HD:(h + 1) * HD][None, :]
        vt[:, h * VW + 96] = 1.0
    vtmpl = vt.astype(bf16)

    Wpd = Wproj[:, g * 640:(g + 1) * 640].T  # [640, 1280] dense attn rows
    wprojT = _tile_rows(np.ascontiguousarray(Wpd)).astype(bf16)
    return wqkT, bias2d, wvT, vtmpl, wprojT


def _cos_sin_v3(cos, sin):
    """Dense [0:80] rope coefficient layouts (bf16), full sequence."""
    import concourse.mybir as mybir
    bf16 = mybir.dt.np(mybir.dt.bfloat16)
    S = cos.shape[0]
    cp = np.zeros((128, S), np.float32)
    sp = np.zeros((128, S), np.float32)
    cp[0:BLK] = cos.T[0:BLK]
    cp[BLK:HD] = cos.T[BLK:HD]
    sp[0:BLK] = -sin.T[0:BLK]
    sp[BLK:HD] = sin.T[BLK:HD]
    return cp.astype(bf16), sp.astype(bf16)


def kernel(hidden_states, cos, sin, Wqkv, bqkv, Wproj, bproj, cu_seqlens):
    sys.path.insert(0, "/opt/trn_rl_repo")
    from concourse import bass_utils

    hidden_states = np.asarray(hidden_states, np.float32)
    cos = np.asarray(cos, np.float32)
    sin = np.asarray(sin, np.float32)
    Wqkv = np.asarray(Wqkv, np.float32)
    bqkv = np.asarray(bqkv, np.float32)
    Wproj = np.asarray(Wproj, np.float32)
    bproj = np.asarray(bproj, np.float32)

    S, D_ = hidden_states.shape
    assert D_ == D
    segs = _segments(cu_seqlens, S)
    uniform = (S % 4 == 0) and segs == [(i * S // 4, (i + 1) * S // 4)
                                        for i in range(4)]

    hiddenT = np.ascontiguousarray(hidden_states.T)
    cosP, sin2P = _pack_cos_sin(cos, sin)

    def _vinit(segs_local):
        n_tt = sum(-(-(e - a) // 128) for a, e in segs_local)
        v = np.zeros((128, n_tt, 17), np.float32)
        v[:, :, 16] = 1.0
        return np.ascontiguousarray(v.reshape(128, n_tt * 17))

    if uniform:
        # v3: 2 head-groups x 4 segments, bf16 on-chip
        import concourse.mybir as mybir
        bf16 = mybir.dt.np(mybir.dt.bfloat16)
        S_core = S // 4
        key = ("V3", S)
        if key not in _CACHE:
            _CACHE[key] = _build_v3()
        nc = _CACHE[key]
        cosPd, sinPd = _cos_sin_v3(cos, sin)
        hidT_b = hiddenT.astype(bf16)
        in_maps = []
        meta = []
        for g in range(2):
            wqkT, b2, wvT, vtmpl, wprojT = _pack_v3(Wqkv, bqkv, Wproj,
                                                    bproj, g)
            for s in range(4):
                sl = slice(s * S_core, (s + 1) * S_core)
                in_maps.append({
                    "hidT": _tile_rows(hidT_b[:, sl]),
                    "wqkT": wqkT,
                    "bias2d": b2,
                    "wvT": wvT,
                    "vtmpl": vtmpl,
                    "cosP": np.ascontiguousarray(cosPd[:, sl]),
                    "sinP": np.ascontiguousarray(sinPd[:, sl]),
                    "wprojT": wprojT,
                })
                meta.append((g, s))
        res = bass_utils.run_bass_kernel_spmd(nc, in_maps,
                                              core_ids=list(range(N_CORES)))
        out = np.zeros((D, S), np.float32)
        for c, (g, s) in enumerate(meta):
            out[:, s * S_core:(s + 1) * S_core] += \
                res.results[c]["outT"].astype(np.float32)
    else:
        # mode C: 8-way head parallel, full sequence per core
        n_h, S_core = H // N_CORES, S
        key = ("C", S, tuple(np.asarray(cu_seqlens).tolist()))
        if key not in _CACHE:
            _CACHE[key] = _build_program(n_h, S_core, segs,
                                         resident_hidden=False)
        nc = _CACHE[key]
        vinit = _vinit(segs)
        hid_tiled = _tile_rows(hiddenT)
        in_maps = []
        for c in range(N_CORES):
            heads = list(range(c * n_h, (c + 1) * n_h))
            wt, b2 = _pack_w(Wqkv, bqkv, heads, n_h)
            in_maps.append({
                "hiddenT": hid_tiled,
                "wqkvT": wt,
                "bias2d": b2,
                "cosP": cosP,
                "sin2P": sin2P,
                "wprojT": _pack_wproj(Wproj, heads),
                "vinit": vinit,
            })
        res = bass_utils.run_bass_kernel_spmd(nc, in_maps,
                                              core_ids=list(range(N_CORES)))
        out = np.zeros((D, S), np.float32)
        for c in range(N_CORES):
            out += res.results[c]["outT"]

    return np.ascontiguousarray(out.T) + bproj[None, :]

